# revision 8
# baseline (speedup 1.0000x reference)
"""Distributed GQA attention kernel for one TRN2 chip (8 NeuronCores).

Sharding: tensor-parallel over heads. Core g owns query heads [4g, 4g+4)
and kv head g. Each core computes its heads' attention and a partial
output projection; a chunked ReduceScatter sums the partials and leaves
each core with a 1/8 token-slice of the final output.

All device tensors are laid out so that no on-device transposes of the
big activations are needed:
  - x is passed pre-transposed (xT [D, B*L]) so projections contract D
    on the partition axis.
  - q/k are produced directly as qT/kT [head_dim, tokens]; scores are
    computed keys-on-partitions, so the P@V matmul consumes exp(scores)
    directly and the wo matmul consumes the attention output directly.
  - RoPE head_dim pairs are permuted (on the host, into wq/wk rows) so
    each rotation partner lives 16 partitions away within a 32-partition
    quadrant -> one DVE stream_shuffle does the swap.
  - softmax denominator comes from an all-ones matmul (partition
    broadcast for free); no max subtraction (fp32 logits here are <~15).
"""

import numpy as np

import concourse.bass as bass
import concourse.mybir as mybir
import concourse.tile as tile
from concourse import bacc
from concourse.alu_op_type import AluOpType
from concourse.masks import make_identity

F32 = mybir.dt.float32

N_CORES = 8
NHL = 4           # local q heads per core
HD = 128          # head dim
THETA = 10000.0
SCALE = HD ** -0.5
TW = 512          # token block width (free dim of most matmuls)
KW = 128          # key tile width (partition dim of score tiles)

# module-level knobs for test.py
TRACE = False
LAST_RESULTS = None


class Cfg:
    def __init__(self, B=2, L=2048, D=4096):
        self.B, self.L, self.D = B, L, D
        self.BL = B * L
        self.DC = D // 128         # contraction chunks for projections
        self.NB = L // TW          # query blocks per batch
        self.NT = self.BL // TW    # token blocks total
        self.KT = L // KW          # key tiles per batch
        self.NBLK = D // TW        # wo output column blocks
        self.NCH = self.NT         # ReduceScatter chunks (one per token block)
        assert self.BL % TW == 0 and TW % N_CORES == 0


# stream_shuffle mask: swap 16-partition halves within each 32-partition quadrant
SWAP16 = [(i + 16) % 32 for i in range(32)]


def _rope_perm():
    """Permutation of head_dim rows: pair i=(16q + r) lives at partitions
    32q+r (x1 = even dim 2i) and 32q+16+r (x2 = odd dim 2i+1)."""
    perm = np.zeros(HD, dtype=np.int64)
    for p in range(HD):
        q, r = divmod(p, 32)
        i = 16 * q + (r % 16)
        perm[p] = 2 * i + (0 if r < 16 else 1)
    return perm


def _rope_tables(cfg):
    """cosT/sinT [128, L] in the permuted-partition layout, sin sign-folded."""
    perm = _rope_perm()
    t = np.arange(cfg.L, dtype=np.float64)
    freqs = THETA ** (-np.arange(0, HD, 2, dtype=np.float64) / HD)  # [64]
    theta = t[None, :] * freqs[:, None]                             # [64, L]
    cos, sin = np.cos(theta), np.sin(theta)
    C = np.zeros((HD, cfg.L), dtype=np.float32)
    S = np.zeros((HD, cfg.L), dtype=np.float32)
    for p in range(HD):
        q, r = divmod(p, 32)
        i = 16 * q + (r % 16)
        C[p] = cos[i]
        S[p] = sin[i] if r >= 16 else -sin[i]
    return C, S


def classify_mask(mask, cfg):
    """cls[kt][qb] in {'Z','N','M'} for tile mask[qb*TW:(qb+1)*TW, kt*KW:(kt+1)*KW]."""
    cls = [[None] * cfg.NB for _ in range(cfg.KT)]
    for kt in range(cfg.KT):
        for qb in range(cfg.NB):
            t = mask[qb * TW:(qb + 1) * TW, kt * KW:(kt + 1) * KW]
            if np.all(t == 0.0):
                cls[kt][qb] = 'Z'
            elif np.all(t <= -1e8):
                cls[kt][qb] = 'N'
            else:
                cls[kt][qb] = 'M'
    # guard: every query block must attend to at least one key tile
    for qb in range(cfg.NB):
        assert any(cls[kt][qb] != 'N' for kt in range(cfg.KT)), \
            "fully-masked query block unsupported"
    return cls


def build_bass(cfg, cls):
    nc = bacc.Bacc("TRN2", target_bir_lowering=False, debug=False,
                   num_devices=N_CORES)

    xT_d = nc.dram_tensor("xT", [cfg.D, cfg.BL], F32, kind="ExternalInput")
    wqT_d = nc.dram_tensor("wqT", [cfg.D, NHL * HD], F32, kind="ExternalInput")
    wkT_d = nc.dram_tensor("wkT", [cfg.D, HD], F32, kind="ExternalInput")
    wvT_d = nc.dram_tensor("wvT", [cfg.D, HD], F32, kind="ExternalInput")
    woT_d = nc.dram_tensor("woT", [NHL * HD, cfg.D], F32, kind="ExternalInput")
    maskT_d = nc.dram_tensor("maskT", [cfg.L, cfg.L], F32, kind="ExternalInput")
    ropeC_d = nc.dram_tensor("ropeC", [HD, cfg.L], F32, kind="ExternalInput")
    ropeS_d = nc.dram_tensor("ropeS", [HD, cfg.L], F32, kind="ExternalInput")
    out_d = nc.dram_tensor("out", [cfg.BL // N_CORES, cfg.D], F32,
                           kind="ExternalOutput")

    rg = [list(range(N_CORES))]
    QD = NHL * HD  # 512

    with tile.TileContext(nc) as tc:
        # ---- constants / tables -------------------------------------------
        const_pool = tc.alloc_tile_pool(name="const", bufs=1)
        ones_sb = const_pool.tile([128, 128], F32, name="ones_sb")
        nc.vector.memset(ones_sb[:], 1.0)
        ident = const_pool.tile([128, 128], F32, name="ident")
        make_identity(nc, ident[:])
        ropeC = const_pool.tile([HD, cfg.L], F32, name="ropeC_sb")
        ropeS = const_pool.tile([HD, cfg.L], F32, name="ropeS_sb")
        nc.sync.dma_start(out=ropeC[:], in_=ropeC_d.ap())
        nc.sync.dma_start(out=ropeS[:], in_=ropeS_d.ap())

        # ---- resident activations -----------------------------------------
        kv_pool = tc.alloc_tile_pool(name="kv", bufs=1)
        kT_sb = kv_pool.tile([HD, cfg.BL], F32, name="kT_sb")
        v_sb = kv_pool.tile([128, cfg.BL], F32, name="v_sb")

        # DRAM scratch
        dram_pool = tc.alloc_tile_pool(name="dram", bufs=1, space="DRAM")
        qT_dram = dram_pool.tile([QD, cfg.BL], F32, name="qT_dram")
        rs_in = [dram_pool.tile([TW, cfg.D], F32, name=f"rs_in{c}")
                 for c in range(cfg.NCH)]
        rs_out = [dram_pool.tile([TW // N_CORES, cfg.D], F32,
                                 name=f"rs_out{c}")
                  for c in range(cfg.NCH)]

        # ================= phase 1: QKV projections + RoPE =================
        with tc.tile_pool(name="wqkv", bufs=1) as w_pool, \
             tc.tile_pool(name="xload", bufs=4) as x_pool, \
             tc.tile_pool(name="qkpsum", bufs=1, space="PSUM") as qk_psum, \
             tc.tile_pool(name="ropetmp", bufs=3) as rtmp_pool, \
             tc.tile_pool(name="qrot", bufs=3) as qrot_pool, \
             tc.tile_pool(name="vstage", bufs=2) as vst_pool, \
             tc.tile_pool(name="vtpsum", bufs=2, space="PSUM") as vt_psum:

            wq_sb = w_pool.tile([128, cfg.DC * QD], F32, name="wq_sb")
            wk_sb = w_pool.tile([128, cfg.DC * HD], F32, name="wk_sb")
            wv_sb = w_pool.tile([128, cfg.DC * HD], F32, name="wv_sb")
            for dc in range(cfg.DC):
                nc.sync.dma_start(out=wq_sb[:, dc * QD:(dc + 1) * QD],
                                  in_=wqT_d.ap()[dc * 128:(dc + 1) * 128, :])
                nc.sync.dma_start(out=wk_sb[:, dc * HD:(dc + 1) * HD],
                                  in_=wkT_d.ap()[dc * 128:(dc + 1) * 128, :])
                nc.sync.dma_start(out=wv_sb[:, dc * HD:(dc + 1) * HD],
                                  in_=wvT_d.ap()[dc * 128:(dc + 1) * 128, :])

            def rope_drain(ps, dst):
                """dst = ps*C + shuffle16(ps)*S at token offset t0 (len TW)."""
                sw = rtmp_pool.tile([128, TW], F32, name="rope_sw")
                t1 = rtmp_pool.tile([128, TW], F32, name="rope_t1")
                t2 = rtmp_pool.tile([128, TW], F32, name="rope_t2")
                nc.vector.stream_shuffle(sw[:], ps, SWAP16)
                nc.vector.tensor_tensor(t1[:], sw[:], Sx, AluOpType.mult)
                nc.vector.tensor_tensor(t2[:], ps, Cx, AluOpType.mult)
                nc.vector.tensor_tensor(dst, t1[:], t2[:], AluOpType.add)

            for tb in range(cfg.NT):
                t0 = (tb % cfg.NB) * TW  # position within batch
                Cx = ropeC[:, t0:t0 + TW]
                Sx = ropeS[:, t0:t0 + TW]

                q_ps = qk_psum.tile([128, NHL * TW], F32, name="q_ps")
                k_ps = qk_psum.tile([128, TW], F32, name="k_ps")
                vT_ps = qk_psum.tile([128, TW], F32, name="vT_ps")
                for dc in range(cfg.DC):
                    xt = x_pool.tile([128, TW], F32, name="x_t")
                    nc.sync.dma_start(
                        out=xt[:],
                        in_=xT_d.ap()[dc * 128:(dc + 1) * 128,
                                      tb * TW:(tb + 1) * TW])
                    st = dict(start=(dc == 0), stop=(dc == cfg.DC - 1))
                    for h in range(NHL):
                        nc.tensor.matmul(
                            q_ps[:, h * TW:h * TW + TW],
                            wq_sb[:, dc * QD + h * HD: dc * QD + (h + 1) * HD],
                            xt[:], **st)
                    nc.tensor.matmul(k_ps[:],
                                     wk_sb[:, dc * HD:(dc + 1) * HD],
                                     xt[:], **st)
                    nc.tensor.matmul(vT_ps[:],
                                     wv_sb[:, dc * HD:(dc + 1) * HD],
                                     xt[:], **st)

                # q: rope -> spill to DRAM
                for h in range(NHL):
                    qr = qrot_pool.tile([128, TW], F32, name="q_rot")
                    rope_drain(q_ps[:, h * TW:h * TW + TW], qr[:])
                    nc.sync.dma_start(
                        out=qT_dram[h * HD:(h + 1) * HD,
                                    tb * TW:(tb + 1) * TW],
                        in_=qr[:])
                # k: rope -> resident
                rope_drain(k_ps[:], kT_sb[:, tb * TW:(tb + 1) * TW])
                # v: vT -> transpose -> resident [ktok, hd] blocks
                vt_sb = vst_pool.tile([128, TW], F32, name="vT_stage")
                nc.scalar.copy(vt_sb[:], vT_ps[:])
                for i in range(TW // 128):
                    vp = vt_psum.tile([128, 128], F32, name="v_tr_ps")
                    nc.tensor.transpose(vp[:], vt_sb[:, i * 128:(i + 1) * 128],
                                        ident[:])
                    nc.scalar.copy(
                        v_sb[:, tb * TW + i * 128: tb * TW + (i + 1) * 128],
                        vp[:])

        # ================= phase 2: attention + wo + ReduceScatter =========
        with tc.tile_pool(name="wo", bufs=1) as wo_pool, \
             tc.tile_pool(name="mask", bufs=max(cfg.KT, 4)) as m_pool, \
             tc.tile_pool(name="qload", bufs=3) as q_pool, \
             tc.tile_pool(name="expsb", bufs=4) as e_pool, \
             tc.tile_pool(name="msum", bufs=3) as msk_pool, \
             tc.tile_pool(name="attnsb", bufs=2) as at_pool, \
             tc.tile_pool(name="recsb", bufs=2) as rec_pool, \
             tc.tile_pool(name="outcp", bufs=4) as oc_pool, \
             tc.tile_pool(name="scps", bufs=2, space="PSUM") as sc_psum, \
             tc.tile_pool(name="avps", bufs=2, space="PSUM") as av_psum, \
             tc.tile_pool(name="seps", bufs=2, space="PSUM") as se_psum, \
             tc.tile_pool(name="ops", bufs=2, space="PSUM") as o_psum:

            wo_sb = wo_pool.tile([128, NHL * cfg.D], F32, name="wo_sb")
            for h in range(NHL):
                nc.sync.dma_start(out=wo_sb[:, h * cfg.D:(h + 1) * cfg.D],
                                  in_=woT_d.ap()[h * HD:(h + 1) * HD, :])

            for qb in range(cfg.NB):
                active = [kt for kt in range(cfg.KT) if cls[kt][qb] != 'N']
                mtiles = {}
                for kt in active:
                    if cls[kt][qb] == 'M':
                        mt = m_pool.tile([KW, TW], F32, name="m_t")
                        nc.sync.dma_start(
                            out=mt[:],
                            in_=maskT_d.ap()[kt * KW:(kt + 1) * KW,
                                             qb * TW:(qb + 1) * TW])
                        mtiles[kt] = mt

                for b in range(cfg.B):
                    attn_sb = at_pool.tile([128, NHL * TW], F32, name="at_sb")
                    for h in range(NHL):
                        qt = q_pool.tile([HD, TW], F32, name="q_t")
                        nc.sync.dma_start(
                            out=qt[:],
                            in_=qT_dram[h * HD:(h + 1) * HD,
                                        (b * cfg.NB + qb) * TW:
                                        (b * cfg.NB + qb + 1) * TW])
                        at_ps = av_psum.tile([HD, TW], F32, name="at_ps")
                        se_ps = se_psum.tile([128, TW], F32, name="se_ps")
                        for idx, kt in enumerate(active):
                            gk = b * cfg.L + kt * KW  # global key token
                            sc_ps = sc_psum.tile([KW, TW], F32, name="sc_ps")
                            nc.tensor.matmul(sc_ps[:],
                                             kT_sb[:, gk:gk + KW],
                                             qt[:], start=True, stop=True)
                            if cls[kt][qb] == 'M':
                                ms = msk_pool.tile([KW, TW], F32, name="ms_t")
                                nc.vector.tensor_tensor(
                                    ms[:], sc_ps[:], mtiles[kt][:],
                                    AluOpType.add)
                                esrc = ms[:]
                            else:
                                esrc = sc_ps[:]
                            ex = e_pool.tile([KW, TW], F32, name="ex_t")
                            nc.scalar.activation(
                                ex[:], esrc,
                                mybir.ActivationFunctionType.Exp,
                                scale=float(SCALE))
                            st = dict(start=(idx == 0),
                                      stop=(idx == len(active) - 1))
                            nc.tensor.matmul(at_ps[:], v_sb[:, gk:gk + KW],
                                             ex[:], **st)
                            nc.tensor.matmul(se_ps[:], ones_sb[:], ex[:], **st)
                        rec = rec_pool.tile([128, TW], F32, name="rec_t")
                        nc.vector.reciprocal(rec[:], se_ps[:])
                        nc.vector.tensor_tensor(
                            attn_sb[:, h * TW:(h + 1) * TW],
                            at_ps[:], rec[:], AluOpType.mult)

                    # ---- wo partial for this (b, qb) token block ----------
                    c = b * cfg.NB + qb
                    for m in range(TW // 128):
                        for n in range(cfg.NBLK):
                            o_ps = o_psum.tile([128, TW], F32, name="o_ps")
                            for h in range(NHL):
                                nc.tensor.matmul(
                                    o_ps[:],
                                    attn_sb[:, h * TW + m * 128:
                                            h * TW + (m + 1) * 128],
                                    wo_sb[:, h * cfg.D + n * TW:
                                          h * cfg.D + (n + 1) * TW],
                                    start=(h == 0), stop=(h == NHL - 1))
                            oc = oc_pool.tile([128, TW], F32, name="oc_t")
                            nc.scalar.copy(oc[:], o_ps[:])
                            nc.sync.dma_start(
                                out=rs_in[c][m * 128:(m + 1) * 128,
                                             n * TW:(n + 1) * TW],
                                in_=oc[:])
                    nc.gpsimd.collective_compute(
                        "ReduceScatter", AluOpType.add, replica_groups=rg,
                        ins=[rs_in[c][:].opt()], outs=[rs_out[c][:].opt()])
                    rw = TW // N_CORES
                    nc.sync.dma_start(out=out_d.ap()[c * rw:(c + 1) * rw, :],
                                      in_=rs_out[c][:])

        dram_pool.release()
        kv_pool.release()
        const_pool.release()

    nc.compile()
    return nc


def host_prepare(cfg, x, mask, wq, wk, wv, wo):
    """Returns (in_maps, cls)."""
    x = np.ascontiguousarray(np.asarray(x, dtype=np.float32))
    mask = np.asarray(mask, dtype=np.float32)
    wq = np.asarray(wq, dtype=np.float32)
    wk = np.asarray(wk, dtype=np.float32)
    wv = np.asarray(wv, dtype=np.float32)
    wo = np.asarray(wo, dtype=np.float32)

    perm = _rope_perm()
    C, S = _rope_tables(cfg)
    xT = np.ascontiguousarray(x.reshape(cfg.BL, cfg.D).T)
    maskT = np.ascontiguousarray(mask.T / SCALE)
    cls = classify_mask(mask, cfg)

    in_maps = []
    for g in range(N_CORES):
        qrows = wq[g * NHL * HD:(g + 1) * NHL * HD]          # [512, D]
        qperm = np.concatenate(
            [qrows[h * HD + perm] for h in range(NHL)], axis=0)
        krows = wk[g * HD:(g + 1) * HD][perm]                # [128, D]
        vrows = wv[g * HD:(g + 1) * HD]                      # [128, D]
        wocols = wo[:, g * NHL * HD:(g + 1) * NHL * HD]      # [D, 512]
        in_maps.append({
            "xT": xT,
            "wqT": np.ascontiguousarray(qperm.T),
            "wkT": np.ascontiguousarray(krows.T),
            "wvT": np.ascontiguousarray(vrows.T),
            "woT": np.ascontiguousarray(wocols.T),
            "maskT": maskT,
            "ropeC": C,
            "ropeS": S,
        })
    return in_maps, cls


def assemble_output(cfg, results):
    """Stitch per-core ReduceScatter shards back into [B, L, D]."""
    full = np.empty((cfg.BL, cfg.D), dtype=np.float32)
    rw = TW // N_CORES
    for g in range(N_CORES):
        r = results[g]["out"]
        for c in range(cfg.NCH):
            full[c * TW + g * rw: c * TW + (g + 1) * rw] = \
                r[c * rw:(c + 1) * rw]
    return full.reshape(cfg.B, cfg.L, cfg.D)


def kernel(x, mask, wq, wk, wv, wo):
    global LAST_RESULTS
    from concourse.bass_utils import run_bass_kernel_spmd
    cfg = Cfg(B=2, L=2048, D=4096)
    in_maps, cls = host_prepare(cfg, x, mask, wq, wk, wv, wo)
    nc = build_bass(cfg, cls)
    res = run_bass_kernel_spmd(nc, in_maps, core_ids=list(range(N_CORES)),
                               trace=TRACE)
    LAST_RESULTS = res
    return assemble_output(cfg, res.results)


# revision 15
# speedup vs baseline: 2.7409x; 2.7409x over previous
"""Distributed GQA attention kernel for one TRN2 chip (8 NeuronCores).

Sharding: tensor-parallel over heads. Core g owns query heads [4g, 4g+4)
and kv head g. Each core computes its heads' attention and a partial
output projection; a chunked ReduceScatter sums the partials and leaves
each core with a 1/8 token-slice of the final output.

All device tensors are laid out so that no on-device transposes of the
big activations are needed:
  - x is passed pre-transposed (xT [D, B*L]) so projections contract D
    on the partition axis.
  - q/k are produced directly as qT/kT [head_dim, tokens]; scores are
    computed keys-on-partitions, so the P@V matmul consumes exp(scores)
    directly and the wo matmul consumes the attention output directly.
  - RoPE head_dim pairs are permuted (on the host, into wq/wk rows) so
    each rotation partner lives 16 partitions away within a 32-partition
    quadrant -> one DVE stream_shuffle does the swap.
  - softmax denominator comes from an all-ones matmul (partition
    broadcast for free); no max subtraction (fp32 logits here are <~15).
"""

import numpy as np

import concourse.bass as bass
import concourse.mybir as mybir
import concourse.tile as tile
from concourse import bacc
from concourse.alu_op_type import AluOpType
from concourse.masks import make_identity

F32 = mybir.dt.float32
BF16 = mybir.dt.bfloat16

N_CORES = 8
NHL = 4           # local q heads per core
HD = 128          # head dim
THETA = 10000.0
SCALE = HD ** -0.5
TW = 512          # token block width (free dim of most matmuls)
KW = 128          # key tile width (partition dim of score tiles)

# module-level knobs for test.py
TRACE = False
LAST_RESULTS = None


class Cfg:
    def __init__(self, B=2, L=2048, D=4096):
        self.B, self.L, self.D = B, L, D
        self.BL = B * L
        self.DC = D // 128         # contraction chunks for projections
        self.NB = L // TW          # query blocks per batch
        self.NT = self.BL // TW    # token blocks total
        self.KT = L // KW          # key tiles per batch
        self.NBLK = D // TW        # wo output column blocks
        self.NCH = self.NT         # ReduceScatter chunks (one per token block)
        assert self.BL % TW == 0 and TW % N_CORES == 0


# stream_shuffle mask: swap 16-partition halves within each 32-partition quadrant
SWAP16 = [(i + 16) % 32 for i in range(32)]


def _rope_perm():
    """Permutation of head_dim rows: pair i=(16q + r) lives at partitions
    32q+r (x1 = even dim 2i) and 32q+16+r (x2 = odd dim 2i+1)."""
    perm = np.zeros(HD, dtype=np.int64)
    for p in range(HD):
        q, r = divmod(p, 32)
        i = 16 * q + (r % 16)
        perm[p] = 2 * i + (0 if r < 16 else 1)
    return perm


def _rope_tables(cfg):
    """cosT/sinT [128, L] in the permuted-partition layout, sin sign-folded."""
    perm = _rope_perm()
    t = np.arange(cfg.L, dtype=np.float64)
    freqs = THETA ** (-np.arange(0, HD, 2, dtype=np.float64) / HD)  # [64]
    theta = t[None, :] * freqs[:, None]                             # [64, L]
    cos, sin = np.cos(theta), np.sin(theta)
    C = np.zeros((HD, cfg.L), dtype=np.float32)
    S = np.zeros((HD, cfg.L), dtype=np.float32)
    for p in range(HD):
        q, r = divmod(p, 32)
        i = 16 * q + (r % 16)
        C[p] = cos[i]
        S[p] = sin[i] if r >= 16 else -sin[i]
    return C, S


def classify_mask(mask, cfg):
    """cls[kt][qb] in {'Z','N','M'} for tile mask[qb*TW:(qb+1)*TW, kt*KW:(kt+1)*KW]."""
    cls = [[None] * cfg.NB for _ in range(cfg.KT)]
    for kt in range(cfg.KT):
        for qb in range(cfg.NB):
            t = mask[qb * TW:(qb + 1) * TW, kt * KW:(kt + 1) * KW]
            if np.all(t == 0.0):
                cls[kt][qb] = 'Z'
            elif np.all(t <= -1e8):
                cls[kt][qb] = 'N'
            else:
                cls[kt][qb] = 'M'
    # guard: every query block must attend to at least one key tile
    for qb in range(cfg.NB):
        assert any(cls[kt][qb] != 'N' for kt in range(cfg.KT)), \
            "fully-masked query block unsupported"
    return cls


def build_bass(cfg, cls):
    nc = bacc.Bacc("TRN2", target_bir_lowering=False, debug=False,
                   num_devices=N_CORES)

    xT_d = nc.dram_tensor("xT", [cfg.D, cfg.BL], BF16, kind="ExternalInput")
    wqT_d = nc.dram_tensor("wqT", [cfg.D, NHL * HD], BF16, kind="ExternalInput")
    wkT_d = nc.dram_tensor("wkT", [cfg.D, HD], BF16, kind="ExternalInput")
    wvT_d = nc.dram_tensor("wvT", [cfg.D, HD], BF16, kind="ExternalInput")
    woT_d = nc.dram_tensor("woT", [NHL * HD, cfg.D], BF16, kind="ExternalInput")
    maskT_d = nc.dram_tensor("maskT", [cfg.L, cfg.L], F32, kind="ExternalInput")
    ropeC_d = nc.dram_tensor("ropeC", [HD, cfg.L], F32, kind="ExternalInput")
    ropeS_d = nc.dram_tensor("ropeS", [HD, cfg.L], F32, kind="ExternalInput")
    out_d = nc.dram_tensor("out", [cfg.BL // N_CORES, cfg.D], F32,
                           kind="ExternalOutput")

    rg = [list(range(N_CORES))]
    QD = NHL * HD  # 512

    with tile.TileContext(nc) as tc:
        # ---- constants / tables -------------------------------------------
        const_pool = tc.alloc_tile_pool(name="const", bufs=1)
        ones_sb = const_pool.tile([128, 128], BF16, name="ones_sb")
        nc.vector.memset(ones_sb[:], 1.0)
        ident = const_pool.tile([128, 128], BF16, name="ident")
        make_identity(nc, ident[:])
        ropeC = const_pool.tile([HD, cfg.L], F32, name="ropeC_sb")
        ropeS = const_pool.tile([HD, cfg.L], F32, name="ropeS_sb")
        nc.sync.dma_start(out=ropeC[:], in_=ropeC_d.ap())
        nc.sync.dma_start(out=ropeS[:], in_=ropeS_d.ap())

        # ---- resident activations -----------------------------------------
        kv_pool = tc.alloc_tile_pool(name="kv", bufs=1)
        kT_sb = kv_pool.tile([HD, cfg.BL], BF16, name="kT_sb")
        v_sb = kv_pool.tile([128, cfg.BL], BF16, name="v_sb")

        # DRAM scratch
        dram_pool = tc.alloc_tile_pool(name="dram", bufs=1, space="DRAM")
        qT_dram = dram_pool.tile([QD, cfg.BL], BF16, name="qT_dram")
        rs_in = [dram_pool.tile([TW, cfg.D], BF16, name=f"rs_in{c}")
                 for c in range(cfg.NCH)]
        rs_out = [dram_pool.tile([TW // N_CORES, cfg.D], BF16,
                                 name=f"rs_out{c}")
                  for c in range(cfg.NCH)]

        # ================= phase 1: QKV projections + RoPE =================
        with tc.tile_pool(name="wqkv", bufs=1) as w_pool, \
             tc.tile_pool(name="xload", bufs=4) as x_pool, \
             tc.tile_pool(name="qkpsum", bufs=1, space="PSUM") as qk_psum, \
             tc.tile_pool(name="ropetmp", bufs=3) as rtmp_pool, \
             tc.tile_pool(name="qrot", bufs=3) as qrot_pool, \
             tc.tile_pool(name="vstage", bufs=2) as vst_pool, \
             tc.tile_pool(name="vtpsum", bufs=2, space="PSUM") as vt_psum:

            wq_sb = w_pool.tile([128, cfg.DC * QD], BF16, name="wq_sb")
            wk_sb = w_pool.tile([128, cfg.DC * HD], BF16, name="wk_sb")
            wv_sb = w_pool.tile([128, cfg.DC * HD], BF16, name="wv_sb")
            for dc in range(cfg.DC):
                nc.sync.dma_start(out=wq_sb[:, dc * QD:(dc + 1) * QD],
                                  in_=wqT_d.ap()[dc * 128:(dc + 1) * 128, :])
                nc.sync.dma_start(out=wk_sb[:, dc * HD:(dc + 1) * HD],
                                  in_=wkT_d.ap()[dc * 128:(dc + 1) * 128, :])
                nc.sync.dma_start(out=wv_sb[:, dc * HD:(dc + 1) * HD],
                                  in_=wvT_d.ap()[dc * 128:(dc + 1) * 128, :])

            def rope_drain(ps, dst):
                """dst = ps*C + shuffle16(ps)*S at token offset t0 (len TW)."""
                sw = rtmp_pool.tile([128, TW], F32, name="rope_sw")
                t1 = rtmp_pool.tile([128, TW], F32, name="rope_t1")
                t2 = rtmp_pool.tile([128, TW], F32, name="rope_t2")
                nc.vector.stream_shuffle(sw[:], ps, SWAP16)
                nc.vector.tensor_tensor(t1[:], sw[:], Sx, AluOpType.mult)
                nc.vector.tensor_tensor(t2[:], ps, Cx, AluOpType.mult)
                nc.vector.tensor_tensor(dst, t1[:], t2[:], AluOpType.add)

            for tb in range(cfg.NT):
                t0 = (tb % cfg.NB) * TW  # position within batch
                Cx = ropeC[:, t0:t0 + TW]
                Sx = ropeS[:, t0:t0 + TW]

                q_ps = qk_psum.tile([128, NHL * TW], F32, name="q_ps")
                k_ps = qk_psum.tile([128, TW], F32, name="k_ps")
                vT_ps = qk_psum.tile([128, TW], F32, name="vT_ps")
                for dc in range(cfg.DC):
                    xt = x_pool.tile([128, TW], BF16, name="x_t")
                    nc.sync.dma_start(
                        out=xt[:],
                        in_=xT_d.ap()[dc * 128:(dc + 1) * 128,
                                      tb * TW:(tb + 1) * TW])
                    st = dict(start=(dc == 0), stop=(dc == cfg.DC - 1))
                    for h in range(NHL):
                        nc.tensor.matmul(
                            q_ps[:, h * TW:h * TW + TW],
                            wq_sb[:, dc * QD + h * HD: dc * QD + (h + 1) * HD],
                            xt[:], **st)
                    nc.tensor.matmul(k_ps[:],
                                     wk_sb[:, dc * HD:(dc + 1) * HD],
                                     xt[:], **st)
                    nc.tensor.matmul(vT_ps[:],
                                     wv_sb[:, dc * HD:(dc + 1) * HD],
                                     xt[:], **st)

                # q: rope -> spill to DRAM
                for h in range(NHL):
                    qr = qrot_pool.tile([128, TW], BF16, name="q_rot")
                    rope_drain(q_ps[:, h * TW:h * TW + TW], qr[:])
                    nc.sync.dma_start(
                        out=qT_dram[h * HD:(h + 1) * HD,
                                    tb * TW:(tb + 1) * TW],
                        in_=qr[:])
                # k: rope -> resident
                rope_drain(k_ps[:], kT_sb[:, tb * TW:(tb + 1) * TW])
                # v: vT -> transpose -> resident [ktok, hd] blocks
                vt_sb = vst_pool.tile([128, TW], BF16, name="vT_stage")
                nc.scalar.copy(vt_sb[:], vT_ps[:])
                for i in range(TW // 128):
                    vp = vt_psum.tile([128, 128], BF16, name="v_tr_ps")
                    nc.tensor.transpose(vp[:], vt_sb[:, i * 128:(i + 1) * 128],
                                        ident[:])
                    nc.scalar.copy(
                        v_sb[:, tb * TW + i * 128: tb * TW + (i + 1) * 128],
                        vp[:])

        # ================= phase 2: attention + wo + ReduceScatter =========
        with tc.tile_pool(name="wo", bufs=1) as wo_pool, \
             tc.tile_pool(name="mask", bufs=max(cfg.KT, 4)) as m_pool, \
             tc.tile_pool(name="qload", bufs=3) as q_pool, \
             tc.tile_pool(name="expsb", bufs=4) as e_pool, \
             tc.tile_pool(name="msum", bufs=3) as msk_pool, \
             tc.tile_pool(name="attnsb", bufs=2) as at_pool, \
             tc.tile_pool(name="recsb", bufs=2) as rec_pool, \
             tc.tile_pool(name="outcp", bufs=4) as oc_pool, \
             tc.tile_pool(name="fin", bufs=2) as fin_pool, \
             tc.tile_pool(name="scps", bufs=3, space="PSUM") as sc_psum, \
             tc.tile_pool(name="avps", bufs=2, space="PSUM") as av_psum, \
             tc.tile_pool(name="seps", bufs=1, space="PSUM") as se_psum, \
             tc.tile_pool(name="ops", bufs=2, space="PSUM") as o_psum:

            wo_sb = wo_pool.tile([128, NHL * cfg.D], BF16, name="wo_sb")
            for h in range(NHL):
                nc.sync.dma_start(out=wo_sb[:, h * cfg.D:(h + 1) * cfg.D],
                                  in_=woT_d.ap()[h * HD:(h + 1) * HD, :])

            for qb in range(cfg.NB):
                active = [kt for kt in range(cfg.KT) if cls[kt][qb] != 'N']
                mtiles = {}
                for kt in active:
                    if cls[kt][qb] == 'M':
                        mt = m_pool.tile([KW, TW], F32, name="m_t")
                        nc.sync.dma_start(
                            out=mt[:],
                            in_=maskT_d.ap()[kt * KW:(kt + 1) * KW,
                                             qb * TW:(qb + 1) * TW])
                        mtiles[kt] = mt

                for b in range(cfg.B):
                    attn_sb = at_pool.tile([128, NHL * TW], BF16, name="at_sb")
                    for h in range(NHL):
                        qt = q_pool.tile([HD, TW], BF16, name="q_t")
                        nc.sync.dma_start(
                            out=qt[:],
                            in_=qT_dram[h * HD:(h + 1) * HD,
                                        (b * cfg.NB + qb) * TW:
                                        (b * cfg.NB + qb + 1) * TW])
                        at_ps = av_psum.tile([HD, TW], F32, name="at_ps")
                        se_ps = se_psum.tile([128, TW], F32, name="se_ps")
                        for idx, kt in enumerate(active):
                            gk = b * cfg.L + kt * KW  # global key token
                            sc_ps = sc_psum.tile([KW, TW], F32, name="sc_ps")
                            nc.tensor.matmul(sc_ps[:],
                                             kT_sb[:, gk:gk + KW],
                                             qt[:], start=True, stop=True)
                            if cls[kt][qb] == 'M':
                                ms = msk_pool.tile([KW, TW], F32, name="ms_t")
                                nc.vector.tensor_tensor(
                                    ms[:], sc_ps[:], mtiles[kt][:],
                                    AluOpType.add)
                                esrc = ms[:]
                            else:
                                esrc = sc_ps[:]
                            ex = e_pool.tile([KW, TW], BF16, name="ex_t")
                            nc.scalar.activation(
                                ex[:], esrc,
                                mybir.ActivationFunctionType.Exp,
                                scale=float(SCALE))
                            st = dict(start=(idx == 0),
                                      stop=(idx == len(active) - 1))
                            nc.tensor.matmul(at_ps[:], v_sb[:, gk:gk + KW],
                                             ex[:], **st)
                            nc.tensor.matmul(se_ps[:], ones_sb[:], ex[:], **st)
                        rec = rec_pool.tile([128, TW], F32, name="rec_t")
                        nc.vector.reciprocal(rec[:], se_ps[:])
                        nc.vector.tensor_tensor(
                            attn_sb[:, h * TW:(h + 1) * TW],
                            at_ps[:], rec[:], AluOpType.mult)

                    # ---- wo partial for this (b, qb) token block ----------
                    c = b * cfg.NB + qb
                    for m in range(TW // 128):
                        for n in range(cfg.NBLK):
                            o_ps = o_psum.tile([128, TW], F32, name="o_ps")
                            for h in range(NHL):
                                nc.tensor.matmul(
                                    o_ps[:],
                                    attn_sb[:, h * TW + m * 128:
                                            h * TW + (m + 1) * 128],
                                    wo_sb[:, h * cfg.D + n * TW:
                                          h * cfg.D + (n + 1) * TW],
                                    start=(h == 0), stop=(h == NHL - 1))
                            oc = oc_pool.tile([128, TW], BF16, name="oc_t")
                            nc.scalar.copy(oc[:], o_ps[:])
                            nc.sync.dma_start(
                                out=rs_in[c][m * 128:(m + 1) * 128,
                                             n * TW:(n + 1) * TW],
                                in_=oc[:])
                    nc.gpsimd.collective_compute(
                        "ReduceScatter", AluOpType.add, replica_groups=rg,
                        ins=[rs_in[c][:].opt()], outs=[rs_out[c][:].opt()])
                    # bf16 RS shard -> f32 external output (cast via SBUF,
                    # viewed flat as [128, rw*D/128])
                    rw = TW // N_CORES
                    g2 = 128 // rw
                    fw = cfg.D // g2
                    ocb = fin_pool.tile([128, fw], BF16, name="fin_b")
                    ocf = fin_pool.tile([128, fw], F32, name="fin_f")
                    nc.sync.dma_start(
                        out=ocb[:],
                        in_=rs_out[c][:].rearrange("r (g f) -> (r g) f", g=g2))
                    nc.scalar.copy(ocf[:], ocb[:])
                    nc.sync.dma_start(
                        out=out_d.ap()[c * rw:(c + 1) * rw, :]
                        .rearrange("r (g f) -> (r g) f", g=g2),
                        in_=ocf[:])

        dram_pool.release()
        kv_pool.release()
        const_pool.release()

    nc.compile()
    return nc


def host_prepare(cfg, x, mask, wq, wk, wv, wo):
    """Returns (in_maps, cls)."""
    x = np.ascontiguousarray(np.asarray(x, dtype=np.float32))
    mask = np.asarray(mask, dtype=np.float32)
    wq = np.asarray(wq, dtype=np.float32)
    wk = np.asarray(wk, dtype=np.float32)
    wv = np.asarray(wv, dtype=np.float32)
    wo = np.asarray(wo, dtype=np.float32)

    import ml_dtypes
    bf16 = ml_dtypes.bfloat16
    perm = _rope_perm()
    C, S = _rope_tables(cfg)
    xT = np.ascontiguousarray(x.reshape(cfg.BL, cfg.D).T).astype(bf16)
    maskT = np.ascontiguousarray(mask.T / SCALE)
    cls = classify_mask(mask, cfg)

    in_maps = []
    for g in range(N_CORES):
        qrows = wq[g * NHL * HD:(g + 1) * NHL * HD]          # [512, D]
        qperm = np.concatenate(
            [qrows[h * HD + perm] for h in range(NHL)], axis=0)
        krows = wk[g * HD:(g + 1) * HD][perm]                # [128, D]
        vrows = wv[g * HD:(g + 1) * HD]                      # [128, D]
        wocols = wo[:, g * NHL * HD:(g + 1) * NHL * HD]      # [D, 512]
        in_maps.append({
            "xT": xT,
            "wqT": np.ascontiguousarray(qperm.T).astype(bf16),
            "wkT": np.ascontiguousarray(krows.T).astype(bf16),
            "wvT": np.ascontiguousarray(vrows.T).astype(bf16),
            "woT": np.ascontiguousarray(wocols.T).astype(bf16),
            "maskT": maskT,
            "ropeC": C,
            "ropeS": S,
        })
    return in_maps, cls


def assemble_output(cfg, results):
    """Stitch per-core ReduceScatter shards back into [B, L, D]."""
    full = np.empty((cfg.BL, cfg.D), dtype=np.float32)
    rw = TW // N_CORES
    for g in range(N_CORES):
        r = results[g]["out"]
        for c in range(cfg.NCH):
            full[c * TW + g * rw: c * TW + (g + 1) * rw] = \
                r[c * rw:(c + 1) * rw]
    return full.reshape(cfg.B, cfg.L, cfg.D)


def kernel(x, mask, wq, wk, wv, wo):
    global LAST_RESULTS
    from concourse.bass_utils import run_bass_kernel_spmd
    cfg = Cfg(B=2, L=2048, D=4096)
    in_maps, cls = host_prepare(cfg, x, mask, wq, wk, wv, wo)
    nc = build_bass(cfg, cls)
    res = run_bass_kernel_spmd(nc, in_maps, core_ids=list(range(N_CORES)),
                               trace=TRACE)
    LAST_RESULTS = res
    return assemble_output(cfg, res.results)


# revision 23
# speedup vs baseline: 2.9094x; 1.0615x over previous
"""Distributed GQA attention kernel for one TRN2 chip (8 NeuronCores).

Sharding: tensor-parallel over heads. Core g owns query heads [4g, 4g+4)
and kv head g. Each core computes its heads' attention and a partial
output projection; a chunked ReduceScatter sums the partials and leaves
each core with a 1/8 token-slice of the final output.

All device tensors are laid out so that no on-device transposes of the
big activations are needed:
  - x is passed pre-transposed (xT [D, B*L]) so projections contract D
    on the partition axis.
  - q/k are produced directly as qT/kT [head_dim, tokens]; scores are
    computed keys-on-partitions, so the P@V matmul consumes exp(scores)
    directly and the wo matmul consumes the attention output directly.
  - RoPE head_dim pairs are permuted (on the host, into wq/wk rows) so
    each rotation partner lives 16 partitions away within a 32-partition
    quadrant -> one DVE stream_shuffle does the swap.
  - softmax denominator comes from an all-ones matmul (partition
    broadcast for free); no max subtraction (fp32 logits here are <~15).
"""

import numpy as np

import concourse.bass as bass
import concourse.mybir as mybir
import concourse.tile as tile
from concourse import bacc
from concourse.alu_op_type import AluOpType
from concourse.masks import make_identity

F32 = mybir.dt.float32
BF16 = mybir.dt.bfloat16

N_CORES = 8
NHL = 4           # local q heads per core
HD = 128          # head dim
THETA = 10000.0
SCALE = HD ** -0.5
TW = 512          # token block width (free dim of most matmuls)
KW = 128          # key tile width (partition dim of score tiles)

# module-level knobs for test.py
TRACE = False
LAST_RESULTS = None


class Cfg:
    def __init__(self, B=2, L=2048, D=4096):
        self.B, self.L, self.D = B, L, D
        self.BL = B * L
        self.DC = D // 128         # contraction chunks for projections
        self.NB = L // TW          # query blocks per batch
        self.NT = self.BL // TW    # token blocks total
        self.KT = L // KW          # key tiles per batch
        self.NBLK = D // TW        # wo output column blocks
        self.NCH = self.NT         # ReduceScatter chunks (one per token block)
        assert self.BL % TW == 0 and TW % N_CORES == 0


# stream_shuffle mask: swap 16-partition halves within each 32-partition quadrant
SWAP16 = [(i + 16) % 32 for i in range(32)]


def _rope_perm():
    """Permutation of head_dim rows: pair i=(16q + r) lives at partitions
    32q+r (x1 = even dim 2i) and 32q+16+r (x2 = odd dim 2i+1)."""
    perm = np.zeros(HD, dtype=np.int64)
    for p in range(HD):
        q, r = divmod(p, 32)
        i = 16 * q + (r % 16)
        perm[p] = 2 * i + (0 if r < 16 else 1)
    return perm


def _rope_tables(cfg):
    """cosT/sinT [128, L] in the permuted-partition layout, sin sign-folded."""
    perm = _rope_perm()
    t = np.arange(cfg.L, dtype=np.float64)
    freqs = THETA ** (-np.arange(0, HD, 2, dtype=np.float64) / HD)  # [64]
    theta = t[None, :] * freqs[:, None]                             # [64, L]
    cos, sin = np.cos(theta), np.sin(theta)
    C = np.zeros((HD, cfg.L), dtype=np.float32)
    S = np.zeros((HD, cfg.L), dtype=np.float32)
    for p in range(HD):
        q, r = divmod(p, 32)
        i = 16 * q + (r % 16)
        C[p] = cos[i]
        S[p] = sin[i] if r >= 16 else -sin[i]
    return C, S


def classify_mask(mask, cfg):
    """cls[kt][qb] in {'Z','N','M'} for tile mask[qb*TW:(qb+1)*TW, kt*KW:(kt+1)*KW]."""
    cls = [[None] * cfg.NB for _ in range(cfg.KT)]
    for kt in range(cfg.KT):
        for qb in range(cfg.NB):
            t = mask[qb * TW:(qb + 1) * TW, kt * KW:(kt + 1) * KW]
            if np.all(t == 0.0):
                cls[kt][qb] = 'Z'
            elif np.all(t <= -1e8):
                cls[kt][qb] = 'N'
            else:
                cls[kt][qb] = 'M'
    # guard: every query block must attend to at least one key tile
    for qb in range(cfg.NB):
        assert any(cls[kt][qb] != 'N' for kt in range(cfg.KT)), \
            "fully-masked query block unsupported"
    return cls


def build_bass(cfg, cls):
    nc = bacc.Bacc("TRN2", target_bir_lowering=False, debug=False,
                   num_devices=N_CORES)

    xT_d = nc.dram_tensor("xT", [cfg.D, cfg.BL], BF16, kind="ExternalInput")
    wqT_d = nc.dram_tensor("wqT", [cfg.D, NHL * HD], BF16, kind="ExternalInput")
    wkT_d = nc.dram_tensor("wkT", [cfg.D, HD], BF16, kind="ExternalInput")
    wvT_d = nc.dram_tensor("wvT", [cfg.D, HD], BF16, kind="ExternalInput")
    woT_d = nc.dram_tensor("woT", [NHL * HD, cfg.D], BF16, kind="ExternalInput")
    maskT_d = nc.dram_tensor("maskT", [cfg.L, cfg.L], F32, kind="ExternalInput")
    ropeC_d = nc.dram_tensor("ropeC", [HD, cfg.L], F32, kind="ExternalInput")
    ropeS_d = nc.dram_tensor("ropeS", [HD, cfg.L], F32, kind="ExternalInput")
    out_d = nc.dram_tensor("out", [cfg.BL // N_CORES, cfg.D], F32,
                           kind="ExternalOutput")

    rg = [list(range(N_CORES))]
    QD = NHL * HD  # 512

    with tile.TileContext(nc) as tc:
        # ---- constants / tables -------------------------------------------
        const_pool = tc.alloc_tile_pool(name="const", bufs=1)
        ones_sb = const_pool.tile([128, 128], BF16, name="ones_sb")
        nc.vector.memset(ones_sb[:], 1.0)
        ident = const_pool.tile([128, 128], BF16, name="ident")
        make_identity(nc, ident[:])
        ropeC = const_pool.tile([HD, cfg.L], F32, name="ropeC_sb")
        ropeS = const_pool.tile([HD, cfg.L], F32, name="ropeS_sb")
        nc.sync.dma_start(out=ropeC[:], in_=ropeC_d.ap())
        nc.sync.dma_start(out=ropeS[:], in_=ropeS_d.ap())

        # ---- resident activations -----------------------------------------
        kv_pool = tc.alloc_tile_pool(name="kv", bufs=1)
        kT_sb = kv_pool.tile([HD, cfg.BL], BF16, name="kT_sb")
        v_sb = kv_pool.tile([128, cfg.BL], BF16, name="v_sb")

        # DRAM scratch
        dram_pool = tc.alloc_tile_pool(name="dram", bufs=1, space="DRAM")
        qT_dram = dram_pool.tile([QD, cfg.BL], BF16, name="qT_dram")
        rs_in = [dram_pool.tile([TW, cfg.D], BF16, name=f"rs_in{c}")
                 for c in range(cfg.NCH)]
        rs_out = [dram_pool.tile([TW // N_CORES, cfg.D], BF16,
                                 name=f"rs_out{c}")
                  for c in range(cfg.NCH)]

        # ---- all weights resident upfront (bf16 halves the footprint) -----
        w_pool = tc.alloc_tile_pool(name="weights", bufs=1)
        wq_sb = w_pool.tile([128, cfg.DC * QD], BF16, name="wq_sb")
        wk_sb = w_pool.tile([128, cfg.DC * HD], BF16, name="wk_sb")
        wv_sb = w_pool.tile([128, cfg.DC * HD], BF16, name="wv_sb")
        wo_sb = w_pool.tile([128, NHL * cfg.D], BF16, name="wo_sb")
        for dc in range(cfg.DC):
            nc.sync.dma_start(out=wq_sb[:, dc * QD:(dc + 1) * QD],
                              in_=wqT_d.ap()[dc * 128:(dc + 1) * 128, :])
            nc.sync.dma_start(out=wk_sb[:, dc * HD:(dc + 1) * HD],
                              in_=wkT_d.ap()[dc * 128:(dc + 1) * 128, :])
            nc.sync.dma_start(out=wv_sb[:, dc * HD:(dc + 1) * HD],
                              in_=wvT_d.ap()[dc * 128:(dc + 1) * 128, :])
        for h in range(NHL):
            nc.sync.dma_start(out=wo_sb[:, h * cfg.D:(h + 1) * cfg.D],
                              in_=woT_d.ap()[h * HD:(h + 1) * HD, :])

        # ================= phase 1: QKV projections + RoPE =================
        with tc.tile_pool(name="xload", bufs=4) as x_pool, \
             tc.tile_pool(name="qkpsum", bufs=1, space="PSUM") as qk_psum, \
             tc.tile_pool(name="ropetmp", bufs=3) as rtmp_pool, \
             tc.tile_pool(name="qrot", bufs=3) as qrot_pool, \
             tc.tile_pool(name="vstage", bufs=2) as vst_pool, \
             tc.tile_pool(name="vtpsum", bufs=2, space="PSUM") as vt_psum:

            def rope_drain(ps, dst):
                """dst = ps*C + shuffle16(ps)*S at token offset t0 (len TW)."""
                sw = rtmp_pool.tile([128, TW], F32, name="rope_sw")
                t1 = rtmp_pool.tile([128, TW], F32, name="rope_t1")
                t2 = rtmp_pool.tile([128, TW], F32, name="rope_t2")
                nc.vector.stream_shuffle(sw[:], ps, SWAP16)
                nc.vector.tensor_tensor(t1[:], sw[:], Sx, AluOpType.mult)
                nc.vector.tensor_tensor(t2[:], ps, Cx, AluOpType.mult)
                nc.vector.tensor_tensor(dst, t1[:], t2[:], AluOpType.add)

            for tb in range(cfg.NT):
                t0 = (tb % cfg.NB) * TW  # position within batch
                Cx = ropeC[:, t0:t0 + TW]
                Sx = ropeS[:, t0:t0 + TW]

                q_ps = qk_psum.tile([128, NHL * TW], F32, name="q_ps")
                k_ps = qk_psum.tile([128, TW], F32, name="k_ps")
                vT_ps = qk_psum.tile([128, TW], F32, name="vT_ps")
                for dc in range(cfg.DC):
                    xt = x_pool.tile([128, TW], BF16, name="x_t")
                    nc.sync.dma_start(
                        out=xt[:],
                        in_=xT_d.ap()[dc * 128:(dc + 1) * 128,
                                      tb * TW:(tb + 1) * TW])
                    st = dict(start=(dc == 0), stop=(dc == cfg.DC - 1))
                    # k/vT first: their banks are drained (freed) first, so
                    # the next tokblock's leading matmuls stall least
                    nc.tensor.matmul(k_ps[:],
                                     wk_sb[:, dc * HD:(dc + 1) * HD],
                                     xt[:], **st)
                    nc.tensor.matmul(vT_ps[:],
                                     wv_sb[:, dc * HD:(dc + 1) * HD],
                                     xt[:], **st)
                    for h in range(NHL):
                        nc.tensor.matmul(
                            q_ps[:, h * TW:h * TW + TW],
                            wq_sb[:, dc * QD + h * HD: dc * QD + (h + 1) * HD],
                            xt[:], **st)

                # k: rope -> resident (drain first: next tb needs this bank)
                rope_drain(k_ps[:], kT_sb[:, tb * TW:(tb + 1) * TW])
                # q: rope -> spill to DRAM
                for h in range(NHL):
                    qr = qrot_pool.tile([128, TW], BF16, name="q_rot")
                    rope_drain(q_ps[:, h * TW:h * TW + TW], qr[:])
                    nc.sync.dma_start(
                        out=qT_dram[h * HD:(h + 1) * HD,
                                    tb * TW:(tb + 1) * TW],
                        in_=qr[:])
                # v: vT -> transpose -> resident [ktok, hd] blocks
                vt_sb = vst_pool.tile([128, TW], BF16, name="vT_stage")
                nc.scalar.copy(vt_sb[:], vT_ps[:])
                for i in range(TW // 128):
                    vp = vt_psum.tile([128, 128], BF16, name="v_tr_ps")
                    nc.tensor.transpose(vp[:], vt_sb[:, i * 128:(i + 1) * 128],
                                        ident[:])
                    nc.scalar.copy(
                        v_sb[:, tb * TW + i * 128: tb * TW + (i + 1) * 128],
                        vp[:])

        # ================= phase 2: attention + wo + ReduceScatter =========
        with tc.tile_pool(name="mask", bufs=max(cfg.KT, 4)) as m_pool, \
             tc.tile_pool(name="qload", bufs=3) as q_pool, \
             tc.tile_pool(name="expsb", bufs=4) as e_pool, \
             tc.tile_pool(name="msum", bufs=3) as msk_pool, \
             tc.tile_pool(name="attnsb", bufs=2) as at_pool, \
             tc.tile_pool(name="recsb", bufs=2) as rec_pool, \
             tc.tile_pool(name="outcp", bufs=4) as oc_pool, \
             tc.tile_pool(name="fin", bufs=2) as fin_pool, \
             tc.tile_pool(name="scps", bufs=3, space="PSUM") as sc_psum, \
             tc.tile_pool(name="avps", bufs=2, space="PSUM") as av_psum, \
             tc.tile_pool(name="seps", bufs=1, space="PSUM") as se_psum, \
             tc.tile_pool(name="ops", bufs=2, space="PSUM") as o_psum:

            for qb in range(cfg.NB):
                active = [kt for kt in range(cfg.KT) if cls[kt][qb] != 'N']
                mtiles = {}
                for kt in active:
                    if cls[kt][qb] == 'M':
                        mt = m_pool.tile([KW, TW], F32, name="m_t")
                        nc.sync.dma_start(
                            out=mt[:],
                            in_=maskT_d.ap()[kt * KW:(kt + 1) * KW,
                                             qb * TW:(qb + 1) * TW])
                        mtiles[kt] = mt

                for b in range(cfg.B):
                    attn_sb = at_pool.tile([128, NHL * TW], BF16, name="at_sb")
                    for h in range(NHL):
                        qt = q_pool.tile([HD, TW], BF16, name="q_t")
                        nc.sync.dma_start(
                            out=qt[:],
                            in_=qT_dram[h * HD:(h + 1) * HD,
                                        (b * cfg.NB + qb) * TW:
                                        (b * cfg.NB + qb + 1) * TW])
                        at_ps = av_psum.tile([HD, TW], F32, name="at_ps")
                        se_ps = se_psum.tile([128, TW], F32, name="se_ps")
                        # software pipeline: issue score matmuls LOOKAHEAD
                        # iterations ahead so the PE never waits on exp (ACT)
                        LOOKAHEAD = 2
                        n_act = len(active)
                        sc_tiles = [None] * n_act

                        def emit_sc(j):
                            gk2 = b * cfg.L + active[j] * KW
                            sc = sc_psum.tile([KW, TW], F32, name="sc_ps")
                            nc.tensor.matmul(sc[:], kT_sb[:, gk2:gk2 + KW],
                                             qt[:], start=True, stop=True)
                            sc_tiles[j] = sc

                        for j in range(min(LOOKAHEAD, n_act)):
                            emit_sc(j)
                        for idx, kt in enumerate(active):
                            if idx + LOOKAHEAD < n_act:
                                emit_sc(idx + LOOKAHEAD)
                            gk = b * cfg.L + kt * KW  # global key token
                            sc_ps = sc_tiles[idx]
                            sc_tiles[idx] = None
                            if cls[kt][qb] == 'M':
                                ms = msk_pool.tile([KW, TW], F32, name="ms_t")
                                nc.vector.tensor_tensor(
                                    ms[:], sc_ps[:], mtiles[kt][:],
                                    AluOpType.add)
                                esrc = ms[:]
                            else:
                                esrc = sc_ps[:]
                            ex = e_pool.tile([KW, TW], BF16, name="ex_t")
                            nc.scalar.activation(
                                ex[:], esrc,
                                mybir.ActivationFunctionType.Exp,
                                scale=float(SCALE))
                            st = dict(start=(idx == 0),
                                      stop=(idx == len(active) - 1))
                            nc.tensor.matmul(se_ps[:], ones_sb[:], ex[:], **st)
                            nc.tensor.matmul(at_ps[:], v_sb[:, gk:gk + KW],
                                             ex[:], **st)
                        rec = rec_pool.tile([128, TW], F32, name="rec_t")
                        nc.vector.reciprocal_approx_fast(rec[:], se_ps[:])
                        nc.vector.tensor_tensor(
                            attn_sb[:, h * TW:(h + 1) * TW],
                            at_ps[:], rec[:], AluOpType.mult)

                    # ---- wo partial for this (b, qb) token block ----------
                    c = b * cfg.NB + qb
                    for m in range(TW // 128):
                        for n in range(cfg.NBLK):
                            o_ps = o_psum.tile([128, TW], F32, name="o_ps")
                            for h in range(NHL):
                                nc.tensor.matmul(
                                    o_ps[:],
                                    attn_sb[:, h * TW + m * 128:
                                            h * TW + (m + 1) * 128],
                                    wo_sb[:, h * cfg.D + n * TW:
                                          h * cfg.D + (n + 1) * TW],
                                    start=(h == 0), stop=(h == NHL - 1))
                            oc = oc_pool.tile([128, TW], BF16, name="oc_t")
                            # alternate drain engine: keep ACT free for exp
                            if n % 2 == 0:
                                nc.vector.tensor_copy(oc[:], o_ps[:])
                            else:
                                nc.scalar.copy(oc[:], o_ps[:])
                            nc.sync.dma_start(
                                out=rs_in[c][m * 128:(m + 1) * 128,
                                             n * TW:(n + 1) * TW],
                                in_=oc[:])
                    nc.gpsimd.collective_compute(
                        "ReduceScatter", AluOpType.add, replica_groups=rg,
                        ins=[rs_in[c][:].opt()], outs=[rs_out[c][:].opt()])
                    # bf16 RS shard -> f32 external output (cast via SBUF,
                    # viewed flat as [128, rw*D/128])
                    rw = TW // N_CORES
                    g2 = 128 // rw
                    fw = cfg.D // g2
                    ocb = fin_pool.tile([128, fw], BF16, name="fin_b")
                    ocf = fin_pool.tile([128, fw], F32, name="fin_f")
                    nc.sync.dma_start(
                        out=ocb[:],
                        in_=rs_out[c][:].rearrange("r (g f) -> (r g) f", g=g2))
                    nc.scalar.copy(ocf[:], ocb[:])
                    nc.sync.dma_start(
                        out=out_d.ap()[c * rw:(c + 1) * rw, :]
                        .rearrange("r (g f) -> (r g) f", g=g2),
                        in_=ocf[:])

        w_pool.release()
        dram_pool.release()
        kv_pool.release()
        const_pool.release()

    nc.compile()
    return nc


def host_prepare(cfg, x, mask, wq, wk, wv, wo):
    """Returns (in_maps, cls)."""
    x = np.ascontiguousarray(np.asarray(x, dtype=np.float32))
    mask = np.asarray(mask, dtype=np.float32)
    wq = np.asarray(wq, dtype=np.float32)
    wk = np.asarray(wk, dtype=np.float32)
    wv = np.asarray(wv, dtype=np.float32)
    wo = np.asarray(wo, dtype=np.float32)

    import ml_dtypes
    bf16 = ml_dtypes.bfloat16
    perm = _rope_perm()
    C, S = _rope_tables(cfg)
    xT = np.ascontiguousarray(x.reshape(cfg.BL, cfg.D).T).astype(bf16)
    maskT = np.ascontiguousarray(mask.T / SCALE)
    cls = classify_mask(mask, cfg)

    in_maps = []
    for g in range(N_CORES):
        qrows = wq[g * NHL * HD:(g + 1) * NHL * HD]          # [512, D]
        qperm = np.concatenate(
            [qrows[h * HD + perm] for h in range(NHL)], axis=0)
        krows = wk[g * HD:(g + 1) * HD][perm]                # [128, D]
        vrows = wv[g * HD:(g + 1) * HD]                      # [128, D]
        wocols = wo[:, g * NHL * HD:(g + 1) * NHL * HD]      # [D, 512]
        in_maps.append({
            "xT": xT,
            "wqT": np.ascontiguousarray(qperm.T).astype(bf16),
            "wkT": np.ascontiguousarray(krows.T).astype(bf16),
            "wvT": np.ascontiguousarray(vrows.T).astype(bf16),
            "woT": np.ascontiguousarray(wocols.T).astype(bf16),
            "maskT": maskT,
            "ropeC": C,
            "ropeS": S,
        })
    return in_maps, cls


def assemble_output(cfg, results):
    """Stitch per-core ReduceScatter shards back into [B, L, D]."""
    full = np.empty((cfg.BL, cfg.D), dtype=np.float32)
    rw = TW // N_CORES
    for g in range(N_CORES):
        r = results[g]["out"]
        for c in range(cfg.NCH):
            full[c * TW + g * rw: c * TW + (g + 1) * rw] = \
                r[c * rw:(c + 1) * rw]
    return full.reshape(cfg.B, cfg.L, cfg.D)


def kernel(x, mask, wq, wk, wv, wo):
    global LAST_RESULTS
    from concourse.bass_utils import run_bass_kernel_spmd
    cfg = Cfg(B=2, L=2048, D=4096)
    in_maps, cls = host_prepare(cfg, x, mask, wq, wk, wv, wo)
    nc = build_bass(cfg, cls)
    res = run_bass_kernel_spmd(nc, in_maps, core_ids=list(range(N_CORES)),
                               trace=TRACE)
    LAST_RESULTS = res
    return assemble_output(cfg, res.results)


# revision 29
# speedup vs baseline: 3.3964x; 1.1674x over previous
"""Distributed GQA attention kernel for one TRN2 chip (8 NeuronCores).

Sharding: tensor-parallel over heads. Core g owns query heads [4g, 4g+4)
and kv head g. Each core computes its heads' attention and a partial
output projection; a chunked ReduceScatter sums the partials and leaves
each core with a 1/8 token-slice of the final output.

All device tensors are laid out so that no on-device transposes of the
big activations are needed:
  - x is passed pre-transposed (xT [D, B*L]) so projections contract D
    on the partition axis.
  - q/k are produced directly as qT/kT [head_dim, tokens]; scores are
    computed keys-on-partitions, so the P@V matmul consumes exp(scores)
    directly and the wo matmul consumes the attention output directly.
  - RoPE head_dim pairs are permuted (on the host, into wq/wk rows) so
    each rotation partner lives 16 partitions away within a 32-partition
    quadrant -> one DVE stream_shuffle does the swap.
  - softmax denominator comes from an all-ones matmul (partition
    broadcast for free); no max subtraction (fp32 logits here are <~15).
"""

import numpy as np

import concourse.bass as bass
import concourse.mybir as mybir
import concourse.tile as tile
from concourse import bacc
from concourse.alu_op_type import AluOpType
from concourse.masks import make_identity

F32 = mybir.dt.float32
BF16 = mybir.dt.bfloat16

N_CORES = 8
NHL = 4           # local q heads per core
HD = 128          # head dim
THETA = 10000.0
SCALE = HD ** -0.5
TW = 512          # token block width (free dim of most matmuls)
KW = 128          # key tile width (partition dim of score tiles)

# module-level knobs for test.py
TRACE = False
LAST_RESULTS = None


class Cfg:
    def __init__(self, B=2, L=2048, D=4096):
        self.B, self.L, self.D = B, L, D
        self.BL = B * L
        self.DC = D // 128         # contraction chunks for projections
        self.NB = L // TW          # query blocks per batch
        self.NT = self.BL // TW    # token blocks total
        self.KT = L // KW          # key tiles per batch
        self.NBLK = D // TW        # wo output column blocks
        self.NCH = self.NT         # ReduceScatter chunks (one per token block)
        assert self.BL % TW == 0 and TW % N_CORES == 0


# stream_shuffle mask: swap 16-partition halves within each 32-partition quadrant
SWAP16 = [(i + 16) % 32 for i in range(32)]


def _rope_perm():
    """Permutation of head_dim rows: pair i=(16q + r) lives at partitions
    32q+r (x1 = even dim 2i) and 32q+16+r (x2 = odd dim 2i+1)."""
    perm = np.zeros(HD, dtype=np.int64)
    for p in range(HD):
        q, r = divmod(p, 32)
        i = 16 * q + (r % 16)
        perm[p] = 2 * i + (0 if r < 16 else 1)
    return perm


def _rope_tables(cfg):
    """cosT/sinT [128, L] in the permuted-partition layout, sin sign-folded."""
    perm = _rope_perm()
    t = np.arange(cfg.L, dtype=np.float64)
    freqs = THETA ** (-np.arange(0, HD, 2, dtype=np.float64) / HD)  # [64]
    theta = t[None, :] * freqs[:, None]                             # [64, L]
    cos, sin = np.cos(theta), np.sin(theta)
    C = np.zeros((HD, cfg.L), dtype=np.float32)
    S = np.zeros((HD, cfg.L), dtype=np.float32)
    for p in range(HD):
        q, r = divmod(p, 32)
        i = 16 * q + (r % 16)
        C[p] = cos[i]
        S[p] = sin[i] if r >= 16 else -sin[i]
    return C, S


def classify_mask(mask, cfg):
    """cls[kt][qb] = (kind, off): kind in {'Z','N','M'} for tile
    mask[qb*TW:(qb+1)*TW, kt*KW:(kt+1)*KW]; off = count of leading query
    columns in the tile that are fully masked (safe to skip: exp would
    be exactly 0 there)."""
    cls = [[None] * cfg.NB for _ in range(cfg.KT)]
    for kt in range(cfg.KT):
        for qb in range(cfg.NB):
            t = mask[qb * TW:(qb + 1) * TW, kt * KW:(kt + 1) * KW]
            if np.all(t == 0.0):
                cls[kt][qb] = ('Z', 0)
            elif np.all(t <= -1e8):
                cls[kt][qb] = ('N', 0)
            else:
                dead_q = np.all(t <= -1e8, axis=1)  # [TW]
                off = 0
                while off < len(dead_q) and dead_q[off]:
                    off += 1
                off = (off // 64) * 64  # keep offsets 64-aligned
                cls[kt][qb] = ('M', off)
    # guard: every query block must attend to at least one key tile
    for qb in range(cfg.NB):
        assert any(cls[kt][qb][0] != 'N' for kt in range(cfg.KT)), \
            "fully-masked query block unsupported"
    return cls


def build_bass(cfg, cls):
    nc = bacc.Bacc("TRN2", target_bir_lowering=False, debug=False,
                   num_devices=N_CORES)

    xT_d = nc.dram_tensor("xT", [cfg.D, cfg.BL], BF16, kind="ExternalInput")
    wqT_d = nc.dram_tensor("wqT", [cfg.D, NHL * HD], BF16, kind="ExternalInput")
    wkT_d = nc.dram_tensor("wkT", [cfg.D, HD], BF16, kind="ExternalInput")
    wvT_d = nc.dram_tensor("wvT", [cfg.D, HD], BF16, kind="ExternalInput")
    woT_d = nc.dram_tensor("woT", [NHL * HD, cfg.D], BF16, kind="ExternalInput")
    maskT_d = nc.dram_tensor("maskT", [cfg.L, cfg.L], BF16, kind="ExternalInput")
    ropeC_d = nc.dram_tensor("ropeC", [HD, cfg.L], F32, kind="ExternalInput")
    ropeS_d = nc.dram_tensor("ropeS", [HD, cfg.L], F32, kind="ExternalInput")
    out_d = nc.dram_tensor("out", [cfg.BL // N_CORES, cfg.D], F32,
                           kind="ExternalOutput")

    rg = [list(range(N_CORES))]
    QD = NHL * HD  # 512

    with tile.TileContext(nc) as tc:
        # ---- constants / tables -------------------------------------------
        const_pool = tc.alloc_tile_pool(name="const", bufs=1)
        ones_sb = const_pool.tile([128, 128], BF16, name="ones_sb")
        nc.vector.memset(ones_sb[:], 1.0)
        ident = const_pool.tile([128, 128], BF16, name="ident")
        make_identity(nc, ident[:])

        # ---- resident activations -----------------------------------------
        kv_pool = tc.alloc_tile_pool(name="kv", bufs=1)
        kT_sb = kv_pool.tile([HD, cfg.BL], BF16, name="kT_sb")
        v_sb = kv_pool.tile([128, cfg.BL], BF16, name="v_sb")

        # DRAM scratch
        dram_pool = tc.alloc_tile_pool(name="dram", bufs=1, space="DRAM")
        qT_dram = dram_pool.tile([QD, cfg.BL], BF16, name="qT_dram")
        rs_in = [dram_pool.tile([TW, cfg.D], BF16, name=f"rs_in{c}")
                 for c in range(cfg.NCH)]
        rs_out = [dram_pool.tile([TW // N_CORES, cfg.D], BF16,
                                 name=f"rs_out{c}")
                  for c in range(cfg.NCH)]

        # ---- all weights resident upfront (bf16 halves the footprint) -----
        w_pool = tc.alloc_tile_pool(name="weights", bufs=1)
        wq_sb = w_pool.tile([128, cfg.DC * QD], BF16, name="wq_sb")
        wk_sb = w_pool.tile([128, cfg.DC * HD], BF16, name="wk_sb")
        wv_sb = w_pool.tile([128, cfg.DC * HD], BF16, name="wv_sb")
        wo_sb = w_pool.tile([128, NHL * cfg.D], BF16, name="wo_sb")
        for dc in range(cfg.DC):
            nc.sync.dma_start(out=wq_sb[:, dc * QD:(dc + 1) * QD],
                              in_=wqT_d.ap()[dc * 128:(dc + 1) * 128, :])
            nc.sync.dma_start(out=wk_sb[:, dc * HD:(dc + 1) * HD],
                              in_=wkT_d.ap()[dc * 128:(dc + 1) * 128, :])
            nc.sync.dma_start(out=wv_sb[:, dc * HD:(dc + 1) * HD],
                              in_=wvT_d.ap()[dc * 128:(dc + 1) * 128, :])
        for h in range(NHL):
            nc.sync.dma_start(out=wo_sb[:, h * cfg.D:(h + 1) * cfg.D],
                              in_=woT_d.ap()[h * HD:(h + 1) * HD, :])

        # ================= phase 1: QKV projections + RoPE =================
        with tc.tile_pool(name="xload", bufs=cfg.DC + 2) as x_pool, \
             tc.tile_pool(name="qpsum", bufs=1, space="PSUM") as q_psum, \
             tc.tile_pool(name="kpsum", bufs=2, space="PSUM") as k_psum, \
             tc.tile_pool(name="vpsum", bufs=1, space="PSUM") as v_psum, \
             tc.tile_pool(name="ropetmp", bufs=3) as rtmp_pool, \
             tc.tile_pool(name="qrot", bufs=3) as qrot_pool, \
             tc.tile_pool(name="vstage", bufs=2) as vst_pool, \
             tc.tile_pool(name="vtpsum", bufs=1, space="PSUM") as vt_psum, \
             tc.tile_pool(name="ropetbl", bufs=1) as rtbl_pool:

            ropeC = rtbl_pool.tile([HD, cfg.L], F32, name="ropeC_sb")
            ropeS = rtbl_pool.tile([HD, cfg.L], F32, name="ropeS_sb")
            nc.sync.dma_start(out=ropeC[:], in_=ropeC_d.ap())
            nc.sync.dma_start(out=ropeS[:], in_=ropeS_d.ap())

            def rope_drain(ps, dst):
                """dst = ps*C + shuffle16(ps)*S at token offset t0 (len TW)."""
                sw = rtmp_pool.tile([128, TW], F32, name="rope_sw")
                t1 = rtmp_pool.tile([128, TW], F32, name="rope_t1")
                t2 = rtmp_pool.tile([128, TW], F32, name="rope_t2")
                nc.vector.stream_shuffle(sw[:], ps, SWAP16)
                nc.vector.tensor_tensor(t1[:], sw[:], Sx, AluOpType.mult)
                nc.vector.tensor_tensor(t2[:], ps, Cx, AluOpType.mult)
                nc.vector.tensor_tensor(dst, t1[:], t2[:], AluOpType.add)

            for tb in range(cfg.NT):
                t0 = (tb % cfg.NB) * TW  # position within batch
                Cx = ropeC[:, t0:t0 + TW]
                Sx = ropeS[:, t0:t0 + TW]

                q_ps = q_psum.tile([128, NHL * TW], F32, name="q_ps")
                k_ps = k_psum.tile([128, TW], F32, name="k_ps")
                vT_ps = v_psum.tile([128, TW], F32, name="vT_ps")
                # keep the whole tokblock's xT resident so k/vT and q run as
                # two dense passes; next tokblock's k/vT pass overlaps this
                # block's RoPE drains instead of stalling on them
                xts = []
                for dc in range(cfg.DC):
                    xt = x_pool.tile([128, TW], BF16, name="x_t")
                    nc.sync.dma_start(
                        out=xt[:],
                        in_=xT_d.ap()[dc * 128:(dc + 1) * 128,
                                      tb * TW:(tb + 1) * TW])
                    xts.append(xt)
                for dc in range(cfg.DC):
                    st = dict(start=(dc == 0), stop=(dc == cfg.DC - 1))
                    nc.tensor.matmul(k_ps[:],
                                     wk_sb[:, dc * HD:(dc + 1) * HD],
                                     xts[dc][:], **st)
                    nc.tensor.matmul(vT_ps[:],
                                     wv_sb[:, dc * HD:(dc + 1) * HD],
                                     xts[dc][:], **st)
                for dc in range(cfg.DC):
                    st = dict(start=(dc == 0), stop=(dc == cfg.DC - 1))
                    for h in range(NHL):
                        nc.tensor.matmul(
                            q_ps[:, h * TW:h * TW + TW],
                            wq_sb[:, dc * QD + h * HD: dc * QD + (h + 1) * HD],
                            xts[dc][:], **st)

                # k: rope -> resident (drain first: next tb needs this bank)
                rope_drain(k_ps[:], kT_sb[:, tb * TW:(tb + 1) * TW])
                # q: rope -> spill to DRAM
                for h in range(NHL):
                    qr = qrot_pool.tile([128, TW], BF16, name="q_rot")
                    rope_drain(q_ps[:, h * TW:h * TW + TW], qr[:])
                    nc.sync.dma_start(
                        out=qT_dram[h * HD:(h + 1) * HD,
                                    tb * TW:(tb + 1) * TW],
                        in_=qr[:])
                # v: vT -> transpose -> resident [ktok, hd] blocks
                vt_sb = vst_pool.tile([128, TW], BF16, name="vT_stage")
                nc.scalar.copy(vt_sb[:], vT_ps[:])
                for i in range(TW // 128):
                    vp = vt_psum.tile([128, 128], BF16, name="v_tr_ps")
                    nc.tensor.transpose(vp[:], vt_sb[:, i * 128:(i + 1) * 128],
                                        ident[:])
                    nc.scalar.copy(
                        v_sb[:, tb * TW + i * 128: tb * TW + (i + 1) * 128],
                        vp[:])

        # ================= phase 2: attention + wo + ReduceScatter =========
        with tc.tile_pool(name="mask", bufs=max(cfg.KT, 4)) as m_pool, \
             tc.tile_pool(name="qload", bufs=3) as q_pool, \
             tc.tile_pool(name="expsb", bufs=4) as e_pool, \
             tc.tile_pool(name="msum", bufs=3) as msk_pool, \
             tc.tile_pool(name="attnsb", bufs=2) as at_pool, \
             tc.tile_pool(name="recsb", bufs=2) as rec_pool, \
             tc.tile_pool(name="outcp", bufs=4) as oc_pool, \
             tc.tile_pool(name="fin", bufs=2) as fin_pool, \
             tc.tile_pool(name="scps", bufs=3, space="PSUM") as sc_psum, \
             tc.tile_pool(name="avps", bufs=2, space="PSUM") as av_psum, \
             tc.tile_pool(name="seps", bufs=1, space="PSUM") as se_psum, \
             tc.tile_pool(name="ops", bufs=2, space="PSUM") as o_psum:

            for qb in range(cfg.NB):
                active = [kt for kt in range(cfg.KT)
                          if cls[kt][qb][0] != 'N']
                # off[kt]: leading fully-masked query columns -> skip them.
                # Force 0 on the first active tile so PSUM start=True
                # initializes every column.
                offs = {kt: cls[kt][qb][1] for kt in active}
                offs[active[0]] = 0
                mtiles = {}
                for kt in active:
                    if cls[kt][qb][0] == 'M':
                        mt = m_pool.tile([KW, TW], BF16, name="m_t")
                        o = offs[kt]
                        nc.sync.dma_start(
                            out=mt[:, o:],
                            in_=maskT_d.ap()[kt * KW:(kt + 1) * KW,
                                             qb * TW + o:(qb + 1) * TW])
                        mtiles[kt] = mt

                for b in range(cfg.B):
                    attn_sb = at_pool.tile([128, NHL * TW], BF16, name="at_sb")
                    for h in range(NHL):
                        qt = q_pool.tile([HD, TW], BF16, name="q_t")
                        nc.sync.dma_start(
                            out=qt[:],
                            in_=qT_dram[h * HD:(h + 1) * HD,
                                        (b * cfg.NB + qb) * TW:
                                        (b * cfg.NB + qb + 1) * TW])
                        at_ps = av_psum.tile([HD, TW], F32, name="at_ps")
                        se_ps = se_psum.tile([128, TW], F32, name="se_ps")
                        # software pipeline: issue score matmuls LOOKAHEAD
                        # iterations ahead so the PE never waits on exp (ACT)
                        LOOKAHEAD = 2
                        n_act = len(active)
                        sc_tiles = [None] * n_act

                        def emit_sc(j):
                            kt2 = active[j]
                            gk2 = b * cfg.L + kt2 * KW
                            o = offs[kt2]
                            sc = sc_psum.tile([KW, TW], F32, name="sc_ps")
                            nc.tensor.matmul(sc[:, o:], kT_sb[:, gk2:gk2 + KW],
                                             qt[:, o:], start=True, stop=True)
                            sc_tiles[j] = sc

                        for j in range(min(LOOKAHEAD, n_act)):
                            emit_sc(j)
                        for idx, kt in enumerate(active):
                            if idx + LOOKAHEAD < n_act:
                                emit_sc(idx + LOOKAHEAD)
                            gk = b * cfg.L + kt * KW  # global key token
                            o = offs[kt]
                            sc_ps = sc_tiles[idx]
                            sc_tiles[idx] = None
                            if cls[kt][qb][0] == 'M':
                                ms = msk_pool.tile([KW, TW], F32, name="ms_t")
                                nc.vector.tensor_tensor(
                                    ms[:, o:], sc_ps[:, o:], mtiles[kt][:, o:],
                                    AluOpType.add)
                                esrc = ms[:, o:]
                            else:
                                esrc = sc_ps[:, o:]
                            ex = e_pool.tile([KW, TW], BF16, name="ex_t")
                            nc.scalar.activation(
                                ex[:, o:], esrc,
                                mybir.ActivationFunctionType.Exp,
                                scale=float(SCALE))
                            st = dict(start=(idx == 0),
                                      stop=(idx == len(active) - 1))
                            nc.tensor.matmul(se_ps[:, o:], ones_sb[:],
                                             ex[:, o:], **st)
                            nc.tensor.matmul(at_ps[:, o:], v_sb[:, gk:gk + KW],
                                             ex[:, o:], **st)
                        rec = rec_pool.tile([128, TW], F32, name="rec_t")
                        nc.vector.reciprocal_approx_fast(rec[:], se_ps[:])
                        nc.vector.tensor_tensor(
                            attn_sb[:, h * TW:(h + 1) * TW],
                            at_ps[:], rec[:], AluOpType.mult)

                    # ---- wo partial for this (b, qb) token block ----------
                    c = b * cfg.NB + qb
                    NG = min(4, cfg.NBLK)  # n-blocks per batched store
                    for m in range(TW // 128):
                        for ng in range(cfg.NBLK // NG):
                            oc = oc_pool.tile([128, NG * TW], BF16,
                                              name="oc_t")
                            for j in range(NG):
                                n = ng * NG + j
                                o_ps = o_psum.tile([128, TW], F32,
                                                   name="o_ps")
                                for h in range(NHL):
                                    nc.tensor.matmul(
                                        o_ps[:],
                                        attn_sb[:, h * TW + m * 128:
                                                h * TW + (m + 1) * 128],
                                        wo_sb[:, h * cfg.D + n * TW:
                                              h * cfg.D + (n + 1) * TW],
                                        start=(h == 0), stop=(h == NHL - 1))
                                # alternate drain engine: keep ACT free
                                dst = oc[:, j * TW:(j + 1) * TW]
                                if n % 2 == 0:
                                    nc.vector.tensor_copy(dst, o_ps[:])
                                else:
                                    nc.scalar.copy(dst, o_ps[:])
                            nc.sync.dma_start(
                                out=rs_in[c][m * 128:(m + 1) * 128,
                                             ng * NG * TW:(ng + 1) * NG * TW],
                                in_=oc[:])
                    nc.gpsimd.collective_compute(
                        "ReduceScatter", AluOpType.add, replica_groups=rg,
                        ins=[rs_in[c][:].opt()], outs=[rs_out[c][:].opt()])

            # ---- program tail: bf16 RS shards -> f32 external output ------
            # (emitted last so the RS-dependent loads/casts never head-of-line
            # block the ACT / sync queues that attention needs)
            rw = TW // N_CORES
            g2 = 128 // rw
            fw = cfg.D // g2
            for c in range(cfg.NCH):
                ocb = fin_pool.tile([128, fw], BF16, name="fin_b")
                ocf = fin_pool.tile([128, fw], F32, name="fin_f")
                nc.sync.dma_start(
                    out=ocb[:],
                    in_=rs_out[c][:].rearrange("r (g f) -> (r g) f", g=g2))
                nc.scalar.copy(ocf[:], ocb[:])
                nc.sync.dma_start(
                    out=out_d.ap()[c * rw:(c + 1) * rw, :]
                    .rearrange("r (g f) -> (r g) f", g=g2),
                    in_=ocf[:])

        w_pool.release()
        dram_pool.release()
        kv_pool.release()
        const_pool.release()

    nc.compile()
    return nc


def host_prepare(cfg, x, mask, wq, wk, wv, wo):
    """Returns (in_maps, cls)."""
    x = np.ascontiguousarray(np.asarray(x, dtype=np.float32))
    mask = np.asarray(mask, dtype=np.float32)
    wq = np.asarray(wq, dtype=np.float32)
    wk = np.asarray(wk, dtype=np.float32)
    wv = np.asarray(wv, dtype=np.float32)
    wo = np.asarray(wo, dtype=np.float32)

    import ml_dtypes
    bf16 = ml_dtypes.bfloat16
    perm = _rope_perm()
    C, S = _rope_tables(cfg)
    xT = np.ascontiguousarray(x.reshape(cfg.BL, cfg.D).T).astype(bf16)
    maskT = np.ascontiguousarray(mask.T / SCALE).astype(bf16)
    cls = classify_mask(mask, cfg)

    in_maps = []
    for g in range(N_CORES):
        qrows = wq[g * NHL * HD:(g + 1) * NHL * HD]          # [512, D]
        qperm = np.concatenate(
            [qrows[h * HD + perm] for h in range(NHL)], axis=0)
        krows = wk[g * HD:(g + 1) * HD][perm]                # [128, D]
        vrows = wv[g * HD:(g + 1) * HD]                      # [128, D]
        wocols = wo[:, g * NHL * HD:(g + 1) * NHL * HD]      # [D, 512]
        in_maps.append({
            "xT": xT,
            "wqT": np.ascontiguousarray(qperm.T).astype(bf16),
            "wkT": np.ascontiguousarray(krows.T).astype(bf16),
            "wvT": np.ascontiguousarray(vrows.T).astype(bf16),
            "woT": np.ascontiguousarray(wocols.T).astype(bf16),
            "maskT": maskT,
            "ropeC": C,
            "ropeS": S,
        })
    return in_maps, cls


def assemble_output(cfg, results):
    """Stitch per-core ReduceScatter shards back into [B, L, D]."""
    full = np.empty((cfg.BL, cfg.D), dtype=np.float32)
    rw = TW // N_CORES
    for g in range(N_CORES):
        r = results[g]["out"]
        for c in range(cfg.NCH):
            full[c * TW + g * rw: c * TW + (g + 1) * rw] = \
                r[c * rw:(c + 1) * rw]
    return full.reshape(cfg.B, cfg.L, cfg.D)


def kernel(x, mask, wq, wk, wv, wo):
    global LAST_RESULTS
    from concourse.bass_utils import run_bass_kernel_spmd
    cfg = Cfg(B=2, L=2048, D=4096)
    in_maps, cls = host_prepare(cfg, x, mask, wq, wk, wv, wo)
    nc = build_bass(cfg, cls)
    res = run_bass_kernel_spmd(nc, in_maps, core_ids=list(range(N_CORES)),
                               trace=TRACE)
    LAST_RESULTS = res
    return assemble_output(cfg, res.results)


# revision 36
# speedup vs baseline: 3.4514x; 1.0162x over previous
"""Distributed GQA attention kernel for one TRN2 chip (8 NeuronCores).

Sharding: tensor-parallel over heads. Core g owns query heads [4g, 4g+4)
and kv head g. Each core computes its heads' attention and a partial
output projection; a chunked ReduceScatter sums the partials and leaves
each core with a 1/8 token-slice of the final output.

All device tensors are laid out so that no on-device transposes of the
big activations are needed:
  - x is passed pre-transposed (xT [D, B*L]) so projections contract D
    on the partition axis.
  - q/k are produced directly as qT/kT [head_dim, tokens]; scores are
    computed keys-on-partitions, so the P@V matmul consumes exp(scores)
    directly and the wo matmul consumes the attention output directly.
  - RoPE head_dim pairs are permuted (on the host, into wq/wk rows) so
    each rotation partner lives 16 partitions away within a 32-partition
    quadrant -> one DVE stream_shuffle does the swap.
  - softmax denominator comes from an all-ones matmul (partition
    broadcast for free); no max subtraction (fp32 logits here are <~15).
"""

import numpy as np

import concourse.bass as bass
import concourse.mybir as mybir
import concourse.tile as tile
from concourse import bacc
from concourse.alu_op_type import AluOpType
from concourse.masks import make_identity

F32 = mybir.dt.float32
BF16 = mybir.dt.bfloat16

N_CORES = 8
NHL = 4           # local q heads per core
HD = 128          # head dim
THETA = 10000.0
SCALE = HD ** -0.5
TW = 512          # token block width (free dim of most matmuls)
KW = 128          # key tile width (partition dim of score tiles)

# module-level knobs for test.py
TRACE = False
LAST_RESULTS = None


class Cfg:
    def __init__(self, B=2, L=2048, D=4096):
        self.B, self.L, self.D = B, L, D
        self.BL = B * L
        self.DC = D // 128         # contraction chunks for projections
        self.NB = L // TW          # query blocks per batch
        self.NT = self.BL // TW    # token blocks total
        self.KT = L // KW          # key tiles per batch
        self.NBLK = D // TW        # wo output column blocks
        self.NCH = self.NT         # ReduceScatter chunks (one per token block)
        assert self.BL % TW == 0 and TW % N_CORES == 0


# stream_shuffle mask: swap 16-partition halves within each 32-partition quadrant
SWAP16 = [(i + 16) % 32 for i in range(32)]


def _rope_perm():
    """Permutation of head_dim rows: pair i=(16q + r) lives at partitions
    32q+r (x1 = even dim 2i) and 32q+16+r (x2 = odd dim 2i+1)."""
    perm = np.zeros(HD, dtype=np.int64)
    for p in range(HD):
        q, r = divmod(p, 32)
        i = 16 * q + (r % 16)
        perm[p] = 2 * i + (0 if r < 16 else 1)
    return perm


def _rope_tables(cfg):
    """cosT/sinT [128, L] in the permuted-partition layout, sin sign-folded."""
    perm = _rope_perm()
    t = np.arange(cfg.L, dtype=np.float64)
    freqs = THETA ** (-np.arange(0, HD, 2, dtype=np.float64) / HD)  # [64]
    theta = t[None, :] * freqs[:, None]                             # [64, L]
    cos, sin = np.cos(theta), np.sin(theta)
    C = np.zeros((HD, cfg.L), dtype=np.float32)
    S = np.zeros((HD, cfg.L), dtype=np.float32)
    for p in range(HD):
        q, r = divmod(p, 32)
        i = 16 * q + (r % 16)
        C[p] = cos[i]
        S[p] = sin[i] if r >= 16 else -sin[i]
    return C, S


def classify_mask(mask, cfg):
    """cls[kt][qb] = (kind, off): kind in {'Z','N','M'} for tile
    mask[qb*TW:(qb+1)*TW, kt*KW:(kt+1)*KW]; off = count of leading query
    columns in the tile that are fully masked (safe to skip: exp would
    be exactly 0 there)."""
    cls = [[None] * cfg.NB for _ in range(cfg.KT)]
    for kt in range(cfg.KT):
        for qb in range(cfg.NB):
            t = mask[qb * TW:(qb + 1) * TW, kt * KW:(kt + 1) * KW]
            if np.all(t == 0.0):
                cls[kt][qb] = ('Z', 0)
            elif np.all(t <= -1e8):
                cls[kt][qb] = ('N', 0)
            else:
                dead_q = np.all(t <= -1e8, axis=1)  # [TW]
                off = 0
                while off < len(dead_q) and dead_q[off]:
                    off += 1
                off = (off // 64) * 64  # keep offsets 64-aligned
                cls[kt][qb] = ('M', off)
    # guard: every query block must attend to at least one key tile
    for qb in range(cfg.NB):
        assert any(cls[kt][qb][0] != 'N' for kt in range(cfg.KT)), \
            "fully-masked query block unsupported"
    return cls


def build_bass(cfg, cls):
    nc = bacc.Bacc("TRN2", target_bir_lowering=False, debug=False,
                   num_devices=N_CORES)

    xT_d = nc.dram_tensor("xT", [cfg.D, cfg.BL], BF16, kind="ExternalInput")
    wqT_d = nc.dram_tensor("wqT", [cfg.D, NHL * HD], BF16, kind="ExternalInput")
    wkT_d = nc.dram_tensor("wkT", [cfg.D, HD], BF16, kind="ExternalInput")
    wvT_d = nc.dram_tensor("wvT", [cfg.D, HD], BF16, kind="ExternalInput")
    woT_d = nc.dram_tensor("woT", [NHL * HD, cfg.D], BF16, kind="ExternalInput")
    maskT_d = nc.dram_tensor("maskT", [cfg.L, cfg.L], BF16, kind="ExternalInput")
    ropeC_d = nc.dram_tensor("ropeC", [HD, cfg.L], F32, kind="ExternalInput")
    ropeS_d = nc.dram_tensor("ropeS", [HD, cfg.L], F32, kind="ExternalInput")
    out_d = nc.dram_tensor("out", [cfg.BL // N_CORES, cfg.D], F32,
                           kind="ExternalOutput")

    rg = [list(range(N_CORES))]
    QD = NHL * HD  # 512

    with tile.TileContext(nc) as tc:
        # ---- constants / tables -------------------------------------------
        const_pool = tc.alloc_tile_pool(name="const", bufs=1)
        ones_sb = const_pool.tile([128, 128], BF16, name="ones_sb")
        nc.vector.memset(ones_sb[:], 1.0)
        ident = const_pool.tile([128, 128], BF16, name="ident")
        make_identity(nc, ident[:])

        # ---- resident activations -----------------------------------------
        kv_pool = tc.alloc_tile_pool(name="kv", bufs=1)
        kT_sb = kv_pool.tile([HD, cfg.BL], BF16, name="kT_sb")
        v_sb = kv_pool.tile([128, cfg.BL], BF16, name="v_sb")

        # DRAM scratch
        dram_pool = tc.alloc_tile_pool(name="dram", bufs=1, space="DRAM")
        qT_dram = dram_pool.tile([QD, cfg.BL], BF16, name="qT_dram")
        rs_in = [dram_pool.tile([TW, cfg.D], BF16, name=f"rs_in{c}")
                 for c in range(cfg.NCH)]
        rs_out = [dram_pool.tile([TW // N_CORES, cfg.D], BF16,
                                 name=f"rs_out{c}")
                  for c in range(cfg.NCH)]

        # ---- all weights resident upfront (bf16 halves the footprint) -----
        # Few, ~0.5-1MB DMAs (sequencer dispatch is ~0.6us each); k/v first
        # so the first tokblock's matmuls can start ASAP; wo on the scalar
        # queue so it never delays the x stream.
        w_pool = tc.alloc_tile_pool(name="weights", bufs=1)
        wq_sb = w_pool.tile([128, cfg.DC * QD], BF16, name="wq_sb")
        wk_sb = w_pool.tile([128, cfg.DC * HD], BF16, name="wk_sb")
        wv_sb = w_pool.tile([128, cfg.DC * HD], BF16, name="wv_sb")
        wo_sb = w_pool.tile([128, NHL * cfg.D], BF16, name="wo_sb")

        def load_w3d(eng, dst, src_d, width, chunk):
            """dst[:, dc*width+c] = src[dc*128+p, c], batched `chunk` dcs/DMA."""
            for d0 in range(0, cfg.DC, chunk):
                d1 = min(d0 + chunk, cfg.DC)
                eng.dma_start(
                    out=dst[:, d0 * width:d1 * width]
                    .rearrange("p (dc c) -> p dc c", dc=d1 - d0),
                    in_=src_d.ap()[d0 * 128:d1 * 128, :]
                    .rearrange("(dc p) c -> p dc c", p=128))

        load_w3d(nc.sync, wk_sb, wkT_d, HD, 8)     # 4 DMAs of 512KB
        load_w3d(nc.sync, wv_sb, wvT_d, HD, 8)
        load_w3d(nc.sync, wq_sb, wqT_d, QD, 4)     # 8 DMAs of 1MB
        for h in range(NHL):                       # 4 DMAs of 1MB (ACT queue)
            nc.scalar.dma_start(out=wo_sb[:, h * cfg.D:(h + 1) * cfg.D],
                                in_=woT_d.ap()[h * HD:(h + 1) * HD, :])

        # ================= phase 1: QKV projections + RoPE =================
        with tc.tile_pool(name="xload", bufs=cfg.DC // 4 + 2) as x_pool, \
             tc.tile_pool(name="qpsum", bufs=1, space="PSUM") as q_psum, \
             tc.tile_pool(name="kpsum", bufs=2, space="PSUM") as k_psum, \
             tc.tile_pool(name="vpsum", bufs=1, space="PSUM") as v_psum, \
             tc.tile_pool(name="ropetmp", bufs=3) as rtmp_pool, \
             tc.tile_pool(name="qrot", bufs=3) as qrot_pool, \
             tc.tile_pool(name="vstage", bufs=2) as vst_pool, \
             tc.tile_pool(name="vtpsum", bufs=1, space="PSUM") as vt_psum, \
             tc.tile_pool(name="ropetbl", bufs=1) as rtbl_pool:

            ropeC = rtbl_pool.tile([HD, cfg.L], F32, name="ropeC_sb")
            ropeS = rtbl_pool.tile([HD, cfg.L], F32, name="ropeS_sb")
            nc.scalar.dma_start(out=ropeC[:], in_=ropeC_d.ap())
            nc.scalar.dma_start(out=ropeS[:], in_=ropeS_d.ap())

            def rope_drain(ps, dst):
                """dst = ps*C + shuffle16(ps)*S at token offset t0 (len TW)."""
                sw = rtmp_pool.tile([128, TW], F32, name="rope_sw")
                t1 = rtmp_pool.tile([128, TW], F32, name="rope_t1")
                t2 = rtmp_pool.tile([128, TW], F32, name="rope_t2")
                nc.vector.stream_shuffle(sw[:], ps, SWAP16)
                nc.vector.tensor_tensor(t1[:], sw[:], Sx, AluOpType.mult)
                nc.vector.tensor_tensor(t2[:], ps, Cx, AluOpType.mult)
                nc.vector.tensor_tensor(dst, t1[:], t2[:], AluOpType.add)

            for tb in range(cfg.NT):
                t0 = (tb % cfg.NB) * TW  # position within batch
                Cx = ropeC[:, t0:t0 + TW]
                Sx = ropeS[:, t0:t0 + TW]

                q_ps = q_psum.tile([128, NHL * TW], F32, name="q_ps")
                k_ps = k_psum.tile([128, TW], F32, name="k_ps")
                vT_ps = v_psum.tile([128, TW], F32, name="vT_ps")
                # keep the whole tokblock's xT resident so k/vT and q run as
                # two dense passes; next tokblock's k/vT pass overlaps this
                # block's RoPE drains instead of stalling on them
                xts = []
                XB = 4  # dc-chunks per DMA (~512KB each)
                for dc in range(0, cfg.DC, XB):
                    d1 = min(dc + XB, cfg.DC)
                    xt = x_pool.tile([128, (d1 - dc) * TW], BF16, name="x_t")
                    nc.sync.dma_start(
                        out=xt[:].rearrange("p (dc t) -> p dc t", dc=d1 - dc),
                        in_=xT_d.ap()[dc * 128:d1 * 128,
                                      tb * TW:(tb + 1) * TW]
                        .rearrange("(dc p) t -> p dc t", p=128))
                    for j in range(d1 - dc):
                        xts.append(xt[:, j * TW:(j + 1) * TW])
                for dc in range(cfg.DC):
                    st = dict(start=(dc == 0), stop=(dc == cfg.DC - 1))
                    nc.tensor.matmul(k_ps[:],
                                     wk_sb[:, dc * HD:(dc + 1) * HD],
                                     xts[dc], **st)
                    nc.tensor.matmul(vT_ps[:],
                                     wv_sb[:, dc * HD:(dc + 1) * HD],
                                     xts[dc], **st)
                for dc in range(cfg.DC):
                    st = dict(start=(dc == 0), stop=(dc == cfg.DC - 1))
                    for h in range(NHL):
                        nc.tensor.matmul(
                            q_ps[:, h * TW:h * TW + TW],
                            wq_sb[:, dc * QD + h * HD: dc * QD + (h + 1) * HD],
                            xts[dc], **st)

                # k: rope -> resident (drain first: next tb needs this bank)
                rope_drain(k_ps[:], kT_sb[:, tb * TW:(tb + 1) * TW])
                # q: rope -> spill to DRAM (one batched DMA per tokblock)
                qr = qrot_pool.tile([128, NHL * TW], BF16, name="q_rot")
                for h in range(NHL):
                    rope_drain(q_ps[:, h * TW:h * TW + TW],
                               qr[:, h * TW:(h + 1) * TW])
                nc.sync.dma_start(
                    out=qT_dram[0:QD, tb * TW:(tb + 1) * TW]
                    .rearrange("(h p) t -> p h t", p=128),
                    in_=qr[:].rearrange("p (h t) -> p h t", h=NHL))
                # v: vT -> transpose -> resident [ktok, hd] blocks
                vt_sb = vst_pool.tile([128, TW], BF16, name="vT_stage")
                nc.scalar.copy(vt_sb[:], vT_ps[:])
                for i in range(TW // 128):
                    vp = vt_psum.tile([128, 128], BF16, name="v_tr_ps")
                    nc.tensor.transpose(vp[:], vt_sb[:, i * 128:(i + 1) * 128],
                                        ident[:])
                    nc.scalar.copy(
                        v_sb[:, tb * TW + i * 128: tb * TW + (i + 1) * 128],
                        vp[:])

        # ================= phase 2: attention + wo + ReduceScatter =========
        with tc.tile_pool(name="mask", bufs=max(cfg.KT, 4)) as m_pool, \
             tc.tile_pool(name="qload", bufs=3) as q_pool, \
             tc.tile_pool(name="expsb", bufs=4) as e_pool, \
             tc.tile_pool(name="msum", bufs=3) as msk_pool, \
             tc.tile_pool(name="attnsb", bufs=2) as at_pool, \
             tc.tile_pool(name="recsb", bufs=2) as rec_pool, \
             tc.tile_pool(name="outcp", bufs=4) as oc_pool, \
             tc.tile_pool(name="fin", bufs=2) as fin_pool, \
             tc.tile_pool(name="scps", bufs=3, space="PSUM") as sc_psum, \
             tc.tile_pool(name="avps", bufs=2, space="PSUM") as av_psum, \
             tc.tile_pool(name="seps", bufs=1, space="PSUM") as se_psum, \
             tc.tile_pool(name="ops", bufs=2, space="PSUM") as o_psum:

            for qb in range(cfg.NB):
                active = [kt for kt in range(cfg.KT)
                          if cls[kt][qb][0] != 'N']
                # off[kt]: leading fully-masked query columns -> skip them.
                # Force 0 on the first active tile so PSUM start=True
                # initializes every column.
                offs = {kt: cls[kt][qb][1] for kt in active}
                offs[active[0]] = 0
                mtiles = {}
                for kt in active:
                    if cls[kt][qb][0] == 'M':
                        mt = m_pool.tile([KW, TW], BF16, name="m_t")
                        o = offs[kt]
                        nc.sync.dma_start(
                            out=mt[:, o:],
                            in_=maskT_d.ap()[kt * KW:(kt + 1) * KW,
                                             qb * TW + o:(qb + 1) * TW])
                        mtiles[kt] = mt

                for b in range(cfg.B):
                    attn_sb = at_pool.tile([128, NHL * TW], BF16, name="at_sb")
                    qt_all = q_pool.tile([128, NHL * TW], BF16, name="q_t")
                    tb2 = b * cfg.NB + qb
                    nc.sync.dma_start(
                        out=qt_all[:].rearrange("p (h t) -> p h t", h=NHL),
                        in_=qT_dram[0:QD, tb2 * TW:(tb2 + 1) * TW]
                        .rearrange("(h p) t -> p h t", p=128))
                    for h in range(NHL):
                        qt = qt_all[:, h * TW:(h + 1) * TW]
                        at_ps = av_psum.tile([HD, TW], F32, name="at_ps")
                        se_ps = se_psum.tile([128, TW], F32, name="se_ps")
                        # software pipeline: issue score matmuls LOOKAHEAD
                        # iterations ahead so the PE never waits on exp (ACT)
                        LOOKAHEAD = 2
                        n_act = len(active)
                        sc_tiles = [None] * n_act

                        def emit_sc(j):
                            kt2 = active[j]
                            gk2 = b * cfg.L + kt2 * KW
                            o = offs[kt2]
                            sc = sc_psum.tile([KW, TW], F32, name="sc_ps")
                            nc.tensor.matmul(sc[:, o:], kT_sb[:, gk2:gk2 + KW],
                                             qt[:, o:], start=True, stop=True)
                            sc_tiles[j] = sc

                        for j in range(min(LOOKAHEAD, n_act)):
                            emit_sc(j)
                        for idx, kt in enumerate(active):
                            if idx + LOOKAHEAD < n_act:
                                emit_sc(idx + LOOKAHEAD)
                            gk = b * cfg.L + kt * KW  # global key token
                            o = offs[kt]
                            sc_ps = sc_tiles[idx]
                            sc_tiles[idx] = None
                            if cls[kt][qb][0] == 'M':
                                ms = msk_pool.tile([KW, TW], F32, name="ms_t")
                                nc.vector.tensor_tensor(
                                    ms[:, o:], sc_ps[:, o:], mtiles[kt][:, o:],
                                    AluOpType.add)
                                esrc = ms[:, o:]
                            else:
                                esrc = sc_ps[:, o:]
                            ex = e_pool.tile([KW, TW], BF16, name="ex_t")
                            nc.scalar.activation(
                                ex[:, o:], esrc,
                                mybir.ActivationFunctionType.Exp,
                                scale=float(SCALE))
                            st = dict(start=(idx == 0),
                                      stop=(idx == len(active) - 1))
                            nc.tensor.matmul(se_ps[:, o:], ones_sb[:],
                                             ex[:, o:], **st)
                            nc.tensor.matmul(at_ps[:, o:], v_sb[:, gk:gk + KW],
                                             ex[:, o:], **st)
                        rec = rec_pool.tile([128, TW], F32, name="rec_t")
                        nc.vector.reciprocal_approx_fast(rec[:], se_ps[:])
                        nc.vector.tensor_tensor(
                            attn_sb[:, h * TW:(h + 1) * TW],
                            at_ps[:], rec[:], AluOpType.mult)

                    # ---- wo partial for this (b, qb) token block ----------
                    c = b * cfg.NB + qb
                    NG = min(4, cfg.NBLK)  # n-blocks per batched store
                    for m in range(TW // 128):
                        for ng in range(cfg.NBLK // NG):
                            oc = oc_pool.tile([128, NG * TW], BF16,
                                              name="oc_t")
                            for j in range(NG):
                                n = ng * NG + j
                                o_ps = o_psum.tile([128, TW], F32,
                                                   name="o_ps")
                                for h in range(NHL):
                                    nc.tensor.matmul(
                                        o_ps[:],
                                        attn_sb[:, h * TW + m * 128:
                                                h * TW + (m + 1) * 128],
                                        wo_sb[:, h * cfg.D + n * TW:
                                              h * cfg.D + (n + 1) * TW],
                                        start=(h == 0), stop=(h == NHL - 1))
                                # alternate drain engine: keep ACT free
                                dst = oc[:, j * TW:(j + 1) * TW]
                                if n % 2 == 0:
                                    nc.vector.tensor_copy(dst, o_ps[:])
                                else:
                                    nc.scalar.copy(dst, o_ps[:])
                            nc.sync.dma_start(
                                out=rs_in[c][m * 128:(m + 1) * 128,
                                             ng * NG * TW:(ng + 1) * NG * TW],
                                in_=oc[:])
                    nc.gpsimd.collective_compute(
                        "ReduceScatter", AluOpType.add, replica_groups=rg,
                        ins=[rs_in[c][:].opt()], outs=[rs_out[c][:].opt()])

            # ---- program tail: bf16 RS shards -> f32 external output ------
            # (emitted last so the RS-dependent loads/casts never head-of-line
            # block the ACT / sync queues that attention needs)
            rw = TW // N_CORES
            g2 = 128 // rw
            fw = cfg.D // g2
            for c in range(cfg.NCH):
                ocb = fin_pool.tile([128, fw], BF16, name="fin_b")
                ocf = fin_pool.tile([128, fw], F32, name="fin_f")
                nc.sync.dma_start(
                    out=ocb[:],
                    in_=rs_out[c][:].rearrange("r (g f) -> (r g) f", g=g2))
                nc.scalar.copy(ocf[:], ocb[:])
                nc.sync.dma_start(
                    out=out_d.ap()[c * rw:(c + 1) * rw, :]
                    .rearrange("r (g f) -> (r g) f", g=g2),
                    in_=ocf[:])

        w_pool.release()
        dram_pool.release()
        kv_pool.release()
        const_pool.release()

    nc.compile()
    return nc


def host_prepare(cfg, x, mask, wq, wk, wv, wo):
    """Returns (in_maps, cls)."""
    x = np.ascontiguousarray(np.asarray(x, dtype=np.float32))
    mask = np.asarray(mask, dtype=np.float32)
    wq = np.asarray(wq, dtype=np.float32)
    wk = np.asarray(wk, dtype=np.float32)
    wv = np.asarray(wv, dtype=np.float32)
    wo = np.asarray(wo, dtype=np.float32)

    import ml_dtypes
    bf16 = ml_dtypes.bfloat16
    perm = _rope_perm()
    C, S = _rope_tables(cfg)
    xT = np.ascontiguousarray(x.reshape(cfg.BL, cfg.D).T).astype(bf16)
    maskT = np.ascontiguousarray(mask.T / SCALE).astype(bf16)
    cls = classify_mask(mask, cfg)

    in_maps = []
    for g in range(N_CORES):
        qrows = wq[g * NHL * HD:(g + 1) * NHL * HD]          # [512, D]
        qperm = np.concatenate(
            [qrows[h * HD + perm] for h in range(NHL)], axis=0)
        krows = wk[g * HD:(g + 1) * HD][perm]                # [128, D]
        vrows = wv[g * HD:(g + 1) * HD]                      # [128, D]
        wocols = wo[:, g * NHL * HD:(g + 1) * NHL * HD]      # [D, 512]
        in_maps.append({
            "xT": xT,
            "wqT": np.ascontiguousarray(qperm.T).astype(bf16),
            "wkT": np.ascontiguousarray(krows.T).astype(bf16),
            "wvT": np.ascontiguousarray(vrows.T).astype(bf16),
            "woT": np.ascontiguousarray(wocols.T).astype(bf16),
            "maskT": maskT,
            "ropeC": C,
            "ropeS": S,
        })
    return in_maps, cls


def assemble_output(cfg, results):
    """Stitch per-core ReduceScatter shards back into [B, L, D]."""
    full = np.empty((cfg.BL, cfg.D), dtype=np.float32)
    rw = TW // N_CORES
    for g in range(N_CORES):
        r = results[g]["out"]
        for c in range(cfg.NCH):
            full[c * TW + g * rw: c * TW + (g + 1) * rw] = \
                r[c * rw:(c + 1) * rw]
    return full.reshape(cfg.B, cfg.L, cfg.D)


def kernel(x, mask, wq, wk, wv, wo):
    global LAST_RESULTS
    from concourse.bass_utils import run_bass_kernel_spmd
    cfg = Cfg(B=2, L=2048, D=4096)
    in_maps, cls = host_prepare(cfg, x, mask, wq, wk, wv, wo)
    nc = build_bass(cfg, cls)
    res = run_bass_kernel_spmd(nc, in_maps, core_ids=list(range(N_CORES)),
                               trace=TRACE)
    LAST_RESULTS = res
    return assemble_output(cfg, res.results)


# revision 41
# speedup vs baseline: 3.4833x; 1.0092x over previous
"""Distributed GQA attention kernel for one TRN2 chip (8 NeuronCores).

Sharding: tensor-parallel over heads. Core g owns query heads [4g, 4g+4)
and kv head g. Each core computes its heads' attention and a partial
output projection; a chunked ReduceScatter sums the partials and leaves
each core with a 1/8 token-slice of the final output.

All device tensors are laid out so that no on-device transposes of the
big activations are needed:
  - x is passed pre-transposed (xT [D, B*L]) so projections contract D
    on the partition axis.
  - q/k are produced directly as qT/kT [head_dim, tokens]; scores are
    computed keys-on-partitions, so the P@V matmul consumes exp(scores)
    directly and the wo matmul consumes the attention output directly.
  - RoPE head_dim pairs are permuted (on the host, into wq/wk rows) so
    each rotation partner lives 16 partitions away within a 32-partition
    quadrant -> one DVE stream_shuffle does the swap.
  - softmax denominator comes from an all-ones matmul (partition
    broadcast for free); no max subtraction (fp32 logits here are <~15).
"""

import numpy as np

import concourse.bass as bass
import concourse.mybir as mybir
import concourse.tile as tile
from concourse import bacc
from concourse.alu_op_type import AluOpType
from concourse.masks import make_identity

F32 = mybir.dt.float32
BF16 = mybir.dt.bfloat16

N_CORES = 8
NHL = 4           # local q heads per core
HD = 128          # head dim
THETA = 10000.0
SCALE = HD ** -0.5
TW = 512          # token block width (free dim of most matmuls)
KW = 128          # key tile width (partition dim of score tiles)

# module-level knobs for test.py
TRACE = False
LAST_RESULTS = None


class Cfg:
    def __init__(self, B=2, L=2048, D=4096):
        self.B, self.L, self.D = B, L, D
        self.BL = B * L
        self.DC = D // 128         # contraction chunks for projections
        self.NB = L // TW          # query blocks per batch
        self.NT = self.BL // TW    # token blocks total
        self.KT = L // KW          # key tiles per batch
        self.NBLK = D // TW        # wo output column blocks
        self.NCH = self.NT         # ReduceScatter chunks (one per token block)
        assert self.BL % TW == 0 and TW % N_CORES == 0


# stream_shuffle mask: swap 16-partition halves within each 32-partition quadrant
SWAP16 = [(i + 16) % 32 for i in range(32)]


def _rope_perm():
    """Permutation of head_dim rows: pair i=(16q + r) lives at partitions
    32q+r (x1 = even dim 2i) and 32q+16+r (x2 = odd dim 2i+1)."""
    perm = np.zeros(HD, dtype=np.int64)
    for p in range(HD):
        q, r = divmod(p, 32)
        i = 16 * q + (r % 16)
        perm[p] = 2 * i + (0 if r < 16 else 1)
    return perm


def _rope_tables(cfg):
    """cosT/sinT [128, L] in the permuted-partition layout, sin sign-folded."""
    perm = _rope_perm()
    t = np.arange(cfg.L, dtype=np.float64)
    freqs = THETA ** (-np.arange(0, HD, 2, dtype=np.float64) / HD)  # [64]
    theta = t[None, :] * freqs[:, None]                             # [64, L]
    cos, sin = np.cos(theta), np.sin(theta)
    C = np.zeros((HD, cfg.L), dtype=np.float32)
    S = np.zeros((HD, cfg.L), dtype=np.float32)
    for p in range(HD):
        q, r = divmod(p, 32)
        i = 16 * q + (r % 16)
        C[p] = cos[i]
        S[p] = sin[i] if r >= 16 else -sin[i]
    return C, S


def classify_mask(mask, cfg):
    """cls[kt][qb] = (kind, off): kind in {'Z','N','M'} for tile
    mask[qb*TW:(qb+1)*TW, kt*KW:(kt+1)*KW]; off = count of leading query
    columns in the tile that are fully masked (safe to skip: exp would
    be exactly 0 there)."""
    cls = [[None] * cfg.NB for _ in range(cfg.KT)]
    for kt in range(cfg.KT):
        for qb in range(cfg.NB):
            t = mask[qb * TW:(qb + 1) * TW, kt * KW:(kt + 1) * KW]
            if np.all(t == 0.0):
                cls[kt][qb] = ('Z', 0)
            elif np.all(t <= -1e8):
                cls[kt][qb] = ('N', 0)
            else:
                dead_q = np.all(t <= -1e8, axis=1)  # [TW]
                off = 0
                while off < len(dead_q) and dead_q[off]:
                    off += 1
                off = (off // 64) * 64  # keep offsets 64-aligned
                cls[kt][qb] = ('M', off)
    # guard: every query block must attend to at least one key tile
    for qb in range(cfg.NB):
        assert any(cls[kt][qb][0] != 'N' for kt in range(cfg.KT)), \
            "fully-masked query block unsupported"
    return cls


def build_bass(cfg, cls):
    nc = bacc.Bacc("TRN2", target_bir_lowering=False, debug=False,
                   num_devices=N_CORES)

    xT_d = nc.dram_tensor("xT", [cfg.D, cfg.BL], BF16, kind="ExternalInput")
    wqT_d = nc.dram_tensor("wqT", [cfg.D, NHL * HD], BF16, kind="ExternalInput")
    wkT_d = nc.dram_tensor("wkT", [cfg.D, HD], BF16, kind="ExternalInput")
    wvT_d = nc.dram_tensor("wvT", [cfg.D, HD], BF16, kind="ExternalInput")
    woT_d = nc.dram_tensor("woT", [NHL * HD, cfg.D], BF16, kind="ExternalInput")
    maskT_d = nc.dram_tensor("maskT", [cfg.L, cfg.L], BF16, kind="ExternalInput")
    ropeC_d = nc.dram_tensor("ropeC", [HD, cfg.L], F32, kind="ExternalInput")
    ropeS_d = nc.dram_tensor("ropeS", [HD, cfg.L], F32, kind="ExternalInput")
    out_d = nc.dram_tensor("out", [cfg.BL // N_CORES, cfg.D], F32,
                           kind="ExternalOutput")

    rg = [list(range(N_CORES))]
    QD = NHL * HD  # 512

    with tile.TileContext(nc) as tc:
        # ---- constants / tables -------------------------------------------
        const_pool = tc.alloc_tile_pool(name="const", bufs=1)
        ones_sb = const_pool.tile([128, 128], BF16, name="ones_sb")
        nc.vector.memset(ones_sb[:], 1.0)
        ident = const_pool.tile([128, 128], BF16, name="ident")
        make_identity(nc, ident[:])

        # ---- resident activations -----------------------------------------
        kv_pool = tc.alloc_tile_pool(name="kv", bufs=1)
        kT_sb = kv_pool.tile([HD, cfg.BL], BF16, name="kT_sb")
        v_sb = kv_pool.tile([128, cfg.BL], BF16, name="v_sb")

        # DRAM scratch
        dram_pool = tc.alloc_tile_pool(name="dram", bufs=1, space="DRAM")
        qT_dram = dram_pool.tile([QD, cfg.BL], BF16, name="qT_dram")
        rs_in = [dram_pool.tile([TW, cfg.D], BF16, name=f"rs_in{c}")
                 for c in range(cfg.NCH)]
        rs_out = [dram_pool.tile([TW // N_CORES, cfg.D], BF16,
                                 name=f"rs_out{c}")
                  for c in range(cfg.NCH)]

        # ---- all weights resident upfront (bf16 halves the footprint) -----
        # Few, ~0.5-1MB DMAs (sequencer dispatch is ~0.6us each); k/v first
        # so the first tokblock's matmuls can start ASAP; wo on the scalar
        # queue so it never delays the x stream.
        w_pool = tc.alloc_tile_pool(name="weights", bufs=1)
        wq_sb = w_pool.tile([128, cfg.DC * QD], BF16, name="wq_sb")
        wk_sb = w_pool.tile([128, cfg.DC * HD], BF16, name="wk_sb")
        wv_sb = w_pool.tile([128, cfg.DC * HD], BF16, name="wv_sb")
        wo_sb = w_pool.tile([128, NHL * cfg.D], BF16, name="wo_sb")

        def load_w3d(eng, dst, src_d, width, chunk):
            """dst[:, dc*width+c] = src[dc*128+p, c], batched `chunk` dcs/DMA."""
            for d0 in range(0, cfg.DC, chunk):
                d1 = min(d0 + chunk, cfg.DC)
                eng.dma_start(
                    out=dst[:, d0 * width:d1 * width]
                    .rearrange("p (dc c) -> p dc c", dc=d1 - d0),
                    in_=src_d.ap()[d0 * 128:d1 * 128, :]
                    .rearrange("(dc p) c -> p dc c", p=128))

        load_w3d(nc.sync, wk_sb, wkT_d, HD, 8)     # 4 DMAs of 512KB
        load_w3d(nc.sync, wv_sb, wvT_d, HD, 8)
        load_w3d(nc.sync, wq_sb, wqT_d, QD, 4)     # 8 DMAs of 1MB
        for h in range(NHL):                       # 4 DMAs of 1MB (ACT queue)
            nc.scalar.dma_start(out=wo_sb[:, h * cfg.D:(h + 1) * cfg.D],
                                in_=woT_d.ap()[h * HD:(h + 1) * HD, :])

        # ================= phase 1: QKV projections + RoPE =================
        with tc.tile_pool(name="xload", bufs=cfg.DC // 4 + 2) as x_pool, \
             tc.tile_pool(name="qpsum", bufs=1, space="PSUM") as q_psum, \
             tc.tile_pool(name="kpsum", bufs=2, space="PSUM") as k_psum, \
             tc.tile_pool(name="vpsum", bufs=1, space="PSUM") as v_psum, \
             tc.tile_pool(name="ropetmp", bufs=3) as rtmp_pool, \
             tc.tile_pool(name="qrot", bufs=3) as qrot_pool, \
             tc.tile_pool(name="vstage", bufs=2) as vst_pool, \
             tc.tile_pool(name="vtpsum", bufs=1, space="PSUM") as vt_psum, \
             tc.tile_pool(name="ropetbl", bufs=1) as rtbl_pool:

            ropeC = rtbl_pool.tile([HD, cfg.L], F32, name="ropeC_sb")
            ropeS = rtbl_pool.tile([HD, cfg.L], F32, name="ropeS_sb")
            nc.scalar.dma_start(out=ropeC[:], in_=ropeC_d.ap())
            nc.scalar.dma_start(out=ropeS[:], in_=ropeS_d.ap())

            def rope_drain(ps, dst):
                """dst = ps*C + shuffle16(ps)*S at token offset t0 (len TW)."""
                sw = rtmp_pool.tile([128, TW], F32, name="rope_sw")
                t1 = rtmp_pool.tile([128, TW], F32, name="rope_t1")
                t2 = rtmp_pool.tile([128, TW], F32, name="rope_t2")
                nc.vector.stream_shuffle(sw[:], ps, SWAP16)
                nc.vector.tensor_tensor(t1[:], sw[:], Sx, AluOpType.mult)
                nc.vector.tensor_tensor(t2[:], ps, Cx, AluOpType.mult)
                nc.vector.tensor_tensor(dst, t1[:], t2[:], AluOpType.add)

            for tb in range(cfg.NT):
                t0 = (tb % cfg.NB) * TW  # position within batch
                Cx = ropeC[:, t0:t0 + TW]
                Sx = ropeS[:, t0:t0 + TW]

                q_ps = q_psum.tile([128, NHL * TW], F32, name="q_ps")
                k_ps = k_psum.tile([128, TW], F32, name="k_ps")
                vT_ps = v_psum.tile([128, TW], F32, name="vT_ps")
                # keep the whole tokblock's xT resident so k/vT and q run as
                # two dense passes; next tokblock's k/vT pass overlaps this
                # block's RoPE drains instead of stalling on them
                xts = []
                XB = 4  # dc-chunks per DMA (~512KB each)
                for dc in range(0, cfg.DC, XB):
                    d1 = min(dc + XB, cfg.DC)
                    xt = x_pool.tile([128, (d1 - dc) * TW], BF16, name="x_t")
                    nc.sync.dma_start(
                        out=xt[:].rearrange("p (dc t) -> p dc t", dc=d1 - dc),
                        in_=xT_d.ap()[dc * 128:d1 * 128,
                                      tb * TW:(tb + 1) * TW]
                        .rearrange("(dc p) t -> p dc t", p=128))
                    for j in range(d1 - dc):
                        xts.append(xt[:, j * TW:(j + 1) * TW])
                for dc in range(cfg.DC):
                    st = dict(start=(dc == 0), stop=(dc == cfg.DC - 1))
                    nc.tensor.matmul(k_ps[:],
                                     wk_sb[:, dc * HD:(dc + 1) * HD],
                                     xts[dc], **st)
                    nc.tensor.matmul(vT_ps[:],
                                     wv_sb[:, dc * HD:(dc + 1) * HD],
                                     xts[dc], **st)
                for dc in range(cfg.DC):
                    st = dict(start=(dc == 0), stop=(dc == cfg.DC - 1))
                    for h in range(NHL):
                        nc.tensor.matmul(
                            q_ps[:, h * TW:h * TW + TW],
                            wq_sb[:, dc * QD + h * HD: dc * QD + (h + 1) * HD],
                            xts[dc], **st)

                # k: rope -> resident (drain first: next tb needs this bank)
                rope_drain(k_ps[:], kT_sb[:, tb * TW:(tb + 1) * TW])
                # q: rope -> spill to DRAM (one batched DMA per tokblock)
                qr = qrot_pool.tile([128, NHL * TW], BF16, name="q_rot")
                for h in range(NHL):
                    rope_drain(q_ps[:, h * TW:h * TW + TW],
                               qr[:, h * TW:(h + 1) * TW])
                nc.sync.dma_start(
                    out=qT_dram[0:QD, tb * TW:(tb + 1) * TW]
                    .rearrange("(h p) t -> p h t", p=128),
                    in_=qr[:].rearrange("p (h t) -> p h t", h=NHL))
                # v: vT -> transpose -> resident [ktok, hd] blocks
                vt_sb = vst_pool.tile([128, TW], BF16, name="vT_stage")
                nc.scalar.copy(vt_sb[:], vT_ps[:])
                for i in range(TW // 128):
                    vp = vt_psum.tile([128, 128], BF16, name="v_tr_ps")
                    nc.tensor.transpose(vp[:], vt_sb[:, i * 128:(i + 1) * 128],
                                        ident[:])
                    nc.scalar.copy(
                        v_sb[:, tb * TW + i * 128: tb * TW + (i + 1) * 128],
                        vp[:])

        # ================= phase 2: attention + wo + ReduceScatter =========
        with tc.tile_pool(name="mask", bufs=5) as m_pool, \
             tc.tile_pool(name="qload", bufs=3) as q_pool, \
             tc.tile_pool(name="expsb", bufs=4) as e_pool, \
             tc.tile_pool(name="msum", bufs=3) as msk_pool, \
             tc.tile_pool(name="attnsb", bufs=2) as at_pool, \
             tc.tile_pool(name="recsb", bufs=2) as rec_pool, \
             tc.tile_pool(name="outcp", bufs=4) as oc_pool, \
             tc.tile_pool(name="fin", bufs=2) as fin_pool, \
             tc.tile_pool(name="scps", bufs=3, space="PSUM") as sc_psum, \
             tc.tile_pool(name="avps", bufs=2, space="PSUM") as av_psum, \
             tc.tile_pool(name="seps", bufs=1, space="PSUM") as se_psum, \
             tc.tile_pool(name="ops", bufs=2, space="PSUM") as o_psum:

            for qb in range(cfg.NB):
                active = [kt for kt in range(cfg.KT)
                          if cls[kt][qb][0] != 'N']
                # off[kt]: leading fully-masked query columns -> skip them.
                # Force 0 on the first active tile so PSUM start=True
                # initializes every column.
                offs = {kt: cls[kt][qb][1] for kt in active}
                offs[active[0]] = 0
                # batch mask loads: one DMA per contiguous run of M tiles
                mkts = [kt for kt in active if cls[kt][qb][0] == 'M']
                runs = []
                for kt in mkts:
                    if runs and kt == runs[-1][-1] + 1 and len(runs[-1]) < 4:
                        runs[-1].append(kt)
                    else:
                        runs.append([kt])
                mtiles = {}
                for run in runs:
                    nk = len(run)
                    mrun = m_pool.tile([KW, nk * TW], BF16, name="m_t")
                    nc.sync.dma_start(
                        out=mrun[:].rearrange("p (k t) -> p k t", k=nk),
                        in_=maskT_d.ap()[run[0] * KW:(run[-1] + 1) * KW,
                                         qb * TW:(qb + 1) * TW]
                        .rearrange("(k p) t -> p k t", p=KW))
                    for i, kt in enumerate(run):
                        mtiles[kt] = mrun[:, i * TW:(i + 1) * TW]

                for b in range(cfg.B):
                    attn_sb = at_pool.tile([128, NHL * TW], BF16, name="at_sb")
                    qt_all = q_pool.tile([128, NHL * TW], BF16, name="q_t")
                    tb2 = b * cfg.NB + qb
                    hh = NHL // 2
                    for hp in range(2):
                        nc.sync.dma_start(
                            out=qt_all[:, hp * hh * TW:(hp + 1) * hh * TW]
                            .rearrange("p (h t) -> p h t", h=hh),
                            in_=qT_dram[hp * hh * HD:(hp + 1) * hh * HD,
                                        tb2 * TW:(tb2 + 1) * TW]
                            .rearrange("(h p) t -> p h t", p=128))
                    for h in range(NHL):
                        qt = qt_all[:, h * TW:(h + 1) * TW]
                        at_ps = av_psum.tile([HD, TW], F32, name="at_ps")
                        se_ps = se_psum.tile([128, TW], F32, name="se_ps")
                        # software pipeline: issue score matmuls LOOKAHEAD
                        # iterations ahead so the PE never waits on exp (ACT)
                        LOOKAHEAD = 2
                        n_act = len(active)
                        sc_tiles = [None] * n_act

                        def emit_sc(j):
                            kt2 = active[j]
                            gk2 = b * cfg.L + kt2 * KW
                            o = offs[kt2]
                            sc = sc_psum.tile([KW, TW], F32, name="sc_ps")
                            nc.tensor.matmul(sc[:, o:], kT_sb[:, gk2:gk2 + KW],
                                             qt[:, o:], start=True, stop=True)
                            sc_tiles[j] = sc

                        for j in range(min(LOOKAHEAD, n_act)):
                            emit_sc(j)
                        for idx, kt in enumerate(active):
                            if idx + LOOKAHEAD < n_act:
                                emit_sc(idx + LOOKAHEAD)
                            gk = b * cfg.L + kt * KW  # global key token
                            o = offs[kt]
                            sc_ps = sc_tiles[idx]
                            sc_tiles[idx] = None
                            if cls[kt][qb][0] == 'M':
                                ms = msk_pool.tile([KW, TW], F32, name="ms_t")
                                nc.vector.tensor_tensor(
                                    ms[:, o:], sc_ps[:, o:], mtiles[kt][:, o:],
                                    AluOpType.add)
                                esrc = ms[:, o:]
                            else:
                                esrc = sc_ps[:, o:]
                            ex = e_pool.tile([KW, TW], BF16, name="ex_t")
                            nc.scalar.activation(
                                ex[:, o:], esrc,
                                mybir.ActivationFunctionType.Exp,
                                scale=float(SCALE))
                            st = dict(start=(idx == 0),
                                      stop=(idx == len(active) - 1))
                            nc.tensor.matmul(se_ps[:, o:], ones_sb[:],
                                             ex[:, o:], **st)
                            nc.tensor.matmul(at_ps[:, o:], v_sb[:, gk:gk + KW],
                                             ex[:, o:], **st)
                        rec = rec_pool.tile([128, TW], F32, name="rec_t")
                        nc.vector.reciprocal_approx_fast(rec[:], se_ps[:])
                        nc.vector.tensor_tensor(
                            attn_sb[:, h * TW:(h + 1) * TW],
                            at_ps[:], rec[:], AluOpType.mult)

                    # ---- wo partial for this (b, qb) token block ----------
                    c = b * cfg.NB + qb
                    NG = min(4, cfg.NBLK)  # n-blocks per batched store
                    for m in range(TW // 128):
                        for ng in range(cfg.NBLK // NG):
                            oc = oc_pool.tile([128, NG * TW], BF16,
                                              name="oc_t")
                            for j in range(NG):
                                n = ng * NG + j
                                o_ps = o_psum.tile([128, TW], F32,
                                                   name="o_ps")
                                for h in range(NHL):
                                    nc.tensor.matmul(
                                        o_ps[:],
                                        attn_sb[:, h * TW + m * 128:
                                                h * TW + (m + 1) * 128],
                                        wo_sb[:, h * cfg.D + n * TW:
                                              h * cfg.D + (n + 1) * TW],
                                        start=(h == 0), stop=(h == NHL - 1))
                                # alternate drain engine: keep ACT free
                                dst = oc[:, j * TW:(j + 1) * TW]
                                if n % 2 == 0:
                                    nc.vector.tensor_copy(dst, o_ps[:])
                                else:
                                    nc.scalar.copy(dst, o_ps[:])
                            # SWDGE queue: feeds the collective, keeps the
                            # sync sequencer free for PE-critical loads
                            nc.gpsimd.dma_start(
                                out=rs_in[c][m * 128:(m + 1) * 128,
                                             ng * NG * TW:(ng + 1) * NG * TW],
                                in_=oc[:])
                    nc.gpsimd.collective_compute(
                        "ReduceScatter", AluOpType.add, replica_groups=rg,
                        ins=[rs_in[c][:].opt()], outs=[rs_out[c][:].opt()])

            # ---- program tail: bf16 RS shards -> f32 external output ------
            # (emitted last so the RS-dependent loads/casts never head-of-line
            # block the ACT / sync queues that attention needs)
            rw = TW // N_CORES
            g2 = 128 // rw
            fw = cfg.D // g2
            for c in range(cfg.NCH):
                ocb = fin_pool.tile([128, fw], BF16, name="fin_b")
                ocf = fin_pool.tile([128, fw], F32, name="fin_f")
                nc.sync.dma_start(
                    out=ocb[:],
                    in_=rs_out[c][:].rearrange("r (g f) -> (r g) f", g=g2))
                nc.scalar.copy(ocf[:], ocb[:])
                nc.sync.dma_start(
                    out=out_d.ap()[c * rw:(c + 1) * rw, :]
                    .rearrange("r (g f) -> (r g) f", g=g2),
                    in_=ocf[:])

        w_pool.release()
        dram_pool.release()
        kv_pool.release()
        const_pool.release()

    nc.compile()
    return nc


def host_prepare(cfg, x, mask, wq, wk, wv, wo):
    """Returns (in_maps, cls)."""
    x = np.ascontiguousarray(np.asarray(x, dtype=np.float32))
    mask = np.asarray(mask, dtype=np.float32)
    wq = np.asarray(wq, dtype=np.float32)
    wk = np.asarray(wk, dtype=np.float32)
    wv = np.asarray(wv, dtype=np.float32)
    wo = np.asarray(wo, dtype=np.float32)

    import ml_dtypes
    bf16 = ml_dtypes.bfloat16
    perm = _rope_perm()
    C, S = _rope_tables(cfg)
    xT = np.ascontiguousarray(x.reshape(cfg.BL, cfg.D).T).astype(bf16)
    maskT = np.ascontiguousarray(mask.T / SCALE).astype(bf16)
    cls = classify_mask(mask, cfg)

    in_maps = []
    for g in range(N_CORES):
        qrows = wq[g * NHL * HD:(g + 1) * NHL * HD]          # [512, D]
        qperm = np.concatenate(
            [qrows[h * HD + perm] for h in range(NHL)], axis=0)
        krows = wk[g * HD:(g + 1) * HD][perm]                # [128, D]
        vrows = wv[g * HD:(g + 1) * HD]                      # [128, D]
        wocols = wo[:, g * NHL * HD:(g + 1) * NHL * HD]      # [D, 512]
        in_maps.append({
            "xT": xT,
            "wqT": np.ascontiguousarray(qperm.T).astype(bf16),
            "wkT": np.ascontiguousarray(krows.T).astype(bf16),
            "wvT": np.ascontiguousarray(vrows.T).astype(bf16),
            "woT": np.ascontiguousarray(wocols.T).astype(bf16),
            "maskT": maskT,
            "ropeC": C,
            "ropeS": S,
        })
    return in_maps, cls


def assemble_output(cfg, results):
    """Stitch per-core ReduceScatter shards back into [B, L, D]."""
    full = np.empty((cfg.BL, cfg.D), dtype=np.float32)
    rw = TW // N_CORES
    for g in range(N_CORES):
        r = results[g]["out"]
        for c in range(cfg.NCH):
            full[c * TW + g * rw: c * TW + (g + 1) * rw] = \
                r[c * rw:(c + 1) * rw]
    return full.reshape(cfg.B, cfg.L, cfg.D)


def kernel(x, mask, wq, wk, wv, wo):
    global LAST_RESULTS
    from concourse.bass_utils import run_bass_kernel_spmd
    cfg = Cfg(B=2, L=2048, D=4096)
    in_maps, cls = host_prepare(cfg, x, mask, wq, wk, wv, wo)
    nc = build_bass(cfg, cls)
    res = run_bass_kernel_spmd(nc, in_maps, core_ids=list(range(N_CORES)),
                               trace=TRACE)
    LAST_RESULTS = res
    return assemble_output(cfg, res.results)


# revision 45
# speedup vs baseline: 3.5247x; 1.0119x over previous
"""Distributed GQA attention kernel for one TRN2 chip (8 NeuronCores).

Sharding: tensor-parallel over heads. Core g owns query heads [4g, 4g+4)
and kv head g. Each core computes its heads' attention and a partial
output projection; a chunked ReduceScatter sums the partials and leaves
each core with a 1/8 token-slice of the final output.

All device tensors are laid out so that no on-device transposes of the
big activations are needed:
  - x is passed pre-transposed (xT [D, B*L]) so projections contract D
    on the partition axis.
  - q/k are produced directly as qT/kT [head_dim, tokens]; scores are
    computed keys-on-partitions, so the P@V matmul consumes exp(scores)
    directly and the wo matmul consumes the attention output directly.
  - RoPE head_dim pairs are permuted (on the host, into wq/wk rows) so
    each rotation partner lives 16 partitions away within a 32-partition
    quadrant -> one DVE stream_shuffle does the swap.
  - softmax denominator comes from an all-ones matmul (partition
    broadcast for free); no max subtraction (fp32 logits here are <~15).
"""

import numpy as np

import concourse.bass as bass
import concourse.mybir as mybir
import concourse.tile as tile
from concourse import bacc
from concourse.alu_op_type import AluOpType
from concourse.masks import make_identity

F32 = mybir.dt.float32
BF16 = mybir.dt.bfloat16

N_CORES = 8
NHL = 4           # local q heads per core
HD = 128          # head dim
THETA = 10000.0
SCALE = HD ** -0.5
TW = 512          # token block width (free dim of most matmuls)
KW = 128          # key tile width (partition dim of score tiles)

# module-level knobs for test.py
TRACE = False
LAST_RESULTS = None


class Cfg:
    def __init__(self, B=2, L=2048, D=4096):
        self.B, self.L, self.D = B, L, D
        self.BL = B * L
        self.DC = D // 128         # contraction chunks for projections
        self.NB = L // TW          # query blocks per batch
        self.NT = self.BL // TW    # token blocks total
        self.KT = L // KW          # key tiles per batch
        self.NBLK = D // TW        # wo output column blocks
        self.NCH = self.NT         # ReduceScatter chunks (one per token block)
        assert self.BL % TW == 0 and TW % N_CORES == 0


# stream_shuffle mask: swap 16-partition halves within each 32-partition quadrant
SWAP16 = [(i + 16) % 32 for i in range(32)]


def _rope_perm():
    """Permutation of head_dim rows: pair i=(16q + r) lives at partitions
    32q+r (x1 = even dim 2i) and 32q+16+r (x2 = odd dim 2i+1)."""
    perm = np.zeros(HD, dtype=np.int64)
    for p in range(HD):
        q, r = divmod(p, 32)
        i = 16 * q + (r % 16)
        perm[p] = 2 * i + (0 if r < 16 else 1)
    return perm


def _rope_tables(cfg):
    """cosT/sinT [128, L] in the permuted-partition layout, sin sign-folded."""
    perm = _rope_perm()
    t = np.arange(cfg.L, dtype=np.float64)
    freqs = THETA ** (-np.arange(0, HD, 2, dtype=np.float64) / HD)  # [64]
    theta = t[None, :] * freqs[:, None]                             # [64, L]
    cos, sin = np.cos(theta), np.sin(theta)
    C = np.zeros((HD, cfg.L), dtype=np.float32)
    S = np.zeros((HD, cfg.L), dtype=np.float32)
    for p in range(HD):
        q, r = divmod(p, 32)
        i = 16 * q + (r % 16)
        C[p] = cos[i]
        S[p] = sin[i] if r >= 16 else -sin[i]
    return C, S


def classify_mask(mask, cfg):
    """cls[kt][qb] = (kind, off): kind in {'Z','N','M'} for tile
    mask[qb*TW:(qb+1)*TW, kt*KW:(kt+1)*KW]; off = count of leading query
    columns in the tile that are fully masked (safe to skip: exp would
    be exactly 0 there)."""
    cls = [[None] * cfg.NB for _ in range(cfg.KT)]
    for kt in range(cfg.KT):
        for qb in range(cfg.NB):
            t = mask[qb * TW:(qb + 1) * TW, kt * KW:(kt + 1) * KW]
            if np.all(t == 0.0):
                cls[kt][qb] = ('Z', 0)
            elif np.all(t <= -1e8):
                cls[kt][qb] = ('N', 0)
            else:
                dead_q = np.all(t <= -1e8, axis=1)  # [TW]
                off = 0
                while off < len(dead_q) and dead_q[off]:
                    off += 1
                off = (off // 64) * 64  # keep offsets 64-aligned
                cls[kt][qb] = ('M', off)
    # guard: every query block must attend to at least one key tile
    for qb in range(cfg.NB):
        assert any(cls[kt][qb][0] != 'N' for kt in range(cfg.KT)), \
            "fully-masked query block unsupported"
    return cls


def build_bass(cfg, cls):
    nc = bacc.Bacc("TRN2", target_bir_lowering=False, debug=False,
                   num_devices=N_CORES)

    xT_d = nc.dram_tensor("xT", [cfg.D, cfg.BL], BF16, kind="ExternalInput")
    wqT_d = nc.dram_tensor("wqT", [cfg.D, NHL * HD], BF16, kind="ExternalInput")
    wkT_d = nc.dram_tensor("wkT", [cfg.D, HD], BF16, kind="ExternalInput")
    wvT_d = nc.dram_tensor("wvT", [cfg.D, HD], BF16, kind="ExternalInput")
    woT_d = nc.dram_tensor("woT", [NHL * HD, cfg.D], BF16, kind="ExternalInput")
    maskT_d = nc.dram_tensor("maskT", [cfg.L, cfg.L], BF16, kind="ExternalInput")
    ropeC_d = nc.dram_tensor("ropeC", [HD, cfg.L], F32, kind="ExternalInput")
    ropeS_d = nc.dram_tensor("ropeS", [HD, cfg.L], F32, kind="ExternalInput")
    out_d = nc.dram_tensor("out", [cfg.BL // N_CORES, cfg.D], F32,
                           kind="ExternalOutput")

    rg = [list(range(N_CORES))]
    QD = NHL * HD  # 512

    with tile.TileContext(nc) as tc:
        # ---- constants / tables -------------------------------------------
        const_pool = tc.alloc_tile_pool(name="const", bufs=1)
        ones_sb = const_pool.tile([128, 128], BF16, name="ones_sb")
        nc.vector.memset(ones_sb[:], 1.0)
        ident = const_pool.tile([128, 128], BF16, name="ident")
        make_identity(nc, ident[:])

        # ---- resident activations -----------------------------------------
        kv_pool = tc.alloc_tile_pool(name="kv", bufs=1)
        kT_sb = kv_pool.tile([HD, cfg.BL], BF16, name="kT_sb")
        v_sb = kv_pool.tile([128, cfg.BL], BF16, name="v_sb")

        # DRAM scratch
        dram_pool = tc.alloc_tile_pool(name="dram", bufs=1, space="DRAM")
        qT_dram = dram_pool.tile([QD, cfg.BL], BF16, name="qT_dram")
        rs_in = [dram_pool.tile([TW, cfg.D], BF16, name=f"rs_in{c}")
                 for c in range(cfg.NCH)]
        rs_out = [dram_pool.tile([TW // N_CORES, cfg.D], BF16,
                                 name=f"rs_out{c}")
                  for c in range(cfg.NCH)]

        # ---- all weights resident upfront (bf16 halves the footprint) -----
        # Few, ~0.5-1MB DMAs (sequencer dispatch is ~0.6us each); k/v first
        # so the first tokblock's matmuls can start ASAP; wo on the scalar
        # queue so it never delays the x stream.
        w_pool = tc.alloc_tile_pool(name="weights", bufs=1)
        wq_sb = w_pool.tile([128, cfg.DC * QD], BF16, name="wq_sb")
        wk_sb = w_pool.tile([128, cfg.DC * HD], BF16, name="wk_sb")
        wv_sb = w_pool.tile([128, cfg.DC * HD], BF16, name="wv_sb")
        wo_sb = w_pool.tile([128, NHL * cfg.D], BF16, name="wo_sb")

        def load_w3d(eng, dst, src_d, width, chunk):
            """dst[:, dc*width+c] = src[dc*128+p, c], batched `chunk` dcs/DMA."""
            for d0 in range(0, cfg.DC, chunk):
                d1 = min(d0 + chunk, cfg.DC)
                eng.dma_start(
                    out=dst[:, d0 * width:d1 * width]
                    .rearrange("p (dc c) -> p dc c", dc=d1 - d0),
                    in_=src_d.ap()[d0 * 128:d1 * 128, :]
                    .rearrange("(dc p) c -> p dc c", p=128))

        load_w3d(nc.sync, wk_sb, wkT_d, HD, 8)     # 4 DMAs of 512KB
        load_w3d(nc.sync, wv_sb, wvT_d, HD, 8)
        load_w3d(nc.sync, wq_sb, wqT_d, QD, 4)     # 8 DMAs of 1MB
        for h in range(NHL):                       # 4 DMAs of 1MB (ACT queue)
            nc.scalar.dma_start(out=wo_sb[:, h * cfg.D:(h + 1) * cfg.D],
                                in_=woT_d.ap()[h * HD:(h + 1) * HD, :])

        # ================= phase 1: QKV projections + RoPE =================
        with tc.tile_pool(name="xload", bufs=cfg.DC // 4 + 2) as x_pool, \
             tc.tile_pool(name="qpsum", bufs=1, space="PSUM") as q_psum, \
             tc.tile_pool(name="kpsum", bufs=2, space="PSUM") as k_psum, \
             tc.tile_pool(name="vpsum", bufs=1, space="PSUM") as v_psum, \
             tc.tile_pool(name="ropetmp", bufs=3) as rtmp_pool, \
             tc.tile_pool(name="qrot", bufs=3) as qrot_pool, \
             tc.tile_pool(name="vstage", bufs=2) as vst_pool, \
             tc.tile_pool(name="vtpsum", bufs=1, space="PSUM") as vt_psum, \
             tc.tile_pool(name="ropetbl", bufs=1) as rtbl_pool:

            ropeC = rtbl_pool.tile([HD, cfg.L], F32, name="ropeC_sb")
            ropeS = rtbl_pool.tile([HD, cfg.L], F32, name="ropeS_sb")
            nc.scalar.dma_start(out=ropeC[:], in_=ropeC_d.ap())
            nc.scalar.dma_start(out=ropeS[:], in_=ropeS_d.ap())

            def rope_drain(ps, dst):
                """dst = ps*C + shuffle16(ps)*S at token offset t0 (len TW)."""
                sw = rtmp_pool.tile([128, TW], F32, name="rope_sw")
                t1 = rtmp_pool.tile([128, TW], F32, name="rope_t1")
                t2 = rtmp_pool.tile([128, TW], F32, name="rope_t2")
                nc.vector.stream_shuffle(sw[:], ps, SWAP16)
                nc.vector.tensor_tensor(t1[:], sw[:], Sx, AluOpType.mult)
                nc.vector.tensor_tensor(t2[:], ps, Cx, AluOpType.mult)
                nc.vector.tensor_tensor(dst, t1[:], t2[:], AluOpType.add)

            for tb in range(cfg.NT):
                t0 = (tb % cfg.NB) * TW  # position within batch
                Cx = ropeC[:, t0:t0 + TW]
                Sx = ropeS[:, t0:t0 + TW]

                q_ps = q_psum.tile([128, NHL * TW], F32, name="q_ps")
                k_ps = k_psum.tile([128, TW], F32, name="k_ps")
                vT_ps = v_psum.tile([128, TW], F32, name="vT_ps")
                # keep the whole tokblock's xT resident so k/vT and q run as
                # two dense passes; next tokblock's k/vT pass overlaps this
                # block's RoPE drains instead of stalling on them
                xts = []
                XB = 4  # dc-chunks per DMA (~512KB each)
                for dc in range(0, cfg.DC, XB):
                    d1 = min(dc + XB, cfg.DC)
                    xt = x_pool.tile([128, (d1 - dc) * TW], BF16, name="x_t")
                    nc.sync.dma_start(
                        out=xt[:].rearrange("p (dc t) -> p dc t", dc=d1 - dc),
                        in_=xT_d.ap()[dc * 128:d1 * 128,
                                      tb * TW:(tb + 1) * TW]
                        .rearrange("(dc p) t -> p dc t", p=128))
                    for j in range(d1 - dc):
                        xts.append(xt[:, j * TW:(j + 1) * TW])
                for dc in range(cfg.DC):
                    st = dict(start=(dc == 0), stop=(dc == cfg.DC - 1))
                    nc.tensor.matmul(k_ps[:],
                                     wk_sb[:, dc * HD:(dc + 1) * HD],
                                     xts[dc], **st)
                    nc.tensor.matmul(vT_ps[:],
                                     wv_sb[:, dc * HD:(dc + 1) * HD],
                                     xts[dc], **st)
                for dc in range(cfg.DC):
                    st = dict(start=(dc == 0), stop=(dc == cfg.DC - 1))
                    for h in range(NHL):
                        nc.tensor.matmul(
                            q_ps[:, h * TW:h * TW + TW],
                            wq_sb[:, dc * QD + h * HD: dc * QD + (h + 1) * HD],
                            xts[dc], **st)

                # k: rope -> resident (drain first: next tb needs this bank)
                rope_drain(k_ps[:], kT_sb[:, tb * TW:(tb + 1) * TW])
                # q: rope -> spill to DRAM (one batched DMA per tokblock)
                qr = qrot_pool.tile([128, NHL * TW], BF16, name="q_rot")
                for h in range(NHL):
                    rope_drain(q_ps[:, h * TW:h * TW + TW],
                               qr[:, h * TW:(h + 1) * TW])
                nc.sync.dma_start(
                    out=qT_dram[0:QD, tb * TW:(tb + 1) * TW]
                    .rearrange("(h p) t -> p h t", p=128),
                    in_=qr[:].rearrange("p (h t) -> p h t", h=NHL))
                # v: vT -> transpose -> resident [ktok, hd] blocks
                vt_sb = vst_pool.tile([128, TW], BF16, name="vT_stage")
                nc.scalar.copy(vt_sb[:], vT_ps[:])
                for i in range(TW // 128):
                    vp = vt_psum.tile([128, 128], BF16, name="v_tr_ps")
                    nc.tensor.transpose(vp[:], vt_sb[:, i * 128:(i + 1) * 128],
                                        ident[:])
                    nc.scalar.copy(
                        v_sb[:, tb * TW + i * 128: tb * TW + (i + 1) * 128],
                        vp[:])

        # ================= phase 2: attention + wo + ReduceScatter =========
        with tc.tile_pool(name="mask", bufs=5) as m_pool, \
             tc.tile_pool(name="qload", bufs=3) as q_pool, \
             tc.tile_pool(name="expsb", bufs=4) as e_pool, \
             tc.tile_pool(name="msum", bufs=3) as msk_pool, \
             tc.tile_pool(name="attnsb", bufs=2) as at_pool, \
             tc.tile_pool(name="recsb", bufs=2) as rec_pool, \
             tc.tile_pool(name="outcp", bufs=4) as oc_pool, \
             tc.tile_pool(name="fin", bufs=2) as fin_pool, \
             tc.tile_pool(name="scps", bufs=3, space="PSUM") as sc_psum, \
             tc.tile_pool(name="avps", bufs=2, space="PSUM") as av_psum, \
             tc.tile_pool(name="seps", bufs=1, space="PSUM") as se_psum, \
             tc.tile_pool(name="ops", bufs=2, space="PSUM") as o_psum:

            for qb in range(cfg.NB):
                active = [kt for kt in range(cfg.KT)
                          if cls[kt][qb][0] != 'N']
                # off[kt]: leading fully-masked query columns -> skip them.
                # Force 0 on the first active tile so PSUM start=True
                # initializes every column.
                offs = {kt: cls[kt][qb][1] for kt in active}
                offs[active[0]] = 0
                # batch mask loads: one DMA per contiguous run of M tiles
                mkts = [kt for kt in active if cls[kt][qb][0] == 'M']
                runs = []
                for kt in mkts:
                    if runs and kt == runs[-1][-1] + 1 and len(runs[-1]) < 4:
                        runs[-1].append(kt)
                    else:
                        runs.append([kt])
                mtiles = {}
                for run in runs:
                    nk = len(run)
                    mrun = m_pool.tile([KW, nk * TW], BF16, name="m_t")
                    nc.sync.dma_start(
                        out=mrun[:].rearrange("p (k t) -> p k t", k=nk),
                        in_=maskT_d.ap()[run[0] * KW:(run[-1] + 1) * KW,
                                         qb * TW:(qb + 1) * TW]
                        .rearrange("(k p) t -> p k t", p=KW))
                    for i, kt in enumerate(run):
                        mtiles[kt] = mrun[:, i * TW:(i + 1) * TW]

                for b in range(cfg.B):
                    attn_sb = at_pool.tile([128, NHL * TW], BF16, name="at_sb")
                    qt_all = q_pool.tile([128, NHL * TW], BF16, name="q_t")
                    tb2 = b * cfg.NB + qb
                    hh = NHL // 2
                    for hp in range(2):
                        nc.sync.dma_start(
                            out=qt_all[:, hp * hh * TW:(hp + 1) * hh * TW]
                            .rearrange("p (h t) -> p h t", h=hh),
                            in_=qT_dram[hp * hh * HD:(hp + 1) * hh * HD,
                                        tb2 * TW:(tb2 + 1) * TW]
                            .rearrange("(h p) t -> p h t", p=128))
                    for h in range(NHL):
                        qt = qt_all[:, h * TW:(h + 1) * TW]
                        at_ps = av_psum.tile([HD, TW], F32, name="at_ps")
                        se_ps = se_psum.tile([128, TW], F32, name="se_ps")
                        # software pipeline: issue score matmuls LOOKAHEAD
                        # iterations ahead so the PE never waits on exp (ACT)
                        LOOKAHEAD = 2
                        n_act = len(active)
                        sc_tiles = [None] * n_act

                        def emit_sc(j):
                            kt2 = active[j]
                            gk2 = b * cfg.L + kt2 * KW
                            o = offs[kt2]
                            sc = sc_psum.tile([KW, TW], F32, name="sc_ps")
                            nc.tensor.matmul(sc[:, o:], kT_sb[:, gk2:gk2 + KW],
                                             qt[:, o:], start=True, stop=True)
                            sc_tiles[j] = sc

                        for j in range(min(LOOKAHEAD, n_act)):
                            emit_sc(j)
                        for idx, kt in enumerate(active):
                            if idx + LOOKAHEAD < n_act:
                                emit_sc(idx + LOOKAHEAD)
                            gk = b * cfg.L + kt * KW  # global key token
                            o = offs[kt]
                            sc_ps = sc_tiles[idx]
                            sc_tiles[idx] = None
                            if cls[kt][qb][0] == 'M':
                                ms = msk_pool.tile([KW, TW], F32, name="ms_t")
                                nc.vector.tensor_tensor(
                                    ms[:, o:], sc_ps[:, o:], mtiles[kt][:, o:],
                                    AluOpType.add)
                                esrc = ms[:, o:]
                            else:
                                esrc = sc_ps[:, o:]
                            ex = e_pool.tile([KW, TW], BF16, name="ex_t")
                            nc.scalar.activation(
                                ex[:, o:], esrc,
                                mybir.ActivationFunctionType.Exp,
                                scale=float(SCALE))
                            st = dict(start=(idx == 0),
                                      stop=(idx == len(active) - 1))
                            nc.tensor.matmul(se_ps[:, o:], ones_sb[:],
                                             ex[:, o:], **st)
                            nc.tensor.matmul(at_ps[:, o:], v_sb[:, gk:gk + KW],
                                             ex[:, o:], **st)
                        rec = rec_pool.tile([128, TW], F32, name="rec_t")
                        nc.vector.reciprocal_approx_fast(rec[:], se_ps[:])
                        nc.vector.tensor_tensor(
                            attn_sb[:, h * TW:(h + 1) * TW],
                            at_ps[:], rec[:], AluOpType.mult)

                    # ---- wo partial for this (b, qb) token block ----------
                    c = b * cfg.NB + qb
                    NG = min(4, cfg.NBLK)  # n-blocks per batched store
                    for m in range(TW // 128):
                        for ng in range(cfg.NBLK // NG):
                            oc = oc_pool.tile([128, NG * TW], BF16,
                                              name="oc_t")
                            for j in range(NG):
                                n = ng * NG + j
                                o_ps = o_psum.tile([128, TW], F32,
                                                   name="o_ps")
                                for h in range(NHL):
                                    nc.tensor.matmul(
                                        o_ps[:],
                                        attn_sb[:, h * TW + m * 128:
                                                h * TW + (m + 1) * 128],
                                        wo_sb[:, h * cfg.D + n * TW:
                                              h * cfg.D + (n + 1) * TW],
                                        start=(h == 0), stop=(h == NHL - 1))
                                # alternate drain engine: keep ACT free
                                dst = oc[:, j * TW:(j + 1) * TW]
                                if n % 2 == 0:
                                    nc.vector.tensor_copy(dst, o_ps[:])
                                else:
                                    nc.scalar.copy(dst, o_ps[:])
                            # SWDGE queue: feeds the collective, keeps the
                            # sync sequencer free for PE-critical loads
                            nc.gpsimd.dma_start(
                                out=rs_in[c][m * 128:(m + 1) * 128,
                                             ng * NG * TW:(ng + 1) * NG * TW],
                                in_=oc[:])
                    nc.gpsimd.collective_compute(
                        "ReduceScatter", AluOpType.add, replica_groups=rg,
                        ins=[rs_in[c][:].opt()], outs=[rs_out[c][:].opt()])

            # ---- program tail: bf16 RS shards -> f32 external output ------
            # (emitted last so the RS-dependent loads/casts never head-of-line
            # block the ACT / sync queues that attention needs)
            rw = TW // N_CORES
            g2 = 128 // rw
            fw = cfg.D // g2
            for c in range(cfg.NCH):
                ocb = fin_pool.tile([128, fw], BF16, name="fin_b")
                ocf = fin_pool.tile([128, fw], F32, name="fin_f")
                nc.sync.dma_start(
                    out=ocb[:],
                    in_=rs_out[c][:].rearrange("r (g f) -> (r g) f", g=g2))
                nc.scalar.copy(ocf[:], ocb[:])
                nc.sync.dma_start(
                    out=out_d.ap()[c * rw:(c + 1) * rw, :]
                    .rearrange("r (g f) -> (r g) f", g=g2),
                    in_=ocf[:])

        w_pool.release()
        dram_pool.release()
        kv_pool.release()
        const_pool.release()

    nc.compile()
    return nc


def host_prepare(cfg, x, mask, wq, wk, wv, wo):
    """Returns (in_maps, cls)."""
    x = np.ascontiguousarray(np.asarray(x, dtype=np.float32))
    mask = np.asarray(mask, dtype=np.float32)
    wq = np.asarray(wq, dtype=np.float32)
    wk = np.asarray(wk, dtype=np.float32)
    wv = np.asarray(wv, dtype=np.float32)
    wo = np.asarray(wo, dtype=np.float32)

    import ml_dtypes
    bf16 = ml_dtypes.bfloat16
    perm = _rope_perm()
    C, S = _rope_tables(cfg)
    xT = np.ascontiguousarray(x.reshape(cfg.BL, cfg.D).T).astype(bf16)
    maskT = np.ascontiguousarray(mask.T / SCALE).astype(bf16)
    cls = classify_mask(mask, cfg)

    in_maps = []
    for g in range(N_CORES):
        qrows = wq[g * NHL * HD:(g + 1) * NHL * HD]          # [512, D]
        qperm = np.concatenate(
            [qrows[h * HD + perm] for h in range(NHL)], axis=0)
        krows = wk[g * HD:(g + 1) * HD][perm]                # [128, D]
        vrows = wv[g * HD:(g + 1) * HD]                      # [128, D]
        wocols = wo[:, g * NHL * HD:(g + 1) * NHL * HD]      # [D, 512]
        in_maps.append({
            "xT": xT,
            "wqT": np.ascontiguousarray(qperm.T).astype(bf16),
            "wkT": np.ascontiguousarray(krows.T).astype(bf16),
            "wvT": np.ascontiguousarray(vrows.T).astype(bf16),
            "woT": np.ascontiguousarray(wocols.T).astype(bf16),
            "maskT": maskT,
            "ropeC": C,
            "ropeS": S,
        })
    return in_maps, cls


def assemble_output(cfg, results):
    """Stitch per-core ReduceScatter shards back into [B, L, D]."""
    full = np.empty((cfg.BL, cfg.D), dtype=np.float32)
    rw = TW // N_CORES
    for g in range(N_CORES):
        r = results[g]["out"]
        for c in range(cfg.NCH):
            full[c * TW + g * rw: c * TW + (g + 1) * rw] = \
                r[c * rw:(c + 1) * rw]
    return full.reshape(cfg.B, cfg.L, cfg.D)


def kernel(x, mask, wq, wk, wv, wo):
    global LAST_RESULTS
    from concourse.bass_utils import run_bass_kernel_spmd
    cfg = Cfg(B=2, L=2048, D=4096)
    in_maps, cls = host_prepare(cfg, x, mask, wq, wk, wv, wo)
    nc = build_bass(cfg, cls)
    res = run_bass_kernel_spmd(nc, in_maps, core_ids=list(range(N_CORES)),
                               trace=TRACE)
    LAST_RESULTS = res
    return assemble_output(cfg, res.results)


# revision 46
# speedup vs baseline: 3.5759x; 1.0145x over previous
"""Distributed GQA attention kernel for one TRN2 chip (8 NeuronCores).

Sharding: tensor-parallel over heads. Core g owns query heads [4g, 4g+4)
and kv head g. Each core computes its heads' attention and a partial
output projection; a chunked ReduceScatter sums the partials and leaves
each core with a 1/8 token-slice of the final output.

All device tensors are laid out so that no on-device transposes of the
big activations are needed:
  - x is passed pre-transposed (xT [D, B*L]) so projections contract D
    on the partition axis.
  - q/k are produced directly as qT/kT [head_dim, tokens]; scores are
    computed keys-on-partitions, so the P@V matmul consumes exp(scores)
    directly and the wo matmul consumes the attention output directly.
  - RoPE head_dim pairs are permuted (on the host, into wq/wk rows) so
    each rotation partner lives 16 partitions away within a 32-partition
    quadrant -> one DVE stream_shuffle does the swap.
  - softmax denominator comes from an all-ones matmul (partition
    broadcast for free); no max subtraction (fp32 logits here are <~15).
"""

import numpy as np

import concourse.bass as bass
import concourse.mybir as mybir
import concourse.tile as tile
from concourse import bacc
from concourse.alu_op_type import AluOpType
from concourse.masks import make_identity

F32 = mybir.dt.float32
BF16 = mybir.dt.bfloat16

N_CORES = 8
NHL = 4           # local q heads per core
HD = 128          # head dim
THETA = 10000.0
SCALE = HD ** -0.5
TW = 512          # token block width (free dim of most matmuls)
KW = 128          # key tile width (partition dim of score tiles)

# module-level knobs for test.py
TRACE = False
LAST_RESULTS = None


class Cfg:
    def __init__(self, B=2, L=2048, D=4096):
        self.B, self.L, self.D = B, L, D
        self.BL = B * L
        self.DC = D // 128         # contraction chunks for projections
        self.NB = L // TW          # query blocks per batch
        self.NT = self.BL // TW    # token blocks total
        self.KT = L // KW          # key tiles per batch
        self.NBLK = D // TW        # wo output column blocks
        self.NCH = self.NT         # ReduceScatter chunks (one per token block)
        assert self.BL % TW == 0 and TW % N_CORES == 0


# stream_shuffle mask: swap 16-partition halves within each 32-partition quadrant
SWAP16 = [(i + 16) % 32 for i in range(32)]


def _rope_perm():
    """Permutation of head_dim rows: pair i=(16q + r) lives at partitions
    32q+r (x1 = even dim 2i) and 32q+16+r (x2 = odd dim 2i+1)."""
    perm = np.zeros(HD, dtype=np.int64)
    for p in range(HD):
        q, r = divmod(p, 32)
        i = 16 * q + (r % 16)
        perm[p] = 2 * i + (0 if r < 16 else 1)
    return perm


def _rope_tables(cfg):
    """cosT/sinT [128, L] in the permuted-partition layout, sin sign-folded."""
    perm = _rope_perm()
    t = np.arange(cfg.L, dtype=np.float64)
    freqs = THETA ** (-np.arange(0, HD, 2, dtype=np.float64) / HD)  # [64]
    theta = t[None, :] * freqs[:, None]                             # [64, L]
    cos, sin = np.cos(theta), np.sin(theta)
    C = np.zeros((HD, cfg.L), dtype=np.float32)
    S = np.zeros((HD, cfg.L), dtype=np.float32)
    for p in range(HD):
        q, r = divmod(p, 32)
        i = 16 * q + (r % 16)
        C[p] = cos[i]
        S[p] = sin[i] if r >= 16 else -sin[i]
    return C, S


def classify_mask(mask, cfg):
    """cls[kt][qb] = (kind, off): kind in {'Z','N','M'} for tile
    mask[qb*TW:(qb+1)*TW, kt*KW:(kt+1)*KW]; off = count of leading query
    columns in the tile that are fully masked (safe to skip: exp would
    be exactly 0 there)."""
    cls = [[None] * cfg.NB for _ in range(cfg.KT)]
    for kt in range(cfg.KT):
        for qb in range(cfg.NB):
            t = mask[qb * TW:(qb + 1) * TW, kt * KW:(kt + 1) * KW]
            if np.all(t == 0.0):
                cls[kt][qb] = ('Z', 0)
            elif np.all(t <= -1e8):
                cls[kt][qb] = ('N', 0)
            else:
                dead_q = np.all(t <= -1e8, axis=1)  # [TW]
                off = 0
                while off < len(dead_q) and dead_q[off]:
                    off += 1
                off = (off // 64) * 64  # keep offsets 64-aligned
                cls[kt][qb] = ('M', off)
    # guard: every query block must attend to at least one key tile
    for qb in range(cfg.NB):
        assert any(cls[kt][qb][0] != 'N' for kt in range(cfg.KT)), \
            "fully-masked query block unsupported"
    return cls


def build_bass(cfg, cls):
    nc = bacc.Bacc("TRN2", target_bir_lowering=False, debug=False,
                   num_devices=N_CORES, num_swdge_queues=4)

    xT_d = nc.dram_tensor("xT", [cfg.D, cfg.BL], BF16, kind="ExternalInput")
    wqT_d = nc.dram_tensor("wqT", [cfg.D, NHL * HD], BF16, kind="ExternalInput")
    wkT_d = nc.dram_tensor("wkT", [cfg.D, HD], BF16, kind="ExternalInput")
    wvT_d = nc.dram_tensor("wvT", [cfg.D, HD], BF16, kind="ExternalInput")
    woT_d = nc.dram_tensor("woT", [NHL * HD, cfg.D], BF16, kind="ExternalInput")
    maskT_d = nc.dram_tensor("maskT", [cfg.L, cfg.L], BF16, kind="ExternalInput")
    ropeC_d = nc.dram_tensor("ropeC", [HD, cfg.L], F32, kind="ExternalInput")
    ropeS_d = nc.dram_tensor("ropeS", [HD, cfg.L], F32, kind="ExternalInput")
    out_d = nc.dram_tensor("out", [cfg.BL // N_CORES, cfg.D], BF16,
                           kind="ExternalOutput")

    rg = [list(range(N_CORES))]
    QD = NHL * HD  # 512

    with tile.TileContext(nc) as tc:
        # ---- constants / tables -------------------------------------------
        const_pool = tc.alloc_tile_pool(name="const", bufs=1)
        ones_sb = const_pool.tile([128, 128], BF16, name="ones_sb")
        nc.vector.memset(ones_sb[:], 1.0)
        ident = const_pool.tile([128, 128], BF16, name="ident")
        make_identity(nc, ident[:])

        # ---- resident activations -----------------------------------------
        kv_pool = tc.alloc_tile_pool(name="kv", bufs=1)
        kT_sb = kv_pool.tile([HD, cfg.BL], BF16, name="kT_sb")
        v_sb = kv_pool.tile([128, cfg.BL], BF16, name="v_sb")

        # DRAM scratch
        dram_pool = tc.alloc_tile_pool(name="dram", bufs=1, space="DRAM")
        qT_dram = dram_pool.tile([QD, cfg.BL], BF16, name="qT_dram")
        rs_in = [dram_pool.tile([TW, cfg.D], BF16, name=f"rs_in{c}")
                 for c in range(cfg.NCH)]
        rs_out = [dram_pool.tile([TW // N_CORES, cfg.D], BF16,
                                 name=f"rs_out{c}")
                  for c in range(cfg.NCH)]

        # ---- all weights resident upfront (bf16 halves the footprint) -----
        # Few, ~0.5-1MB DMAs (sequencer dispatch is ~0.6us each); k/v first
        # so the first tokblock's matmuls can start ASAP; wo on the scalar
        # queue so it never delays the x stream.
        w_pool = tc.alloc_tile_pool(name="weights", bufs=1)
        wq_sb = w_pool.tile([128, cfg.DC * QD], BF16, name="wq_sb")
        wk_sb = w_pool.tile([128, cfg.DC * HD], BF16, name="wk_sb")
        wv_sb = w_pool.tile([128, cfg.DC * HD], BF16, name="wv_sb")
        wo_sb = w_pool.tile([128, NHL * cfg.D], BF16, name="wo_sb")

        def load_w3d(eng, dst, src_d, width, chunk):
            """dst[:, dc*width+c] = src[dc*128+p, c], batched `chunk` dcs/DMA."""
            for d0 in range(0, cfg.DC, chunk):
                d1 = min(d0 + chunk, cfg.DC)
                eng.dma_start(
                    out=dst[:, d0 * width:d1 * width]
                    .rearrange("p (dc c) -> p dc c", dc=d1 - d0),
                    in_=src_d.ap()[d0 * 128:d1 * 128, :]
                    .rearrange("(dc p) c -> p dc c", p=128))

        load_w3d(nc.sync, wk_sb, wkT_d, HD, 8)     # 4 DMAs of 512KB
        load_w3d(nc.sync, wv_sb, wvT_d, HD, 8)
        load_w3d(nc.sync, wq_sb, wqT_d, QD, 4)     # 8 DMAs of 1MB
        for h in range(NHL):                       # 4 DMAs of 1MB (ACT queue)
            nc.scalar.dma_start(out=wo_sb[:, h * cfg.D:(h + 1) * cfg.D],
                                in_=woT_d.ap()[h * HD:(h + 1) * HD, :])

        # ================= phase 1: QKV projections + RoPE =================
        with tc.tile_pool(name="xload", bufs=cfg.DC // 4 + 2) as x_pool, \
             tc.tile_pool(name="qpsum", bufs=1, space="PSUM") as q_psum, \
             tc.tile_pool(name="kpsum", bufs=2, space="PSUM") as k_psum, \
             tc.tile_pool(name="vpsum", bufs=1, space="PSUM") as v_psum, \
             tc.tile_pool(name="ropetmp", bufs=3) as rtmp_pool, \
             tc.tile_pool(name="qrot", bufs=3) as qrot_pool, \
             tc.tile_pool(name="vstage", bufs=2) as vst_pool, \
             tc.tile_pool(name="vtpsum", bufs=1, space="PSUM") as vt_psum, \
             tc.tile_pool(name="ropetbl", bufs=1) as rtbl_pool:

            ropeC = rtbl_pool.tile([HD, cfg.L], F32, name="ropeC_sb")
            ropeS = rtbl_pool.tile([HD, cfg.L], F32, name="ropeS_sb")
            nc.scalar.dma_start(out=ropeC[:], in_=ropeC_d.ap())
            nc.scalar.dma_start(out=ropeS[:], in_=ropeS_d.ap())

            def rope_drain(ps, dst):
                """dst = ps*C + shuffle16(ps)*S at token offset t0 (len TW)."""
                sw = rtmp_pool.tile([128, TW], F32, name="rope_sw")
                t1 = rtmp_pool.tile([128, TW], F32, name="rope_t1")
                t2 = rtmp_pool.tile([128, TW], F32, name="rope_t2")
                nc.vector.stream_shuffle(sw[:], ps, SWAP16)
                nc.vector.tensor_tensor(t1[:], sw[:], Sx, AluOpType.mult)
                nc.vector.tensor_tensor(t2[:], ps, Cx, AluOpType.mult)
                nc.vector.tensor_tensor(dst, t1[:], t2[:], AluOpType.add)

            for tb in range(cfg.NT):
                t0 = (tb % cfg.NB) * TW  # position within batch
                Cx = ropeC[:, t0:t0 + TW]
                Sx = ropeS[:, t0:t0 + TW]

                q_ps = q_psum.tile([128, NHL * TW], F32, name="q_ps")
                k_ps = k_psum.tile([128, TW], F32, name="k_ps")
                vT_ps = v_psum.tile([128, TW], F32, name="vT_ps")
                # keep the whole tokblock's xT resident so k/vT and q run as
                # two dense passes; next tokblock's k/vT pass overlaps this
                # block's RoPE drains instead of stalling on them
                xts = []
                XB = 4  # dc-chunks per DMA (~512KB each)
                for dc in range(0, cfg.DC, XB):
                    d1 = min(dc + XB, cfg.DC)
                    xt = x_pool.tile([128, (d1 - dc) * TW], BF16, name="x_t")
                    nc.sync.dma_start(
                        out=xt[:].rearrange("p (dc t) -> p dc t", dc=d1 - dc),
                        in_=xT_d.ap()[dc * 128:d1 * 128,
                                      tb * TW:(tb + 1) * TW]
                        .rearrange("(dc p) t -> p dc t", p=128))
                    for j in range(d1 - dc):
                        xts.append(xt[:, j * TW:(j + 1) * TW])
                for dc in range(cfg.DC):
                    st = dict(start=(dc == 0), stop=(dc == cfg.DC - 1))
                    nc.tensor.matmul(k_ps[:],
                                     wk_sb[:, dc * HD:(dc + 1) * HD],
                                     xts[dc], **st)
                    nc.tensor.matmul(vT_ps[:],
                                     wv_sb[:, dc * HD:(dc + 1) * HD],
                                     xts[dc], **st)
                for dc in range(cfg.DC):
                    st = dict(start=(dc == 0), stop=(dc == cfg.DC - 1))
                    for h in range(NHL):
                        nc.tensor.matmul(
                            q_ps[:, h * TW:h * TW + TW],
                            wq_sb[:, dc * QD + h * HD: dc * QD + (h + 1) * HD],
                            xts[dc], **st)

                # k: rope -> resident (drain first: next tb needs this bank)
                rope_drain(k_ps[:], kT_sb[:, tb * TW:(tb + 1) * TW])
                # q: rope -> spill to DRAM (one batched DMA per tokblock)
                qr = qrot_pool.tile([128, NHL * TW], BF16, name="q_rot")
                for h in range(NHL):
                    rope_drain(q_ps[:, h * TW:h * TW + TW],
                               qr[:, h * TW:(h + 1) * TW])
                nc.sync.dma_start(
                    out=qT_dram[0:QD, tb * TW:(tb + 1) * TW]
                    .rearrange("(h p) t -> p h t", p=128),
                    in_=qr[:].rearrange("p (h t) -> p h t", h=NHL))
                # v: vT -> transpose -> resident [ktok, hd] blocks
                vt_sb = vst_pool.tile([128, TW], BF16, name="vT_stage")
                nc.scalar.copy(vt_sb[:], vT_ps[:])
                for i in range(TW // 128):
                    vp = vt_psum.tile([128, 128], BF16, name="v_tr_ps")
                    nc.tensor.transpose(vp[:], vt_sb[:, i * 128:(i + 1) * 128],
                                        ident[:])
                    nc.scalar.copy(
                        v_sb[:, tb * TW + i * 128: tb * TW + (i + 1) * 128],
                        vp[:])

        # ================= phase 2: attention + wo + ReduceScatter =========
        with tc.tile_pool(name="mask", bufs=5) as m_pool, \
             tc.tile_pool(name="qload", bufs=3) as q_pool, \
             tc.tile_pool(name="expsb", bufs=4) as e_pool, \
             tc.tile_pool(name="msum", bufs=3) as msk_pool, \
             tc.tile_pool(name="attnsb", bufs=2) as at_pool, \
             tc.tile_pool(name="recsb", bufs=2) as rec_pool, \
             tc.tile_pool(name="outcp", bufs=4) as oc_pool, \
             tc.tile_pool(name="scps", bufs=3, space="PSUM") as sc_psum, \
             tc.tile_pool(name="avps", bufs=2, space="PSUM") as av_psum, \
             tc.tile_pool(name="seps", bufs=1, space="PSUM") as se_psum, \
             tc.tile_pool(name="ops", bufs=2, space="PSUM") as o_psum:

            for qb in range(cfg.NB):
                active = [kt for kt in range(cfg.KT)
                          if cls[kt][qb][0] != 'N']
                # off[kt]: leading fully-masked query columns -> skip them.
                # Force 0 on the first active tile so PSUM start=True
                # initializes every column.
                offs = {kt: cls[kt][qb][1] for kt in active}
                offs[active[0]] = 0
                # batch mask loads: one DMA per contiguous run of M tiles
                mkts = [kt for kt in active if cls[kt][qb][0] == 'M']
                runs = []
                for kt in mkts:
                    if runs and kt == runs[-1][-1] + 1 and len(runs[-1]) < 4:
                        runs[-1].append(kt)
                    else:
                        runs.append([kt])
                mtiles = {}
                for run in runs:
                    nk = len(run)
                    mrun = m_pool.tile([KW, nk * TW], BF16, name="m_t")
                    nc.sync.dma_start(
                        out=mrun[:].rearrange("p (k t) -> p k t", k=nk),
                        in_=maskT_d.ap()[run[0] * KW:(run[-1] + 1) * KW,
                                         qb * TW:(qb + 1) * TW]
                        .rearrange("(k p) t -> p k t", p=KW))
                    for i, kt in enumerate(run):
                        mtiles[kt] = mrun[:, i * TW:(i + 1) * TW]

                for b in range(cfg.B):
                    attn_sb = at_pool.tile([128, NHL * TW], BF16, name="at_sb")
                    qt_all = q_pool.tile([128, NHL * TW], BF16, name="q_t")
                    tb2 = b * cfg.NB + qb
                    hh = NHL // 2
                    for hp in range(2):
                        nc.sync.dma_start(
                            out=qt_all[:, hp * hh * TW:(hp + 1) * hh * TW]
                            .rearrange("p (h t) -> p h t", h=hh),
                            in_=qT_dram[hp * hh * HD:(hp + 1) * hh * HD,
                                        tb2 * TW:(tb2 + 1) * TW]
                            .rearrange("(h p) t -> p h t", p=128))
                    for h in range(NHL):
                        qt = qt_all[:, h * TW:(h + 1) * TW]
                        at_ps = av_psum.tile([HD, TW], F32, name="at_ps")
                        se_ps = se_psum.tile([128, TW], F32, name="se_ps")
                        # software pipeline: issue score matmuls LOOKAHEAD
                        # iterations ahead so the PE never waits on exp (ACT)
                        LOOKAHEAD = 2
                        n_act = len(active)
                        sc_tiles = [None] * n_act

                        def emit_sc(j):
                            kt2 = active[j]
                            gk2 = b * cfg.L + kt2 * KW
                            o = offs[kt2]
                            sc = sc_psum.tile([KW, TW], F32, name="sc_ps")
                            nc.tensor.matmul(sc[:, o:], kT_sb[:, gk2:gk2 + KW],
                                             qt[:, o:], start=True, stop=True)
                            sc_tiles[j] = sc

                        for j in range(min(LOOKAHEAD, n_act)):
                            emit_sc(j)
                        for idx, kt in enumerate(active):
                            if idx + LOOKAHEAD < n_act:
                                emit_sc(idx + LOOKAHEAD)
                            gk = b * cfg.L + kt * KW  # global key token
                            o = offs[kt]
                            sc_ps = sc_tiles[idx]
                            sc_tiles[idx] = None
                            if cls[kt][qb][0] == 'M':
                                ms = msk_pool.tile([KW, TW], F32, name="ms_t")
                                nc.vector.tensor_tensor(
                                    ms[:, o:], sc_ps[:, o:], mtiles[kt][:, o:],
                                    AluOpType.add)
                                esrc = ms[:, o:]
                            else:
                                esrc = sc_ps[:, o:]
                            ex = e_pool.tile([KW, TW], BF16, name="ex_t")
                            nc.scalar.activation(
                                ex[:, o:], esrc,
                                mybir.ActivationFunctionType.Exp,
                                scale=float(SCALE))
                            st = dict(start=(idx == 0),
                                      stop=(idx == len(active) - 1))
                            nc.tensor.matmul(se_ps[:, o:], ones_sb[:],
                                             ex[:, o:], **st)
                            nc.tensor.matmul(at_ps[:, o:], v_sb[:, gk:gk + KW],
                                             ex[:, o:], **st)
                        rec = rec_pool.tile([128, TW], F32, name="rec_t")
                        nc.vector.reciprocal_approx_fast(rec[:], se_ps[:])
                        nc.vector.tensor_tensor(
                            attn_sb[:, h * TW:(h + 1) * TW],
                            at_ps[:], rec[:], AluOpType.mult)

                    # ---- wo partial for this (b, qb) token block ----------
                    c = b * cfg.NB + qb
                    NG = min(4, cfg.NBLK)  # n-blocks per batched store
                    for m in range(TW // 128):
                        for ng in range(cfg.NBLK // NG):
                            oc = oc_pool.tile([128, NG * TW], BF16,
                                              name="oc_t")
                            for j in range(NG):
                                n = ng * NG + j
                                o_ps = o_psum.tile([128, TW], F32,
                                                   name="o_ps")
                                for h in range(NHL):
                                    nc.tensor.matmul(
                                        o_ps[:],
                                        attn_sb[:, h * TW + m * 128:
                                                h * TW + (m + 1) * 128],
                                        wo_sb[:, h * cfg.D + n * TW:
                                              h * cfg.D + (n + 1) * TW],
                                        start=(h == 0), stop=(h == NHL - 1))
                                # alternate drain engine: keep ACT free
                                dst = oc[:, j * TW:(j + 1) * TW]
                                if n % 2 == 0:
                                    nc.vector.tensor_copy(dst, o_ps[:])
                                else:
                                    nc.scalar.copy(dst, o_ps[:])
                            # SWDGE queue: feeds the collective, keeps the
                            # sync sequencer free for PE-critical loads
                            nc.gpsimd.dma_start(
                                out=rs_in[c][m * 128:(m + 1) * 128,
                                             ng * NG * TW:(ng + 1) * NG * TW],
                                in_=oc[:])
                    nc.gpsimd.collective_compute(
                        "ReduceScatter", AluOpType.add, replica_groups=rg,
                        ins=[rs_in[c][:].opt()], outs=[rs_out[c][:].opt()])

            # ---- RS shard -> external output: plain DRAM->DRAM DMA on the
            # gpsimd queue (no compute engine ever waits on a collective)
            rw = TW // N_CORES
            for c in range(cfg.NCH):
                nc.gpsimd.dma_start(
                    out=out_d.ap()[c * rw:(c + 1) * rw, :],
                    in_=rs_out[c][:])

        w_pool.release()
        dram_pool.release()
        kv_pool.release()
        const_pool.release()

    nc.compile()
    return nc


def host_prepare(cfg, x, mask, wq, wk, wv, wo):
    """Returns (in_maps, cls)."""
    x = np.ascontiguousarray(np.asarray(x, dtype=np.float32))
    mask = np.asarray(mask, dtype=np.float32)
    wq = np.asarray(wq, dtype=np.float32)
    wk = np.asarray(wk, dtype=np.float32)
    wv = np.asarray(wv, dtype=np.float32)
    wo = np.asarray(wo, dtype=np.float32)

    import ml_dtypes
    bf16 = ml_dtypes.bfloat16
    perm = _rope_perm()
    C, S = _rope_tables(cfg)
    xT = np.ascontiguousarray(x.reshape(cfg.BL, cfg.D).T).astype(bf16)
    maskT = np.ascontiguousarray(mask.T / SCALE).astype(bf16)
    cls = classify_mask(mask, cfg)

    in_maps = []
    for g in range(N_CORES):
        qrows = wq[g * NHL * HD:(g + 1) * NHL * HD]          # [512, D]
        qperm = np.concatenate(
            [qrows[h * HD + perm] for h in range(NHL)], axis=0)
        krows = wk[g * HD:(g + 1) * HD][perm]                # [128, D]
        vrows = wv[g * HD:(g + 1) * HD]                      # [128, D]
        wocols = wo[:, g * NHL * HD:(g + 1) * NHL * HD]      # [D, 512]
        in_maps.append({
            "xT": xT,
            "wqT": np.ascontiguousarray(qperm.T).astype(bf16),
            "wkT": np.ascontiguousarray(krows.T).astype(bf16),
            "wvT": np.ascontiguousarray(vrows.T).astype(bf16),
            "woT": np.ascontiguousarray(wocols.T).astype(bf16),
            "maskT": maskT,
            "ropeC": C,
            "ropeS": S,
        })
    return in_maps, cls


def assemble_output(cfg, results):
    """Stitch per-core ReduceScatter shards back into [B, L, D]."""
    full = np.empty((cfg.BL, cfg.D), dtype=np.float32)
    rw = TW // N_CORES
    for g in range(N_CORES):
        r = np.asarray(results[g]["out"]).astype(np.float32)
        for c in range(cfg.NCH):
            full[c * TW + g * rw: c * TW + (g + 1) * rw] = \
                r[c * rw:(c + 1) * rw]
    return full.reshape(cfg.B, cfg.L, cfg.D)


def kernel(x, mask, wq, wk, wv, wo):
    global LAST_RESULTS
    from concourse.bass_utils import run_bass_kernel_spmd
    cfg = Cfg(B=2, L=2048, D=4096)
    in_maps, cls = host_prepare(cfg, x, mask, wq, wk, wv, wo)
    nc = build_bass(cfg, cls)
    res = run_bass_kernel_spmd(nc, in_maps, core_ids=list(range(N_CORES)),
                               trace=TRACE)
    LAST_RESULTS = res
    return assemble_output(cfg, res.results)


# revision 49
# speedup vs baseline: 3.6094x; 1.0094x over previous
"""Distributed GQA attention kernel for one TRN2 chip (8 NeuronCores).

Sharding: tensor-parallel over heads. Core g owns query heads [4g, 4g+4)
and kv head g. Each core computes its heads' attention and a partial
output projection; a chunked ReduceScatter sums the partials and leaves
each core with a 1/8 token-slice of the final output.

All device tensors are laid out so that no on-device transposes of the
big activations are needed:
  - x is passed pre-transposed (xT [D, B*L]) so projections contract D
    on the partition axis.
  - q/k are produced directly as qT/kT [head_dim, tokens]; scores are
    computed keys-on-partitions, so the P@V matmul consumes exp(scores)
    directly and the wo matmul consumes the attention output directly.
  - RoPE head_dim pairs are permuted (on the host, into wq/wk rows) so
    each rotation partner lives 16 partitions away within a 32-partition
    quadrant -> one DVE stream_shuffle does the swap.
  - softmax denominator comes from an all-ones matmul (partition
    broadcast for free); no max subtraction (fp32 logits here are <~15).
"""

import numpy as np

import concourse.bass as bass
import concourse.mybir as mybir
import concourse.tile as tile
from concourse import bacc
from concourse.alu_op_type import AluOpType
from concourse.masks import make_identity

F32 = mybir.dt.float32
BF16 = mybir.dt.bfloat16

N_CORES = 8
NHL = 4           # local q heads per core
HD = 128          # head dim
THETA = 10000.0
SCALE = HD ** -0.5
TW = 512          # token block width (free dim of most matmuls)
KW = 128          # key tile width (partition dim of score tiles)

# module-level knobs for test.py
TRACE = False
LAST_RESULTS = None


class Cfg:
    def __init__(self, B=2, L=2048, D=4096):
        self.B, self.L, self.D = B, L, D
        self.BL = B * L
        self.DC = D // 128         # contraction chunks for projections
        self.NB = L // TW          # query blocks per batch
        self.NT = self.BL // TW    # token blocks total
        self.KT = L // KW          # key tiles per batch
        self.NBLK = D // TW        # wo output column blocks
        self.NCH = self.NT         # ReduceScatter chunks (one per token block)
        assert self.BL % TW == 0 and TW % N_CORES == 0


# stream_shuffle mask: swap 16-partition halves within each 32-partition quadrant
SWAP16 = [(i + 16) % 32 for i in range(32)]


def _rope_perm():
    """Permutation of head_dim rows: pair i=(16q + r) lives at partitions
    32q+r (x1 = even dim 2i) and 32q+16+r (x2 = odd dim 2i+1)."""
    perm = np.zeros(HD, dtype=np.int64)
    for p in range(HD):
        q, r = divmod(p, 32)
        i = 16 * q + (r % 16)
        perm[p] = 2 * i + (0 if r < 16 else 1)
    return perm


def _rope_tables(cfg):
    """cosT/sinT [128, L] in the permuted-partition layout, sin sign-folded."""
    perm = _rope_perm()
    t = np.arange(cfg.L, dtype=np.float64)
    freqs = THETA ** (-np.arange(0, HD, 2, dtype=np.float64) / HD)  # [64]
    theta = t[None, :] * freqs[:, None]                             # [64, L]
    cos, sin = np.cos(theta), np.sin(theta)
    C = np.zeros((HD, cfg.L), dtype=np.float32)
    S = np.zeros((HD, cfg.L), dtype=np.float32)
    for p in range(HD):
        q, r = divmod(p, 32)
        i = 16 * q + (r % 16)
        C[p] = cos[i]
        S[p] = sin[i] if r >= 16 else -sin[i]
    return C, S


def classify_mask(mask, cfg):
    """cls[kt][qb] = (kind, off): kind in {'Z','N','M'} for tile
    mask[qb*TW:(qb+1)*TW, kt*KW:(kt+1)*KW]; off = count of leading query
    columns in the tile that are fully masked (safe to skip: exp would
    be exactly 0 there)."""
    cls = [[None] * cfg.NB for _ in range(cfg.KT)]
    for kt in range(cfg.KT):
        for qb in range(cfg.NB):
            t = mask[qb * TW:(qb + 1) * TW, kt * KW:(kt + 1) * KW]
            if np.all(t == 0.0):
                cls[kt][qb] = ('Z', 0)
            elif np.all(t <= -1e8):
                cls[kt][qb] = ('N', 0)
            else:
                dead_q = np.all(t <= -1e8, axis=1)  # [TW]
                off = 0
                while off < len(dead_q) and dead_q[off]:
                    off += 1
                off = (off // 64) * 64  # keep offsets 64-aligned
                cls[kt][qb] = ('M', off)
    # guard: every query block must attend to at least one key tile
    for qb in range(cfg.NB):
        assert any(cls[kt][qb][0] != 'N' for kt in range(cfg.KT)), \
            "fully-masked query block unsupported"
    return cls


def build_bass(cfg, cls):
    nc = bacc.Bacc("TRN2", target_bir_lowering=False, debug=False,
                   num_devices=N_CORES, num_swdge_queues=4)

    xT_d = nc.dram_tensor("xT", [cfg.D, cfg.BL], BF16, kind="ExternalInput")
    wqT_d = nc.dram_tensor("wqT", [cfg.D, NHL * HD], BF16, kind="ExternalInput")
    wkT_d = nc.dram_tensor("wkT", [cfg.D, HD], BF16, kind="ExternalInput")
    wvT_d = nc.dram_tensor("wvT", [cfg.D, HD], BF16, kind="ExternalInput")
    woT_d = nc.dram_tensor("woT", [NHL * HD, cfg.D], BF16, kind="ExternalInput")
    maskT_d = nc.dram_tensor("maskT", [cfg.L, cfg.L], BF16, kind="ExternalInput")
    ropeC_d = nc.dram_tensor("ropeC", [HD, cfg.L], F32, kind="ExternalInput")
    ropeS_d = nc.dram_tensor("ropeS", [HD, cfg.L], F32, kind="ExternalInput")
    out_d = nc.dram_tensor("out", [cfg.BL // N_CORES, cfg.D], BF16,
                           kind="ExternalOutput")

    rg = [list(range(N_CORES))]
    QD = NHL * HD  # 512

    with tile.TileContext(nc) as tc:
        # ---- constants / tables -------------------------------------------
        const_pool = tc.alloc_tile_pool(name="const", bufs=1)
        ones_sb = const_pool.tile([128, 128], BF16, name="ones_sb")
        nc.vector.memset(ones_sb[:], 1.0)
        ident = const_pool.tile([128, 128], BF16, name="ident")
        make_identity(nc, ident[:])

        # ---- resident activations -----------------------------------------
        kv_pool = tc.alloc_tile_pool(name="kv", bufs=1)
        kT_sb = kv_pool.tile([HD, cfg.BL], BF16, name="kT_sb")
        v_sb = kv_pool.tile([128, cfg.BL], BF16, name="v_sb")

        # DRAM scratch
        dram_pool = tc.alloc_tile_pool(name="dram", bufs=1, space="DRAM")
        qT_dram = dram_pool.tile([QD, cfg.BL], BF16, name="qT_dram")
        rs_in = [dram_pool.tile([TW, cfg.D], BF16, name=f"rs_in{c}")
                 for c in range(cfg.NCH)]
        rs_out = [dram_pool.tile([TW // N_CORES, cfg.D], BF16,
                                 name=f"rs_out{c}")
                  for c in range(cfg.NCH)]

        # ---- all weights resident upfront (bf16 halves the footprint) -----
        # Few, ~0.5-1MB DMAs (sequencer dispatch is ~0.6us each); k/v first
        # so the first tokblock's matmuls can start ASAP; wo on the scalar
        # queue so it never delays the x stream.
        w_pool = tc.alloc_tile_pool(name="weights", bufs=1)
        wq_sb = w_pool.tile([128, cfg.DC * QD], BF16, name="wq_sb")
        wk_sb = w_pool.tile([128, cfg.DC * HD], BF16, name="wk_sb")
        wv_sb = w_pool.tile([128, cfg.DC * HD], BF16, name="wv_sb")
        wo_sb = w_pool.tile([128, NHL * cfg.D], BF16, name="wo_sb")

        def load_w3d(eng, dst, src_d, width, chunk):
            """dst[:, dc*width+c] = src[dc*128+p, c], batched `chunk` dcs/DMA."""
            for d0 in range(0, cfg.DC, chunk):
                d1 = min(d0 + chunk, cfg.DC)
                eng.dma_start(
                    out=dst[:, d0 * width:d1 * width]
                    .rearrange("p (dc c) -> p dc c", dc=d1 - d0),
                    in_=src_d.ap()[d0 * 128:d1 * 128, :]
                    .rearrange("(dc p) c -> p dc c", p=128))

        load_w3d(nc.sync, wk_sb, wkT_d, HD, 8)     # 4 DMAs of 512KB
        load_w3d(nc.sync, wv_sb, wvT_d, HD, 8)
        load_w3d(nc.scalar, wq_sb, wqT_d, QD, 4)   # 8 DMAs of 1MB (ACT queue)
        for h in range(NHL):                       # 4 DMAs of 1MB (ACT queue)
            nc.scalar.dma_start(out=wo_sb[:, h * cfg.D:(h + 1) * cfg.D],
                                in_=woT_d.ap()[h * HD:(h + 1) * HD, :])

        # ================= phase 1: QKV projections + RoPE =================
        with tc.tile_pool(name="xload", bufs=cfg.DC // 4 + 2) as x_pool, \
             tc.tile_pool(name="qpsum", bufs=1, space="PSUM") as q_psum, \
             tc.tile_pool(name="kpsum", bufs=2, space="PSUM") as k_psum, \
             tc.tile_pool(name="vpsum", bufs=1, space="PSUM") as v_psum, \
             tc.tile_pool(name="ropetmp", bufs=3) as rtmp_pool, \
             tc.tile_pool(name="qrot", bufs=3) as qrot_pool, \
             tc.tile_pool(name="vstage", bufs=2) as vst_pool, \
             tc.tile_pool(name="vtpsum", bufs=1, space="PSUM") as vt_psum, \
             tc.tile_pool(name="ropetbl", bufs=1) as rtbl_pool:

            ropeC = rtbl_pool.tile([HD, cfg.L], F32, name="ropeC_sb")
            ropeS = rtbl_pool.tile([HD, cfg.L], F32, name="ropeS_sb")
            nc.scalar.dma_start(out=ropeC[:], in_=ropeC_d.ap())
            nc.scalar.dma_start(out=ropeS[:], in_=ropeS_d.ap())

            def rope_drain(ps, dst):
                """dst = ps*C + shuffle16(ps)*S at token offset t0 (len TW)."""
                sw = rtmp_pool.tile([128, TW], F32, name="rope_sw")
                t1 = rtmp_pool.tile([128, TW], F32, name="rope_t1")
                t2 = rtmp_pool.tile([128, TW], F32, name="rope_t2")
                nc.vector.stream_shuffle(sw[:], ps, SWAP16)
                nc.vector.tensor_tensor(t1[:], sw[:], Sx, AluOpType.mult)
                nc.vector.tensor_tensor(t2[:], ps, Cx, AluOpType.mult)
                nc.vector.tensor_tensor(dst, t1[:], t2[:], AluOpType.add)

            for tb in range(cfg.NT):
                t0 = (tb % cfg.NB) * TW  # position within batch
                Cx = ropeC[:, t0:t0 + TW]
                Sx = ropeS[:, t0:t0 + TW]

                q_ps = q_psum.tile([128, NHL * TW], F32, name="q_ps")
                k_ps = k_psum.tile([128, TW], F32, name="k_ps")
                vT_ps = v_psum.tile([128, TW], F32, name="vT_ps")
                # keep the whole tokblock's xT resident so k/vT and q run as
                # two dense passes; next tokblock's k/vT pass overlaps this
                # block's RoPE drains instead of stalling on them
                xts = []
                XB = 4  # dc-chunks per DMA (~512KB each)
                for dc in range(0, cfg.DC, XB):
                    d1 = min(dc + XB, cfg.DC)
                    xt = x_pool.tile([128, (d1 - dc) * TW], BF16, name="x_t")
                    nc.sync.dma_start(
                        out=xt[:].rearrange("p (dc t) -> p dc t", dc=d1 - dc),
                        in_=xT_d.ap()[dc * 128:d1 * 128,
                                      tb * TW:(tb + 1) * TW]
                        .rearrange("(dc p) t -> p dc t", p=128))
                    for j in range(d1 - dc):
                        xts.append(xt[:, j * TW:(j + 1) * TW])
                for dc in range(cfg.DC):
                    st = dict(start=(dc == 0), stop=(dc == cfg.DC - 1))
                    nc.tensor.matmul(k_ps[:],
                                     wk_sb[:, dc * HD:(dc + 1) * HD],
                                     xts[dc], **st)
                    nc.tensor.matmul(vT_ps[:],
                                     wv_sb[:, dc * HD:(dc + 1) * HD],
                                     xts[dc], **st)
                for dc in range(cfg.DC):
                    st = dict(start=(dc == 0), stop=(dc == cfg.DC - 1))
                    for h in range(NHL):
                        nc.tensor.matmul(
                            q_ps[:, h * TW:h * TW + TW],
                            wq_sb[:, dc * QD + h * HD: dc * QD + (h + 1) * HD],
                            xts[dc], **st)

                # k: rope -> resident (drain first: next tb needs this bank)
                rope_drain(k_ps[:], kT_sb[:, tb * TW:(tb + 1) * TW])
                # q: rope -> spill to DRAM (one batched DMA per tokblock)
                qr = qrot_pool.tile([128, NHL * TW], BF16, name="q_rot")
                for h in range(NHL):
                    rope_drain(q_ps[:, h * TW:h * TW + TW],
                               qr[:, h * TW:(h + 1) * TW])
                nc.sync.dma_start(
                    out=qT_dram[0:QD, tb * TW:(tb + 1) * TW]
                    .rearrange("(h p) t -> p h t", p=128),
                    in_=qr[:].rearrange("p (h t) -> p h t", h=NHL))
                # v: vT -> transpose -> resident [ktok, hd] blocks
                vt_sb = vst_pool.tile([128, TW], BF16, name="vT_stage")
                nc.scalar.copy(vt_sb[:], vT_ps[:])
                for i in range(TW // 128):
                    vp = vt_psum.tile([128, 128], BF16, name="v_tr_ps")
                    nc.tensor.transpose(vp[:], vt_sb[:, i * 128:(i + 1) * 128],
                                        ident[:])
                    nc.scalar.copy(
                        v_sb[:, tb * TW + i * 128: tb * TW + (i + 1) * 128],
                        vp[:])

        # ================= phase 2: attention + wo + ReduceScatter =========
        with tc.tile_pool(name="mask", bufs=5) as m_pool, \
             tc.tile_pool(name="qload", bufs=3) as q_pool, \
             tc.tile_pool(name="expsb", bufs=4) as e_pool, \
             tc.tile_pool(name="msum", bufs=3) as msk_pool, \
             tc.tile_pool(name="attnsb", bufs=2) as at_pool, \
             tc.tile_pool(name="recsb", bufs=2) as rec_pool, \
             tc.tile_pool(name="outcp", bufs=4) as oc_pool, \
             tc.tile_pool(name="scps", bufs=3, space="PSUM") as sc_psum, \
             tc.tile_pool(name="avps", bufs=2, space="PSUM") as av_psum, \
             tc.tile_pool(name="seps", bufs=1, space="PSUM") as se_psum, \
             tc.tile_pool(name="ops", bufs=2, space="PSUM") as o_psum:

            for qb in range(cfg.NB):
                active = [kt for kt in range(cfg.KT)
                          if cls[kt][qb][0] != 'N']
                # off[kt]: leading fully-masked query columns -> skip them.
                # Force 0 on the first active tile so PSUM start=True
                # initializes every column.
                offs = {kt: cls[kt][qb][1] for kt in active}
                offs[active[0]] = 0
                # batch mask loads: one DMA per contiguous run of M tiles
                mkts = [kt for kt in active if cls[kt][qb][0] == 'M']
                runs = []
                for kt in mkts:
                    if runs and kt == runs[-1][-1] + 1 and len(runs[-1]) < 4:
                        runs[-1].append(kt)
                    else:
                        runs.append([kt])
                mtiles = {}
                for run in runs:
                    nk = len(run)
                    mrun = m_pool.tile([KW, nk * TW], BF16, name="m_t")
                    nc.sync.dma_start(
                        out=mrun[:].rearrange("p (k t) -> p k t", k=nk),
                        in_=maskT_d.ap()[run[0] * KW:(run[-1] + 1) * KW,
                                         qb * TW:(qb + 1) * TW]
                        .rearrange("(k p) t -> p k t", p=KW))
                    for i, kt in enumerate(run):
                        mtiles[kt] = mrun[:, i * TW:(i + 1) * TW]

                for b in range(cfg.B):
                    attn_sb = at_pool.tile([128, NHL * TW], BF16, name="at_sb")
                    qt_all = q_pool.tile([128, NHL * TW], BF16, name="q_t")
                    tb2 = b * cfg.NB + qb
                    hh = NHL // 2
                    for hp in range(2):
                        nc.sync.dma_start(
                            out=qt_all[:, hp * hh * TW:(hp + 1) * hh * TW]
                            .rearrange("p (h t) -> p h t", h=hh),
                            in_=qT_dram[hp * hh * HD:(hp + 1) * hh * HD,
                                        tb2 * TW:(tb2 + 1) * TW]
                            .rearrange("(h p) t -> p h t", p=128))
                    for h in range(NHL):
                        qt = qt_all[:, h * TW:(h + 1) * TW]
                        at_ps = av_psum.tile([HD, TW], F32, name="at_ps")
                        se_ps = se_psum.tile([128, TW], F32, name="se_ps")
                        # software pipeline: issue score matmuls LOOKAHEAD
                        # iterations ahead so the PE never waits on exp (ACT)
                        LOOKAHEAD = 2
                        n_act = len(active)
                        sc_tiles = [None] * n_act

                        def emit_sc(j):
                            kt2 = active[j]
                            gk2 = b * cfg.L + kt2 * KW
                            o = offs[kt2]
                            sc = sc_psum.tile([KW, TW], F32, name="sc_ps")
                            nc.tensor.matmul(sc[:, o:], kT_sb[:, gk2:gk2 + KW],
                                             qt[:, o:], start=True, stop=True)
                            sc_tiles[j] = sc

                        for j in range(min(LOOKAHEAD, n_act)):
                            emit_sc(j)
                        for idx, kt in enumerate(active):
                            if idx + LOOKAHEAD < n_act:
                                emit_sc(idx + LOOKAHEAD)
                            gk = b * cfg.L + kt * KW  # global key token
                            o = offs[kt]
                            sc_ps = sc_tiles[idx]
                            sc_tiles[idx] = None
                            if cls[kt][qb][0] == 'M':
                                ms = msk_pool.tile([KW, TW], F32, name="ms_t")
                                nc.vector.tensor_tensor(
                                    ms[:, o:], sc_ps[:, o:], mtiles[kt][:, o:],
                                    AluOpType.add)
                                esrc = ms[:, o:]
                            else:
                                esrc = sc_ps[:, o:]
                            ex = e_pool.tile([KW, TW], BF16, name="ex_t")
                            nc.scalar.activation(
                                ex[:, o:], esrc,
                                mybir.ActivationFunctionType.Exp,
                                scale=float(SCALE))
                            st = dict(start=(idx == 0),
                                      stop=(idx == len(active) - 1))
                            nc.tensor.matmul(se_ps[:, o:], ones_sb[:],
                                             ex[:, o:], **st)
                            nc.tensor.matmul(at_ps[:, o:], v_sb[:, gk:gk + KW],
                                             ex[:, o:], **st)
                        rec = rec_pool.tile([128, TW], F32, name="rec_t")
                        nc.vector.reciprocal_approx_fast(rec[:], se_ps[:])
                        nc.vector.tensor_tensor(
                            attn_sb[:, h * TW:(h + 1) * TW],
                            at_ps[:], rec[:], AluOpType.mult)

                    # ---- wo partial for this (b, qb) token block ----------
                    c = b * cfg.NB + qb
                    NG = min(4, cfg.NBLK)  # n-blocks per batched store
                    for m in range(TW // 128):
                        for ng in range(cfg.NBLK // NG):
                            oc = oc_pool.tile([128, NG * TW], BF16,
                                              name="oc_t")
                            for j in range(NG):
                                n = ng * NG + j
                                o_ps = o_psum.tile([128, TW], F32,
                                                   name="o_ps")
                                for h in range(NHL):
                                    nc.tensor.matmul(
                                        o_ps[:],
                                        attn_sb[:, h * TW + m * 128:
                                                h * TW + (m + 1) * 128],
                                        wo_sb[:, h * cfg.D + n * TW:
                                              h * cfg.D + (n + 1) * TW],
                                        start=(h == 0), stop=(h == NHL - 1))
                                # alternate drain engine: keep ACT free
                                dst = oc[:, j * TW:(j + 1) * TW]
                                if n % 2 == 0:
                                    nc.vector.tensor_copy(dst, o_ps[:])
                                else:
                                    nc.scalar.copy(dst, o_ps[:])
                            # SWDGE queue: feeds the collective, keeps the
                            # sync sequencer free for PE-critical loads
                            nc.gpsimd.dma_start(
                                out=rs_in[c][m * 128:(m + 1) * 128,
                                             ng * NG * TW:(ng + 1) * NG * TW],
                                in_=oc[:])
                    nc.gpsimd.collective_compute(
                        "ReduceScatter", AluOpType.add, replica_groups=rg,
                        ins=[rs_in[c][:].opt()], outs=[rs_out[c][:].opt()])

            # ---- RS shard -> external output: plain DRAM->DRAM DMA on the
            # gpsimd queue (no compute engine ever waits on a collective)
            rw = TW // N_CORES
            for c in range(cfg.NCH):
                nc.gpsimd.dma_start(
                    out=out_d.ap()[c * rw:(c + 1) * rw, :],
                    in_=rs_out[c][:])

        w_pool.release()
        dram_pool.release()
        kv_pool.release()
        const_pool.release()

    nc.compile()
    return nc


def host_prepare(cfg, x, mask, wq, wk, wv, wo):
    """Returns (in_maps, cls)."""
    x = np.ascontiguousarray(np.asarray(x, dtype=np.float32))
    mask = np.asarray(mask, dtype=np.float32)
    wq = np.asarray(wq, dtype=np.float32)
    wk = np.asarray(wk, dtype=np.float32)
    wv = np.asarray(wv, dtype=np.float32)
    wo = np.asarray(wo, dtype=np.float32)

    import ml_dtypes
    bf16 = ml_dtypes.bfloat16
    perm = _rope_perm()
    C, S = _rope_tables(cfg)
    xT = np.ascontiguousarray(x.reshape(cfg.BL, cfg.D).T).astype(bf16)
    maskT = np.ascontiguousarray(mask.T / SCALE).astype(bf16)
    cls = classify_mask(mask, cfg)

    in_maps = []
    for g in range(N_CORES):
        qrows = wq[g * NHL * HD:(g + 1) * NHL * HD]          # [512, D]
        qperm = np.concatenate(
            [qrows[h * HD + perm] for h in range(NHL)], axis=0)
        krows = wk[g * HD:(g + 1) * HD][perm]                # [128, D]
        vrows = wv[g * HD:(g + 1) * HD]                      # [128, D]
        wocols = wo[:, g * NHL * HD:(g + 1) * NHL * HD]      # [D, 512]
        in_maps.append({
            "xT": xT,
            "wqT": np.ascontiguousarray(qperm.T).astype(bf16),
            "wkT": np.ascontiguousarray(krows.T).astype(bf16),
            "wvT": np.ascontiguousarray(vrows.T).astype(bf16),
            "woT": np.ascontiguousarray(wocols.T).astype(bf16),
            "maskT": maskT,
            "ropeC": C,
            "ropeS": S,
        })
    return in_maps, cls


def assemble_output(cfg, results):
    """Stitch per-core ReduceScatter shards back into [B, L, D]."""
    full = np.empty((cfg.BL, cfg.D), dtype=np.float32)
    rw = TW // N_CORES
    for g in range(N_CORES):
        r = np.asarray(results[g]["out"]).astype(np.float32)
        for c in range(cfg.NCH):
            full[c * TW + g * rw: c * TW + (g + 1) * rw] = \
                r[c * rw:(c + 1) * rw]
    return full.reshape(cfg.B, cfg.L, cfg.D)


def kernel(x, mask, wq, wk, wv, wo):
    global LAST_RESULTS
    from concourse.bass_utils import run_bass_kernel_spmd
    cfg = Cfg(B=2, L=2048, D=4096)
    in_maps, cls = host_prepare(cfg, x, mask, wq, wk, wv, wo)
    nc = build_bass(cfg, cls)
    res = run_bass_kernel_spmd(nc, in_maps, core_ids=list(range(N_CORES)),
                               trace=TRACE)
    LAST_RESULTS = res
    return assemble_output(cfg, res.results)
